# revision 21
# baseline (speedup 1.0000x reference)
"""Trainium2 Bass kernel for nn_AttentionBlock_33724083208839 (sparse_attention).

Data-parallel over batch (8 batches -> 8 cores). Per core:
  1. chunked x load (all DMAs issued upfront), PE transpose -> xT f32;
     K projected in exact f32 (feeds selection), Q in f32; both also copied
     to bf16 (kTb via ACT, qTb via DVE) for the attention matmuls; V in f32
     -> bf16 [V|1] tiles.
  2. K_reduce via the exact CVaR identity sum_top_l = l*t + sum(relu(x-t)),
     t from Gaussian quantile + one Newton step on the exact count (f32 DVE).
  3. query selection: sqk = x @ (Wq @ K_reduce) exactly on PE; threshold =
     LQ-th largest of sqk via two full 128-ary counting passes on a
     partition-replicated copy, then sparse_gather (GPSIMD) compacts the
     ~10 in-interval values (num_found-masked: HW leaves garbage pads) and
     three cheap passes on the compacted set finish to f32 resolution.
  4. attention for all 4096 queries, 512-query slabs with a 1-slab lag and
     triple-buffered P^T tiles (so scores of slab s never wait on AV of
     slab s-2 releasing a buffer); x is loaded through a 16-chunk rolling
     window:
     scores^T on PE (bf16, 2-ktile PSUM strips) -> exp split between ACT
     (exact, scale=1/8, own 2-slot strip ring) and DVE (Schraudolph bitcast
     exp int16(A*s+B) -> bf16, own strip ring) -> P^T bf16 -> reversed AV:
     lhsT = P^T tile (stationary), rhs = [V|1] (65 moving cols) accumulated
     over 32 k-tiles, interleaved into the score-group stream, so outputs
     land directly in [query-partition, dv] layout; normalize by the
     ones-column denominator (DVE), blend non-selected rows to meanV
     (copy_predicated), DMA out per 128-query chunk.
"""
import os
import sys

sys.path.insert(0, "/opt/trn_rl_repo")

KSKIP = set(os.environ.get("KSKIP", "").split(","))

import math
from statistics import NormalDist

import numpy as np

import concourse.bacc as bacc
import concourse.bass as bass
import concourse.bass_isa as bass_isa
import concourse.mybir as mybir
from concourse.tile import TileContext
from concourse.masks import make_identity
from concourse.bass_utils import run_bass_kernel_spmd

B, L, D = 8, 4096, 64
LQ = int((1.0 - 0.33) * L)  # 2744
PART = 128
NT = L // PART
NS = L // 512
N_CORES = 8

QFRAC = 1.0 - LQ / L
Z = NormalDist().inv_cdf(QFRAC)
PHI = math.exp(-Z * Z / 2.0) / math.sqrt(2.0 * math.pi)

f32 = mybir.dt.float32
f32r = mybir.dt.float32r
bf16 = mybir.dt.bfloat16
u8 = mybir.dt.uint8
i16 = mybir.dt.int16
i32 = mybir.dt.int32
AF = mybir.ActivationFunctionType
OP = mybir.AluOpType

N_PASS = 5
BOUND = 512.0

# Schraudolph exp for bf16 bit patterns: bf16_bits(exp(s/8)) ~= A*s + B.
# A = 128*log2(e)/8; B centers the piecewise-linear sawtooth (mean-unbiased)
# and adds +0.5 to compensate truncation in the float->int16 convert.
SCH_A = 128.0 * math.log2(math.e) / 8.0
SCH_B = 16256.0 + 0.5 - 128.0 * math.log2(1.0407)

GROUPS = [(g, 2) for g in range(0, NT, 2)]

# exp-engine split: selection runs on GPSIMD, so the DVE takes a fixed share
# of the exp strips (Schraudolph), spread through the slab so both engines
# drain the strip ring concurrently.  Pool (gpsimd) also takes a share.
DVE_GROUPS = {2, 4, 6, 9, 11, 13, 15}
POOL_GROUPS = set()
DVE_FROM_SLAB = 1
DVE_GROUPS_S0 = set()
POOL_GROUPS_S0 = set()


def build(debug: bool = False):
    nc = bacc.Bacc("TRN2")
    x = nc.dram_tensor("x", [L, D], f32, kind="ExternalInput")
    wq = nc.dram_tensor("Wq", [D, D], f32, kind="ExternalInput")
    wk = nc.dram_tensor("Wk", [D, D], f32, kind="ExternalInput")
    wv = nc.dram_tensor("Wv", [D, D], f32, kind="ExternalInput")
    out = nc.dram_tensor("out", [L, D], f32, kind="ExternalOutput")
    scr_row = nc.dram_tensor("scr_row", [1, L], f32, kind="Internal")
    scr_cmp = nc.dram_tensor("scr_cmp", [1, 64], f32, kind="Internal")
    scr_chi = nc.dram_tensor("scr_chi", [1, 16], f32, kind="Internal")
    dbg = {}
    if debug:
        for name, shape in [
            ("dbg_kr", [D, 1]), ("dbg_thr", [PART, 1]), ("dbg_sqk", [PART, NT]),
            ("dbg_mask", [PART, NT]), ("dbg_tk", [PART, 1]), ("dbg_cnt", [PART, 1]),
            ("dbg_sel", [PART, 8]), ("dbg_valrep", [PART, 64]),
        ]:
            dbg[name] = nc.dram_tensor(name, shape, f32, kind="ExternalOutput")

    x_re = x[:].rearrange("(c p) d -> p c d", p=PART)
    out_re = out[:].rearrange("(c p) d -> p c d", p=PART)

    with TileContext(nc) as tc, \
         tc.tile_pool(name="cst", bufs=1) as cst, \
         tc.tile_pool(name="big", bufs=1) as big, \
         tc.tile_pool(name="sc", bufs=1) as sc, \
         tc.tile_pool(name="mn", bufs=2) as mn:

        # ---- warm the exp activation table immediately ----
        warm = cst.tile([1, 8], f32)
        nc.vector.memset(warm[:], 0.0)
        warm2 = cst.tile([1, 8], f32)
        nc.scalar.activation(out=warm2[:], in_=warm[:], func=AF.Exp)

        # ---- constants ----
        ident = cst.tile([PART, PART], f32)
        make_identity(nc, ident[:])
        onesb = cst.tile([PART, 1], bf16)
        nc.vector.memset(onesb[:], 1.0)
        ones1x128 = cst.tile([1, PART], f32)
        nc.vector.memset(ones1x128[:], 1.0)
        iotc_i = cst.tile([16, 4], i32)
        nc.gpsimd.iota(iotc_i[:], pattern=[[16, 4]], base=0, channel_multiplier=1)
        iotc = cst.tile([16, 4], f32)
        nc.vector.tensor_copy(iotc[:], iotc_i[:])
        pidx1i = cst.tile([PART, 1], i32)
        nc.gpsimd.iota(pidx1i[:], pattern=[[1, 1]], base=1, channel_multiplier=1)
        pidx1 = cst.tile([PART, 1], f32)
        nc.vector.tensor_copy(pidx1[:], pidx1i[:])

        # ---- persistent tensors ----
        x_sb = big.tile([PART, 16, D], f32)
        xT32 = big.tile([D, L], f32)
        xTb = big.tile([D, L], bf16)
        qTb = big.tile([D, L], bf16)
        kT32 = big.tile([D, L], f32)
        kTb = big.tile([D, L], bf16)
        vp = big.tile([PART, NT, D + 1], bf16)
        pt_a = big.tile([PART, NT, 512], bf16)
        pt_b = big.tile([PART, NT, 512], bf16)
        pt_c = big.tile([PART, NT, 512], bf16)
        res = big.tile([PART, NT, D], f32)
        mvf = big.tile([PART, D], f32)
        mask = big.tile([PART, NT], f32)
        inv_u8 = big.tile([PART, NT], u8)
        sqk = big.tile([PART, NT], f32)
        kr = big.tile([D, 1], f32)
        wvec = big.tile([D, 1], f32)
        sqk_rep = big.tile([PART, L], f32)
        cmp_rep = big.tile([PART, L], bf16)
        sqk16 = big.tile([16, 256], f32)
        valrep = big.tile([PART, 64], f32)
        cmpc = big.tile([PART, 64], bf16)

        # weights
        wq_s = cst.tile([D, D], f32)
        wk_s = cst.tile([D, D], f32)
        wv_s = cst.tile([D, D], f32)
        nc.sync.dma_start(out=wq_s[:], in_=wq[:])
        nc.sync.dma_start(out=wk_s[:], in_=wk[:])
        nc.sync.dma_start(out=wv_s[:], in_=wv[:])
        # bf16 copies of Wq/Wv for the bf16 Q/V projections (Pool: it's idle)
        wq_b = cst.tile([D, D], bf16)
        wv_b = cst.tile([D, D], bf16)
        nc.gpsimd.tensor_copy(wq_b[:], wq_s[:])
        nc.gpsimd.tensor_copy(wv_b[:], wv_s[:])

        # =============== phase 1: load / project / slab-0 scores+exp ===============
        with tc.tile_pool(name="ps_xv", bufs=2, space="PSUM") as ps_xv, \
             tc.tile_pool(name="ps_pj", bufs=2, space="PSUM") as ps_pj, \
             tc.tile_pool(name="ps_s0", bufs=2, space="PSUM") as ps_s0:

            def load_tiles(c0, c1):
                for c in range(c0, c1):
                    pxt = ps_xv.tile([PART, PART], f32, tag="xv")
                    nc.tensor.transpose(out=pxt[0:D, :], in_=x_sb[:, c % 16, :],
                                        identity=ident[:])
                    nc.vector.tensor_copy(xT32[:, PART * c:PART * (c + 1)], pxt[0:D, :])
                    nc.scalar.copy(xTb[:, PART * c:PART * (c + 1)], pxt[0:D, :])
                if c0 + 16 < NT:
                    m = c0 % 16
                    nc.sync.dma_start(out=x_sb[:, m:m + (c1 - c0), :],
                                      in_=x_re[:, c0 + 16:c1 + 16, :])

            def proj_slab(s):
                sl = slice(512 * s, 512 * (s + 1))
                pk = ps_pj.tile([D, 512], f32, tag="pj")
                for h in range(2):
                    hs = slice(512 * s + 256 * h, 512 * s + 256 * (h + 1))
                    nc.tensor.matmul(out=pk[:, 256 * h:256 * (h + 1)], lhsT=wk_s[:],
                                     rhs=xT32[:, hs], start=True, stop=True)
                    nc.vector.tensor_copy(kT32[:, hs], pk[:, 256 * h:256 * (h + 1)])
                    nc.scalar.copy(kTb[:, hs], pk[:, 256 * h:256 * (h + 1)])
                pq = ps_pj.tile([D, 512], f32, tag="pj")
                nc.tensor.matmul(out=pq[:], lhsT=wq_b[:], rhs=xTb[:, sl],
                                 start=True, stop=True)
                nc.scalar.copy(qTb[:, sl], pq[:])

            def sg0(gi):
                g0, glen = GROUPS[gi]
                strip = ps_s0.tile([PART, 2, 512], f32, tag="s0")
                for i in range(glen):
                    j = g0 + i
                    nc.tensor.matmul(out=strip[:, i, :],
                                     lhsT=kTb[:, PART * j:PART * (j + 1)],
                                     rhs=qTb[:, 0:512], start=True, stop=True)
                if gi in DVE_GROUPS_S0:  # slab 0
                    nc.vector.tensor_scalar(
                        out=pt_a[:, g0:g0 + glen, :].bitcast(i16),
                        in0=strip[:, 0:glen, :], scalar1=SCH_A, scalar2=SCH_B,
                        op0=OP.mult, op1=OP.add)
                elif gi in POOL_GROUPS_S0:
                    nc.gpsimd.tensor_scalar(
                        out=pt_a[:, g0:g0 + glen, :].bitcast(i16),
                        in0=strip[:, 0:glen, :], scalar1=SCH_A, scalar2=SCH_B,
                        op0=OP.mult, op1=OP.add)
                else:
                    nc.scalar.activation(out=pt_a[:, g0:g0 + glen, :],
                                         in_=strip[:, 0:glen, :], func=AF.Exp, scale=0.125)

            def proj_v(c0, c1):
                for c in range(c0, c1):
                    pv = ps_xv.tile([PART, PART], f32, tag="xv")
                    nc.tensor.matmul(out=pv[:, 0:D],
                                     lhsT=xTb[:, PART * c:PART * (c + 1)],
                                     rhs=wv_b[:], start=True, stop=True)
                    nc.vector.tensor_copy(vp[:, c, 0:D], pv[:, 0:D])

            for c0 in range(0, 16, 4):
                nc.sync.dma_start(out=x_sb[:, c0:c0 + 4, :], in_=x_re[:, c0:c0 + 4, :])
            load_tiles(0, 2)
            load_tiles(2, 4)
            proj_slab(0)
            proj_v(0, 4)
            sg0(0); sg0(1)
            load_tiles(4, 8)
            proj_slab(1)
            proj_v(4, 8)
            sg0(2); sg0(3)
            load_tiles(8, 12)
            load_tiles(12, 16)
            proj_slab(2); proj_v(8, 12); sg0(4); sg0(5)
            proj_slab(3); proj_v(12, 16); sg0(6); sg0(7)
            load_tiles(16, 20)
            load_tiles(20, 24)
            proj_slab(4); proj_v(16, 20); sg0(8); sg0(9)
            proj_slab(5); proj_v(20, 24); sg0(10); sg0(11)
            load_tiles(24, 28)
            load_tiles(28, 32)
            proj_slab(6); proj_v(24, 28); sg0(12); sg0(13)
            proj_slab(7); proj_v(28, 32)
            nc.vector.memset(vp[:, :, D:D + 1], 1.0)
            sg0(14)
            sg0(15)

        kst = {}

        def emit_kred_a():
            kst['bstats'] = sc.tile([D, 8, 6], f32, tag="bstats", name="bstats")
            for a in range(8):
                nc.vector.bn_stats(kst['bstats'][:, a, :], kT32[:, 512 * a:512 * (a + 1)])
            kst['aggr'] = sc.tile([D, 2], f32, tag="aggr", name="aggr")
            nc.vector.bn_aggr(kst['aggr'][:], kst['bstats'][:])
            kst['sig'] = sc.tile([D, 1], f32, tag="sig", name="sig")
            nc.vector.memset(kst['sig'][:], 1.0)
            for _ in range(4):
                kst['rs'] = sc.tile([D, 1], f32, tag="rs", name="rs")
                nc.vector.reciprocal(kst['rs'][:], kst['sig'][:])
                nc.vector.tensor_tensor(out=kst['rs'][:], in0=kst['rs'][:], in1=kst['aggr'][:, 1:2], op=OP.mult)
                nc.vector.tensor_tensor(out=kst['rs'][:], in0=kst['rs'][:], in1=kst['sig'][:], op=OP.add)
                nc.vector.tensor_scalar_mul(kst['sig'][:], kst['rs'][:], 0.5)
            kst['tk'] = sc.tile([D, 1], f32, tag="tk", name="tk")
            nc.vector.tensor_scalar(out=kst['tk'][:], in0=kst['sig'][:], scalar1=float(Z),
                                    scalar2=None, op0=OP.mult)
            nc.vector.tensor_tensor(out=kst['tk'][:], in0=kst['tk'][:], in1=kst['aggr'][:, 0:1], op=OP.add)
            kst['cnt_c'] = sc.tile([D, 1], f32, tag="cnt_c", name="cnt_c")
            kst['cnt_p'] = sc.tile([D, 1], f32, tag="cnt_p", name="cnt_p")
            HL = L // 2
            nc.vector.tensor_scalar(out=sqk_rep[0:D, 0:HL], in0=kT32[:, 0:HL],
                                    scalar1=kst['tk'][:, 0:1],
                                    scalar2=None, op0=OP.is_gt, op1=OP.add,
                                    accum_out=kst['cnt_c'][:])
            nc.gpsimd.tensor_scalar(out=sqk_rep[0:D, HL:L], in0=kT32[:, HL:L],
                                    scalar1=kst['tk'][:, 0:1],
                                    scalar2=None, op0=OP.is_gt, op1=OP.add,
                                    accum_out=kst['cnt_p'][:])
            nc.vector.tensor_tensor(out=kst['cnt_c'][:], in0=kst['cnt_c'][:],
                                    in1=kst['cnt_p'][:], op=OP.add)
            kst['adj'] = sc.tile([D, 1], f32, tag="adj", name="adj")
            nc.vector.tensor_scalar(out=kst['adj'][:], in0=kst['cnt_c'][:], scalar1=float(-LQ),
                                    scalar2=1.0 / (L * PHI), op0=OP.add, op1=OP.mult)
            nc.vector.tensor_tensor(out=kst['adj'][:], in0=kst['adj'][:], in1=kst['sig'][:], op=OP.mult)
            kst['t1'] = sc.tile([D, 1], f32, tag="t1", name="t1")
            nc.vector.tensor_tensor(out=kst['t1'][:], in0=kst['tk'][:], in1=kst['adj'][:], op=OP.add)

        def emit_kred_b():
            HL = L // 2
            kst['s1c'] = sc.tile([D, 1], f32, tag="s1c", name="s1c")
            kst['s1p'] = sc.tile([D, 1], f32, tag="s1p", name="s1p")
            nc.vector.tensor_scalar(out=sqk_rep[0:D, 0:HL], in0=kT32[:, 0:HL],
                                    scalar1=kst['t1'][:, 0:1],
                                    scalar2=0.0, op0=OP.subtract, op1=OP.max)
            nc.gpsimd.tensor_scalar(out=sqk_rep[0:D, HL:L], in0=kT32[:, HL:L],
                                    scalar1=kst['t1'][:, 0:1],
                                    scalar2=0.0, op0=OP.subtract, op1=OP.max)
            nc.vector.tensor_reduce(out=kst['s1c'][:], in_=sqk_rep[0:D, 0:HL],
                                    axis=mybir.AxisListType.X, op=OP.add)
            nc.gpsimd.tensor_scalar(out=cmp_rep[0:D, HL:L], in0=sqk_rep[0:D, HL:L],
                                    scalar1=1.0, scalar2=None,
                                    op0=OP.mult, op1=OP.add,
                                    accum_out=kst['s1p'][:])
            nc.vector.tensor_tensor(out=kst['s1c'][:], in0=kst['s1c'][:],
                                    in1=kst['s1p'][:], op=OP.add)
            nc.vector.tensor_scalar(out=kr[:], in0=kst['s1c'][:], scalar1=1.0 / LQ,
                                    scalar2=None, op0=OP.mult)
            nc.vector.tensor_tensor(out=kr[:], in0=kr[:], in1=kst['t1'][:], op=OP.add)

        emit_kred_a()

        # =============== phase 2: attention + selection ===============
        # PSUM budget (8 banks): ACT pair-strips 2x2 + shared DVE/Pool
        # single-ktile strip ring 2x1 + AV/misc pool 2x1.
        with tc.tile_pool(name="ps_strip", bufs=2, space="PSUM") as ps_strip, \
             tc.tile_pool(name="ps_vstrip", bufs=2, space="PSUM") as ps_vstrip, \
             tc.tile_pool(name="ps_av", bufs=2, space="PSUM") as ps_av:
            def mis_tile():
                return ps_av.tile([PART, PART], f32, tag="av", name="avm")

            def pt_of(s):
                return (pt_a, pt_b, pt_c)[s % 3]

            def emit_sel_pe():
                pwt = mis_tile()
                nc.tensor.transpose(out=pwt[0:D, 0:D], in_=wq_s[:], identity=ident[0:D, 0:D])
                wqT = sc.tile([D, D], f32, tag="wqT")
                nc.vector.tensor_copy(wqT[:], pwt[0:D, 0:D])
                pw = mis_tile()
                nc.tensor.matmul(out=pw[0:D, 0:1], lhsT=wqT[:], rhs=kr[:],
                                 start=True, stop=True)
                nc.vector.tensor_copy(wvec[:], pw[0:D, 0:1])
                psq = mis_tile()
                for c in range(NT):
                    nc.tensor.matmul(out=psq[:, c:c + 1],
                                     lhsT=xT32[:, PART * c:PART * (c + 1)],
                                     rhs=wvec[:], start=True, stop=True)
                nc.vector.tensor_copy(sqk[:], psq[:, 0:NT])

                psqT = mis_tile()
                nc.tensor.transpose(out=psqT[0:NT, 0:PART], in_=sqk[:], identity=ident[:])
                sqkT = sc.tile([NT, PART], f32, tag="sqkT")
                nc.vector.tensor_copy(sqkT[:], psqT[0:NT, 0:PART])
                nc.sync.dma_start(out=scr_row[:], in_=sqkT[:])
                nc.sync.dma_start(out=sqk_rep[:], in_=scr_row[:].to_broadcast([PART, L]))

                if debug:
                    nc.sync.dma_start(out=dbg["dbg_kr"][:], in_=kr[:])
                    nc.sync.dma_start(out=dbg["dbg_sqk"][:], in_=sqk[:])
                    nc.sync.dma_start(out=dbg["dbg_tk"][0:D, :], in_=kst["t1"][:])


            def emit_meanv_pe():
                # meanV on PE

                pmv = mis_tile()
                for c in range(NT):
                    nc.tensor.matmul(out=pmv[0:D + 1, 0:1], lhsT=vp[:, c, :], rhs=onesb[:],
                                     start=(c == 0), stop=(c == NT - 1))
                mv_col = sc.tile([D, 1], f32, tag="mv_col")
                nc.vector.tensor_scalar_mul(mv_col[:], pmv[0:D, 0:1], 1.0 / L)
                pmvT = mis_tile()
                nc.tensor.transpose(out=pmvT[0:1, 0:D], in_=mv_col[:],
                                    identity=ident[0:D, 0:D])
                mv_row = sc.tile([1, D], f32, tag="mv_row")
                nc.vector.tensor_copy(mv_row[:], pmvT[0:1, 0:D])
                pmvF = mis_tile()
                nc.tensor.matmul(out=pmvF[:, 0:D], lhsT=ones1x128[:], rhs=mv_row[:],
                                 start=True, stop=True)
                nc.vector.tensor_copy(mvf[:], pmvF[:, 0:D])

            # ---- selection: two full 128-ary passes on the replicated sqk,
            # then sparse_gather compacts the ~10 in-interval values and three
            # cheap passes on the compacted set finish to f32 resolution ----
            sel_state = {}
            DLT1 = 2.0 * BOUND / 129.0
            DLT2 = DLT1 / 129.0        # interval width after pass 2
            DLTC = [DLT2 / 129.0, DLT2 / 129.0 ** 2, DLT2 / 129.0 ** 3]

            def sel_pass_init():
                lo = mn.tile([PART, 1], f32, tag="lo_a")
                nc.vector.memset(lo[:], -BOUND)
                sel_state["lo"] = lo

            def sel_pass_full(it):
                # thresholds t_p = lo + p*dlt; count(sqk > t_p) per partition
                lo = sel_state["lo"]
                dlt = DLT1 if it == 0 else DLT2
                tvec = mn.tile([PART, 1], f32, tag=f"tv{it % 2}")
                nc.vector.tensor_scalar(out=tvec[:], in0=pidx1[:], scalar1=float(dlt),
                                        scalar2=None, op0=OP.mult)
                nc.vector.tensor_tensor(out=tvec[:], in0=tvec[:], in1=lo[:], op=OP.add)
                HL = L // 2
                cntq = mn.tile([PART, 1], f32, tag="cntq")
                cntp = mn.tile([PART, 1], f32, tag="cntp")
                nc.vector.tensor_scalar(out=cmp_rep[:, 0:HL], in0=sqk_rep[:, 0:HL],
                                        scalar1=tvec[:, 0:1], scalar2=None,
                                        op0=OP.is_gt, op1=OP.add, accum_out=cntq[:])
                nc.gpsimd.tensor_scalar(out=cmp_rep[:, HL:L], in0=sqk_rep[:, HL:L],
                                        scalar1=tvec[:, 0:1], scalar2=None,
                                        op0=OP.is_gt, op1=OP.add, accum_out=cntp[:])
                nc.vector.tensor_tensor(out=cntq[:], in0=cntq[:],
                                        in1=cntp[:], op=OP.add)
                sel = mn.tile([PART, 1], f32, tag="sel")
                nc.vector.tensor_scalar(out=sel[:], in0=cntq[:], scalar1=float(LQ),
                                        scalar2=None, op0=OP.is_ge)
                jsr = mn.tile([PART, 1], f32, tag="jsr")
                nc.gpsimd.partition_all_reduce(jsr[:], sel[:], channels=PART,
                                               reduce_op=bass_isa.ReduceOp.add)
                nlo = mn.tile([PART, 1], f32, tag=f"lo_{'b' if it % 2 == 0 else 'a'}")
                nc.vector.tensor_scalar(out=jsr[:], in0=jsr[:], scalar1=float(dlt),
                                        scalar2=None, op0=OP.mult)
                nc.vector.tensor_tensor(out=nlo[:], in0=lo[:], in1=jsr[:], op=OP.add)
                sel_state["lo"] = nlo

            def sel_compact():
                # threshold in (lo2, lo2 + DLT2]; c_hi = count(sqk > hi2) exact;
                # compact v' = sqk - lo2 for in-interval values via sparse_gather
                lo2 = sel_state["lo"]
                hi2 = mn.tile([PART, 1], f32, tag="hi2")
                nc.vector.tensor_scalar(out=hi2[:], in0=lo2[:], scalar1=float(DLT2),
                                        scalar2=None, op0=OP.add)
                nc.sync.dma_start(out=sqk16[:], in_=scr_row[0, :].rearrange(
                    "(f p) -> p f", p=16))
                # c_hi = count(sqk > hi2), counted on the [128, 32] per-query
                # tile + a channels=128 all-reduce (HW-proven path)
                j32 = sc.tile([PART, NT], f32, tag="j32")
                chi = sc.tile([PART, 1], f32, tag="chi")
                nc.vector.tensor_scalar(out=j32[:], in0=sqk[:],
                                        scalar1=hi2[:, 0:1], scalar2=None,
                                        op0=OP.is_gt, op1=OP.add, accum_out=chi[:])
                nc.gpsimd.partition_all_reduce(chi[:], chi[:], channels=PART,
                                               reduce_op=bass_isa.ReduceOp.add)
                rvec = sc.tile([PART, 1], f32, tag="rvec")
                nc.vector.tensor_scalar(out=rvec[:], in0=chi[:], scalar1=-1.0,
                                        scalar2=float(LQ), op0=OP.mult, op1=OP.add)
                sel_state["rvec"] = rvec
                # Tv = (sqk-lo2)*b + (b-1) with b = (sqk <= hi2): in-interval ->
                # positive v', others -> negative (sparse_gather keeps >= 0)
                a16 = sc.tile([16, 256], f32, tag="a16")
                nc.vector.tensor_scalar(out=a16[:], in0=sqk16[:],
                                        scalar1=lo2[0:16, 0:1], scalar2=None,
                                        op0=OP.subtract)
                b16 = sc.tile([16, 256], f32, tag="b16")
                nc.vector.tensor_scalar(out=b16[:], in0=sqk16[:],
                                        scalar1=hi2[0:16, 0:1], scalar2=None,
                                        op0=OP.is_le)
                tv16 = sc.tile([16, 256], f32, tag="tv16")
                nc.vector.tensor_tensor(out=tv16[:], in0=a16[:], in1=b16[:],
                                        op=OP.mult)
                nc.vector.tensor_scalar(out=b16[:], in0=b16[:], scalar1=1.0,
                                        scalar2=None, op0=OP.subtract)
                nc.vector.tensor_tensor(out=tv16[:], in0=tv16[:], in1=b16[:],
                                        op=OP.add)
                valc = sc.tile([16, 4], f32, tag="valc")
                nc.vector.memset(valc[:], -1.0)
                nfound = sc.tile([1, 1], mybir.dt.uint32, tag="nfound")
                nc.gpsimd.sparse_gather(valc[:], tv16[:], num_found=nfound[:])
                # HW sparse_gather leaves garbage beyond num_found: mask pads
                nf32 = sc.tile([1, 1], f32, tag="nf32")
                nc.vector.tensor_copy(nf32[:], nfound[:])
                nc.sync.dma_start(out=scr_chi[0:1, 0:1], in_=nf32[:])
                nfb = sc.tile([16, 1], f32, tag="nfb")
                nc.sync.dma_start(out=nfb[:],
                                  in_=scr_chi[0:1, 0:1].to_broadcast([16, 1]))
                vmask = sc.tile([16, 4], f32, tag="vmask")
                nc.vector.tensor_scalar(out=vmask[:], in0=iotc[:],
                                        scalar1=nfb[:, 0:1], scalar2=None,
                                        op0=OP.is_lt)
                nc.vector.tensor_tensor(out=valc[:], in0=valc[:], in1=vmask[:],
                                        op=OP.mult)
                nc.vector.tensor_scalar(out=vmask[:], in0=vmask[:], scalar1=1.0,
                                        scalar2=None, op0=OP.subtract)
                nc.vector.tensor_tensor(out=valc[:], in0=valc[:], in1=vmask[:],
                                        op=OP.add)
                nc.sync.dma_start(out=scr_cmp[0, :].rearrange("(f p) -> p f", p=16),
                                  in_=valc[:])
                nc.sync.dma_start(out=valrep[:],
                                  in_=scr_cmp[:].to_broadcast([PART, 64]))
                loc = mn.tile([PART, 1], f32, tag="loc_a")
                nc.vector.memset(loc[:], 0.0)
                sel_state["loc"] = loc

            def sel_pass_c(it):
                loc, rvec = sel_state["loc"], sel_state["rvec"]
                dlt = DLTC[it]
                tvec = mn.tile([PART, 1], f32, tag=f"tvc{it % 2}")
                nc.vector.tensor_scalar(out=tvec[:], in0=pidx1[:], scalar1=float(dlt),
                                        scalar2=None, op0=OP.mult)
                nc.vector.tensor_tensor(out=tvec[:], in0=tvec[:], in1=loc[:], op=OP.add)
                cntq = mn.tile([PART, 1], f32, tag="cntqc")
                nc.vector.tensor_scalar(out=cmpc[:], in0=valrep[:],
                                        scalar1=tvec[:, 0:1], scalar2=None,
                                        op0=OP.is_gt, op1=OP.add, accum_out=cntq[:])
                sel = mn.tile([PART, 1], f32, tag="selc")
                nc.vector.tensor_scalar(out=sel[:], in0=cntq[:],
                                        scalar1=rvec[:, 0:1], scalar2=None,
                                        op0=OP.is_ge)
                jsr = mn.tile([PART, 1], f32, tag="jsrc")
                nc.gpsimd.partition_all_reduce(jsr[:], sel[:], channels=PART,
                                               reduce_op=bass_isa.ReduceOp.add)
                nlo = mn.tile([PART, 1], f32, tag=f"loc_{'b' if it % 2 == 0 else 'a'}")
                nc.vector.tensor_scalar(out=jsr[:], in0=jsr[:], scalar1=float(dlt),
                                        scalar2=None, op0=OP.mult)
                nc.vector.tensor_tensor(out=nlo[:], in0=loc[:], in1=jsr[:], op=OP.add)
                sel_state["loc"] = nlo

            def sel_finish():
                lo2, loc = sel_state["lo"], sel_state["loc"]
                if debug:
                    dsel = mn.tile([PART, 8], f32, tag="dsel")
                    nc.vector.tensor_copy(dsel[:, 0:1], sel_state["rvec"][:])
                    nc.vector.tensor_copy(dsel[:, 1:2], lo2[:])
                    nc.vector.tensor_copy(dsel[:, 2:3], loc[:])
                    nc.sync.dma_start(out=dbg["dbg_sel"][:], in_=dsel[:])
                    nc.sync.dma_start(out=dbg["dbg_valrep"][:], in_=valrep[:, 0:64])
                v128 = mn.tile([PART, NT], f32, tag="v128")
                nc.vector.tensor_scalar(out=v128[:], in0=sqk[:],
                                        scalar1=lo2[:, 0:1], scalar2=None,
                                        op0=OP.subtract)
                nc.vector.tensor_scalar(out=mask[:], in0=v128[:],
                                        scalar1=loc[:, 0:1], scalar2=None,
                                        op0=OP.is_gt)
                minv = mn.tile([PART, NT], f32, tag="minv")
                nc.vector.tensor_scalar(out=minv[:], in0=mask[:], scalar1=-1.0,
                                        scalar2=1.0, op0=OP.mult, op1=OP.add)
                nc.vector.tensor_copy(inv_u8[:], minv[:])
                if debug:
                    thrd = mn.tile([PART, 1], f32, tag="thrd")
                    nc.vector.tensor_tensor(out=thrd[:], in0=lo2[:], in1=loc[:],
                                            op=OP.add)
                    nc.sync.dma_start(out=dbg["dbg_mask"][:], in_=mask[:])
                    nc.sync.dma_start(out=dbg["dbg_thr"][:], in_=thrd[:])
                    cntf = mn.tile([PART, 1], f32, tag="cntf")
                    cmpf = mn.tile([PART, NT], f32, tag="cmpf")
                    nc.vector.tensor_scalar(out=cmpf[:], in0=mask[:], scalar1=1.0,
                                            scalar2=None, op0=OP.mult, op1=OP.add,
                                            accum_out=cntf[:])
                    nc.sync.dma_start(out=dbg["dbg_cnt"][:], in_=cntf[:])

            # ---- attention slab machinery ----
            def score_group(s, ptc, gi):
                g0, glen = GROUPS[gi]
                is_dve = s >= DVE_FROM_SLAB and gi in DVE_GROUPS
                is_pool = gi in POOL_GROUPS
                if is_dve or is_pool:
                    op = nc.vector.tensor_scalar if is_dve else \
                        nc.gpsimd.tensor_scalar
                    for i in range(glen):
                        j = g0 + i
                        strip = ps_vstrip.tile([PART, 1, 512], f32, tag="vstrip")
                        nc.tensor.matmul(out=strip[:, 0, :],
                                         lhsT=kTb[:, PART * j:PART * (j + 1)],
                                         rhs=qTb[:, 512 * s:512 * (s + 1)],
                                         start=True, stop=True)
                        op(out=ptc[:, j:j + 1, :].bitcast(i16),
                           in0=strip[:, 0:1, :], scalar1=SCH_A, scalar2=SCH_B,
                           op0=OP.mult, op1=OP.add)
                else:
                    strip = ps_strip.tile([PART, 2, 512], f32, tag="strip")
                    for i in range(glen):
                        j = g0 + i
                        nc.tensor.matmul(out=strip[:, i, :],
                                         lhsT=kTb[:, PART * j:PART * (j + 1)],
                                         rhs=qTb[:, 512 * s:512 * (s + 1)],
                                         start=True, stop=True)
                    nc.scalar.activation(out=ptc[:, g0:g0 + glen, :],
                                         in_=strip[:, 0:glen, :], func=AF.Exp,
                                         scale=0.125)

            def av_subtile(s, ptp, u):
                if "av" in KSKIP:
                    return
                c = 4 * s + u
                av = mis_tile()
                for j in range(NT):
                    nc.tensor.matmul(out=av[:, 0:D + 1],
                                     lhsT=ptp[:, j, PART * u:PART * (u + 1)],
                                     rhs=vp[:, j, :],
                                     start=(j == 0), stop=(j == NT - 1))
                rec = mn.tile([PART, 1], f32, tag="rec")
                nc.vector.reciprocal_approx_fast(rec[:], av[:, D:D + 1])
                nc.vector.tensor_scalar(out=res[:, c, :], in0=av[:, 0:D],
                                        scalar1=rec[:, 0:1], scalar2=None,
                                        op0=OP.mult)

            def emit_slab(s):
                """scores+exp of slab s (if any) interleaved with AV of s-1."""
                ptc, ptp = pt_of(s), pt_of(s - 1)
                for gi in range(len(GROUPS)):
                    if s < NS:
                        score_group(s, ptc, gi)
                    if gi in (4, 7, 10, 13):
                        av_subtile(s - 1, ptp, (gi - 4) // 3)

            def emit_blend(c):
                if "blend" in KSKIP:
                    return
                nc.vector.copy_predicated(res[:, c, :],
                                          inv_u8[:, c:c + 1].to_broadcast([PART, D]),
                                          mvf[:])
                nc.sync.dma_start(out=out_re[:, c:c + 1, :], in_=res[:, c:c + 1, :])

            # ---- main loop: scores(s) interleaved with AV(s-1) ----
            blended = 0
            for s in range(1, NS + 1):
                emit_slab(s)
                if "sel" not in KSKIP:
                    if s == 1:
                        emit_kred_b()
                        emit_sel_pe()
                        sel_pass_init()
                        sel_pass_full(0)
                    if s == 2:
                        emit_meanv_pe()
                        sel_pass_full(1)
                    if s == 3:
                        sel_compact()
                    if s == 4:
                        sel_pass_c(0); sel_pass_c(1); sel_pass_c(2)
                        sel_finish()
                elif s == 3:
                    nc.vector.memset(mask[:], 1.0)
                    nc.vector.memset(inv_u8[:], 0)
                if s >= 6:
                    # mask is ready; drain blends gradually (a burst would
                    # clog the DVE queue and starve the strip rings)
                    cap = min(4 * (s - 1), blended + 6)
                    while blended < cap:
                        emit_blend(blended)
                        blended += 1
            while blended < NT:
                emit_blend(blended)
                blended += 1

    nc.finalize()
    return nc


_CACHE = {}


def _get_nc(debug=False):
    key = bool(debug)
    if key not in _CACHE:
        _CACHE[key] = build(debug=key)
    return _CACHE[key]


def kernel(x, Wq, Wk, Wv, debug=False):
    nc = _get_nc(debug=debug)
    x = np.asarray(x, dtype=np.float32)
    in_maps = [
        {"x": np.ascontiguousarray(x[i]),
         "Wq": np.asarray(Wq, np.float32), "Wk": np.asarray(Wk, np.float32),
         "Wv": np.asarray(Wv, np.float32)}
        for i in range(B)
    ]
    last_err = None
    for _attempt in range(3):
        try:
            r = run_bass_kernel_spmd(nc, in_maps, core_ids=list(range(N_CORES)))
            out = np.stack([r.results[i]["out"] for i in range(B)]).astype(np.float32)
            break
        except Exception as e:  # transient axon RPC failures
            last_err = e
    else:
        raise last_err
    if debug:
        return out, r.results
    return out



# revision 22
# speedup vs baseline: 1.0516x; 1.0516x over previous
"""Trainium2 Bass kernel for nn_AttentionBlock_33724083208839 (sparse_attention).

Data-parallel over batch (8 batches -> 8 cores). Per core:
  1. chunked x load (all DMAs issued upfront), PE transpose -> xT f32;
     K projected in exact f32 (feeds selection), Q in f32; both also copied
     to bf16 (kTb via ACT, qTb via DVE) for the attention matmuls; V in f32
     -> bf16 [V|1] tiles.
  2. K_reduce via the exact CVaR identity sum_top_l = l*t + sum(relu(x-t)),
     t from Gaussian quantile + one Newton step on the exact count (f32 DVE).
  3. query selection: sqk = x @ (Wq @ K_reduce) exactly on PE; threshold =
     LQ-th largest of sqk via two full 128-ary counting passes on a
     partition-replicated copy, then sparse_gather (GPSIMD) compacts the
     ~10 in-interval values (num_found-masked: HW leaves garbage pads) and
     three cheap passes on the compacted set finish to f32 resolution.
  4. attention for all 4096 queries, 512-query slabs with a 1-slab lag and
     triple-buffered P^T tiles (so scores of slab s never wait on AV of
     slab s-2 releasing a buffer); x is loaded through a 16-chunk rolling
     window:
     scores^T on PE (bf16, 2-ktile PSUM strips) -> exp split between ACT
     (exact, scale=1/8, own 2-slot strip ring) and DVE (Schraudolph bitcast
     exp int16(A*s+B) -> bf16, own strip ring) -> P^T bf16 -> reversed AV:
     lhsT = P^T tile (stationary), rhs = [V|1] (65 moving cols) accumulated
     over 32 k-tiles, interleaved into the score-group stream, so outputs
     land directly in [query-partition, dv] layout; normalize by the
     ones-column denominator (DVE), blend non-selected rows to meanV
     (copy_predicated), DMA out per 128-query chunk.
"""
import os
import sys

sys.path.insert(0, "/opt/trn_rl_repo")

KSKIP = set(os.environ.get("KSKIP", "").split(","))

import math
from statistics import NormalDist

import numpy as np

import concourse.bacc as bacc
import concourse.bass as bass
import concourse.bass_isa as bass_isa
import concourse.mybir as mybir
from concourse.tile import TileContext
from concourse.masks import make_identity
from concourse.bass_utils import run_bass_kernel_spmd

B, L, D = 8, 4096, 64
LQ = int((1.0 - 0.33) * L)  # 2744
PART = 128
NT = L // PART
NS = L // 512
N_CORES = 8

QFRAC = 1.0 - LQ / L
Z = NormalDist().inv_cdf(QFRAC)
PHI = math.exp(-Z * Z / 2.0) / math.sqrt(2.0 * math.pi)

f32 = mybir.dt.float32
f32r = mybir.dt.float32r
bf16 = mybir.dt.bfloat16
u8 = mybir.dt.uint8
i16 = mybir.dt.int16
i32 = mybir.dt.int32
AF = mybir.ActivationFunctionType
OP = mybir.AluOpType

N_PASS = 5
BOUND = 512.0

# Schraudolph exp for bf16 bit patterns: bf16_bits(exp(s/8)) ~= A*s + B.
# A = 128*log2(e)/8; B centers the piecewise-linear sawtooth (mean-unbiased)
# and adds +0.5 to compensate truncation in the float->int16 convert.
SCH_A = 128.0 * math.log2(math.e) / 8.0
SCH_B = 16256.0 + 0.5 - 128.0 * math.log2(1.0407)

GROUPS = [(g, 2) for g in range(0, NT, 2)]

# exp-engine split: selection runs on GPSIMD, so the DVE takes a fixed share
# of the exp strips (Schraudolph), spread through the slab so both engines
# drain the strip ring concurrently.  Pool (gpsimd) also takes a share.
DVE_GROUPS = {2, 4, 7, 9, 12, 14}
POOL_GROUPS = set()
DVE_FROM_SLAB = 1
DVE_GROUPS_S0 = {9, 12, 15}
POOL_GROUPS_S0 = set()


def build(debug: bool = False):
    nc = bacc.Bacc("TRN2")
    x = nc.dram_tensor("x", [L, D], f32, kind="ExternalInput")
    wq = nc.dram_tensor("Wq", [D, D], f32, kind="ExternalInput")
    wk = nc.dram_tensor("Wk", [D, D], f32, kind="ExternalInput")
    wv = nc.dram_tensor("Wv", [D, D], f32, kind="ExternalInput")
    out = nc.dram_tensor("out", [L, D], f32, kind="ExternalOutput")
    scr_row = nc.dram_tensor("scr_row", [1, L], f32, kind="Internal")
    scr_cmp = nc.dram_tensor("scr_cmp", [1, 64], f32, kind="Internal")
    scr_chi = nc.dram_tensor("scr_chi", [1, 16], f32, kind="Internal")
    dbg = {}
    if debug:
        for name, shape in [
            ("dbg_kr", [D, 1]), ("dbg_thr", [PART, 1]), ("dbg_sqk", [PART, NT]),
            ("dbg_mask", [PART, NT]), ("dbg_tk", [PART, 1]), ("dbg_cnt", [PART, 1]),
            ("dbg_sel", [PART, 8]), ("dbg_valrep", [PART, 64]),
        ]:
            dbg[name] = nc.dram_tensor(name, shape, f32, kind="ExternalOutput")

    x_re = x[:].rearrange("(c p) d -> p c d", p=PART)
    out_re = out[:].rearrange("(c p) d -> p c d", p=PART)

    with TileContext(nc) as tc, \
         tc.tile_pool(name="cst", bufs=1) as cst, \
         tc.tile_pool(name="big", bufs=1) as big, \
         tc.tile_pool(name="sc", bufs=1) as sc, \
         tc.tile_pool(name="mn", bufs=2) as mn:

        # ---- warm the exp activation table immediately ----
        warm = cst.tile([1, 8], f32)
        nc.vector.memset(warm[:], 0.0)
        warm2 = cst.tile([1, 8], f32)
        nc.scalar.activation(out=warm2[:], in_=warm[:], func=AF.Exp)

        # ---- constants ----
        ident = cst.tile([PART, PART], f32)
        make_identity(nc, ident[:])
        onesb = cst.tile([PART, 1], bf16)
        nc.vector.memset(onesb[:], 1.0)
        ones1x128 = cst.tile([1, PART], f32)
        nc.vector.memset(ones1x128[:], 1.0)
        iotc_i = cst.tile([16, 4], i32)
        nc.gpsimd.iota(iotc_i[:], pattern=[[16, 4]], base=0, channel_multiplier=1)
        iotc = cst.tile([16, 4], f32)
        nc.vector.tensor_copy(iotc[:], iotc_i[:])
        pidx1i = cst.tile([PART, 1], i32)
        nc.gpsimd.iota(pidx1i[:], pattern=[[1, 1]], base=1, channel_multiplier=1)
        pidx1 = cst.tile([PART, 1], f32)
        nc.vector.tensor_copy(pidx1[:], pidx1i[:])

        # ---- persistent tensors ----
        x_sb = big.tile([PART, 16, D], f32)
        xT32 = big.tile([D, L], f32)
        xTb = big.tile([D, L], bf16)
        qTb = big.tile([D, L], bf16)
        kT32 = big.tile([D, L], f32)
        kTb = big.tile([D, L], bf16)
        vp = big.tile([PART, NT, D + 1], bf16)
        pt_a = big.tile([PART, NT, 512], bf16)
        pt_b = big.tile([PART, NT, 512], bf16)
        pt_c = big.tile([PART, NT, 512], bf16)
        res = big.tile([PART, NT, D], f32)
        mvf = big.tile([PART, D], f32)
        mask = big.tile([PART, NT], f32)
        inv_f = big.tile([PART, NT], f32)
        sqk = big.tile([PART, NT], f32)
        kr = big.tile([D, 1], f32)
        wvec = big.tile([D, 1], f32)
        sqk_rep = big.tile([PART, L], f32)
        cmp_rep = big.tile([PART, L], bf16)
        sqk16 = big.tile([16, 256], f32)
        valrep = big.tile([PART, 64], f32)
        cmpc = big.tile([PART, 64], bf16)

        # weights
        wq_s = cst.tile([D, D], f32)
        wk_s = cst.tile([D, D], f32)
        wv_s = cst.tile([D, D], f32)
        nc.sync.dma_start(out=wq_s[:], in_=wq[:])
        nc.sync.dma_start(out=wk_s[:], in_=wk[:])
        nc.sync.dma_start(out=wv_s[:], in_=wv[:])
        # bf16 copies of Wq/Wv for the bf16 Q/V projections (Pool: it's idle)
        wq_b = cst.tile([D, D], bf16)
        wv_b = cst.tile([D, D], bf16)
        nc.gpsimd.tensor_copy(wq_b[:], wq_s[:])
        nc.gpsimd.tensor_copy(wv_b[:], wv_s[:])

        # =============== phase 1: load / project / slab-0 scores+exp ===============
        with tc.tile_pool(name="ps_xv", bufs=2, space="PSUM") as ps_xv, \
             tc.tile_pool(name="ps_pj", bufs=2, space="PSUM") as ps_pj, \
             tc.tile_pool(name="ps_s0", bufs=2, space="PSUM") as ps_s0:

            def load_tiles(c0, c1):
                for c in range(c0, c1):
                    pxt = ps_xv.tile([PART, PART], f32, tag="xv")
                    nc.tensor.transpose(out=pxt[0:D, :], in_=x_sb[:, c % 16, :],
                                        identity=ident[:])
                    nc.vector.tensor_copy(xT32[:, PART * c:PART * (c + 1)], pxt[0:D, :])
                    nc.scalar.copy(xTb[:, PART * c:PART * (c + 1)], pxt[0:D, :])
                if c0 + 16 < NT:
                    m = c0 % 16
                    nc.sync.dma_start(out=x_sb[:, m:m + (c1 - c0), :],
                                      in_=x_re[:, c0 + 16:c1 + 16, :])

            def proj_slab(s):
                sl = slice(512 * s, 512 * (s + 1))
                pk = ps_pj.tile([D, 512], f32, tag="pj")
                for h in range(2):
                    hs = slice(512 * s + 256 * h, 512 * s + 256 * (h + 1))
                    nc.tensor.matmul(out=pk[:, 256 * h:256 * (h + 1)], lhsT=wk_s[:],
                                     rhs=xT32[:, hs], start=True, stop=True)
                    nc.vector.tensor_copy(kT32[:, hs], pk[:, 256 * h:256 * (h + 1)])
                    nc.scalar.copy(kTb[:, hs], pk[:, 256 * h:256 * (h + 1)])
                pq = ps_pj.tile([D, 512], f32, tag="pj")
                nc.tensor.matmul(out=pq[:], lhsT=wq_b[:], rhs=xTb[:, sl],
                                 start=True, stop=True)
                nc.scalar.copy(qTb[:, sl], pq[:])

            def sg0(gi):
                g0, glen = GROUPS[gi]
                strip = ps_s0.tile([PART, 2, 512], f32, tag="s0")
                for i in range(glen):
                    j = g0 + i
                    nc.tensor.matmul(out=strip[:, i, :],
                                     lhsT=kTb[:, PART * j:PART * (j + 1)],
                                     rhs=qTb[:, 0:512], start=True, stop=True)
                if gi in DVE_GROUPS_S0:  # slab 0
                    nc.vector.tensor_scalar(
                        out=pt_a[:, g0:g0 + glen, :].bitcast(i16),
                        in0=strip[:, 0:glen, :], scalar1=SCH_A, scalar2=SCH_B,
                        op0=OP.mult, op1=OP.add)
                elif gi in POOL_GROUPS_S0:
                    nc.gpsimd.tensor_scalar(
                        out=pt_a[:, g0:g0 + glen, :].bitcast(i16),
                        in0=strip[:, 0:glen, :], scalar1=SCH_A, scalar2=SCH_B,
                        op0=OP.mult, op1=OP.add)
                else:
                    nc.scalar.activation(out=pt_a[:, g0:g0 + glen, :],
                                         in_=strip[:, 0:glen, :], func=AF.Exp, scale=0.125)

            def proj_v(c0, c1):
                for c in range(c0, c1):
                    pv = ps_xv.tile([PART, PART], f32, tag="xv")
                    nc.tensor.matmul(out=pv[:, 0:D],
                                     lhsT=xTb[:, PART * c:PART * (c + 1)],
                                     rhs=wv_b[:], start=True, stop=True)
                    nc.vector.tensor_copy(vp[:, c, 0:D], pv[:, 0:D])

            for c0 in range(0, 16, 4):
                nc.sync.dma_start(out=x_sb[:, c0:c0 + 4, :], in_=x_re[:, c0:c0 + 4, :])
            load_tiles(0, 2)
            load_tiles(2, 4)
            proj_slab(0)
            proj_v(0, 4)
            sg0(0); sg0(1)
            load_tiles(4, 8)
            proj_slab(1)
            proj_v(4, 8)
            sg0(2); sg0(3)
            load_tiles(8, 12)
            load_tiles(12, 16)
            proj_slab(2); proj_v(8, 12); sg0(4); sg0(5)
            proj_slab(3); proj_v(12, 16); sg0(6); sg0(7)
            load_tiles(16, 20)
            load_tiles(20, 24)
            proj_slab(4); proj_v(16, 20); sg0(8); sg0(9)
            proj_slab(5); proj_v(20, 24); sg0(10); sg0(11)
            load_tiles(24, 28)
            load_tiles(28, 32)
            proj_slab(6); proj_v(24, 28); sg0(12); sg0(13)
            proj_slab(7); proj_v(28, 32)
            nc.vector.memset(vp[:, :, D:D + 1], 1.0)
            sg0(14)
            sg0(15)

        kst = {}

        def emit_kred_a():
            kst['bstats'] = sc.tile([D, 8, 6], f32, tag="bstats", name="bstats")
            for a in range(8):
                nc.vector.bn_stats(kst['bstats'][:, a, :], kT32[:, 512 * a:512 * (a + 1)])
            kst['aggr'] = sc.tile([D, 2], f32, tag="aggr", name="aggr")
            nc.vector.bn_aggr(kst['aggr'][:], kst['bstats'][:])
            kst['sig'] = sc.tile([D, 1], f32, tag="sig", name="sig")
            nc.vector.memset(kst['sig'][:], 1.0)
            for _ in range(4):
                kst['rs'] = sc.tile([D, 1], f32, tag="rs", name="rs")
                nc.vector.reciprocal(kst['rs'][:], kst['sig'][:])
                nc.vector.tensor_tensor(out=kst['rs'][:], in0=kst['rs'][:], in1=kst['aggr'][:, 1:2], op=OP.mult)
                nc.vector.tensor_tensor(out=kst['rs'][:], in0=kst['rs'][:], in1=kst['sig'][:], op=OP.add)
                nc.vector.tensor_scalar_mul(kst['sig'][:], kst['rs'][:], 0.5)
            kst['tk'] = sc.tile([D, 1], f32, tag="tk", name="tk")
            nc.vector.tensor_scalar(out=kst['tk'][:], in0=kst['sig'][:], scalar1=float(Z),
                                    scalar2=None, op0=OP.mult)
            nc.vector.tensor_tensor(out=kst['tk'][:], in0=kst['tk'][:], in1=kst['aggr'][:, 0:1], op=OP.add)
            kst['cnt_c'] = sc.tile([D, 1], f32, tag="cnt_c", name="cnt_c")
            kst['cnt_p'] = sc.tile([D, 1], f32, tag="cnt_p", name="cnt_p")
            HL = L // 2
            nc.vector.tensor_scalar(out=sqk_rep[0:D, 0:HL], in0=kT32[:, 0:HL],
                                    scalar1=kst['tk'][:, 0:1],
                                    scalar2=None, op0=OP.is_gt, op1=OP.add,
                                    accum_out=kst['cnt_c'][:])
            nc.gpsimd.tensor_scalar(out=sqk_rep[0:D, HL:L], in0=kT32[:, HL:L],
                                    scalar1=kst['tk'][:, 0:1],
                                    scalar2=None, op0=OP.is_gt, op1=OP.add,
                                    accum_out=kst['cnt_p'][:])
            nc.vector.tensor_tensor(out=kst['cnt_c'][:], in0=kst['cnt_c'][:],
                                    in1=kst['cnt_p'][:], op=OP.add)
            kst['adj'] = sc.tile([D, 1], f32, tag="adj", name="adj")
            nc.vector.tensor_scalar(out=kst['adj'][:], in0=kst['cnt_c'][:], scalar1=float(-LQ),
                                    scalar2=1.0 / (L * PHI), op0=OP.add, op1=OP.mult)
            nc.vector.tensor_tensor(out=kst['adj'][:], in0=kst['adj'][:], in1=kst['sig'][:], op=OP.mult)
            kst['t1'] = sc.tile([D, 1], f32, tag="t1", name="t1")
            nc.vector.tensor_tensor(out=kst['t1'][:], in0=kst['tk'][:], in1=kst['adj'][:], op=OP.add)

        def emit_kred_b():
            HL = L // 2
            kst['s1c'] = sc.tile([D, 1], f32, tag="s1c", name="s1c")
            kst['s1p'] = sc.tile([D, 1], f32, tag="s1p", name="s1p")
            nc.vector.tensor_scalar(out=sqk_rep[0:D, 0:HL], in0=kT32[:, 0:HL],
                                    scalar1=kst['t1'][:, 0:1],
                                    scalar2=0.0, op0=OP.subtract, op1=OP.max)
            nc.gpsimd.tensor_scalar(out=sqk_rep[0:D, HL:L], in0=kT32[:, HL:L],
                                    scalar1=kst['t1'][:, 0:1],
                                    scalar2=0.0, op0=OP.subtract, op1=OP.max)
            nc.vector.tensor_reduce(out=kst['s1c'][:], in_=sqk_rep[0:D, 0:HL],
                                    axis=mybir.AxisListType.X, op=OP.add)
            nc.gpsimd.tensor_scalar(out=cmp_rep[0:D, HL:L], in0=sqk_rep[0:D, HL:L],
                                    scalar1=1.0, scalar2=None,
                                    op0=OP.mult, op1=OP.add,
                                    accum_out=kst['s1p'][:])
            nc.vector.tensor_tensor(out=kst['s1c'][:], in0=kst['s1c'][:],
                                    in1=kst['s1p'][:], op=OP.add)
            nc.vector.tensor_scalar(out=kr[:], in0=kst['s1c'][:], scalar1=1.0 / LQ,
                                    scalar2=None, op0=OP.mult)
            nc.vector.tensor_tensor(out=kr[:], in0=kr[:], in1=kst['t1'][:], op=OP.add)

        emit_kred_a()

        # =============== phase 2: attention + selection ===============
        # PSUM budget (8 banks): ACT pair-strips 2x2 + shared DVE/Pool
        # single-ktile strip ring 2x1 + AV/misc pool 2x1.
        with tc.tile_pool(name="ps_strip", bufs=2, space="PSUM") as ps_strip, \
             tc.tile_pool(name="ps_vstrip", bufs=1, space="PSUM") as ps_vstrip, \
             tc.tile_pool(name="ps_av", bufs=2, space="PSUM") as ps_av:
            def mis_tile():
                return ps_av.tile([PART, PART], f32, tag="av", name="avm")

            def pt_of(s):
                return (pt_a, pt_b, pt_c)[s % 3]

            def emit_sel_pe():
                pwt = mis_tile()
                nc.tensor.transpose(out=pwt[0:D, 0:D], in_=wq_s[:], identity=ident[0:D, 0:D])
                wqT = sc.tile([D, D], f32, tag="wqT")
                nc.vector.tensor_copy(wqT[:], pwt[0:D, 0:D])
                pw = mis_tile()
                nc.tensor.matmul(out=pw[0:D, 0:1], lhsT=wqT[:], rhs=kr[:],
                                 start=True, stop=True)
                nc.vector.tensor_copy(wvec[:], pw[0:D, 0:1])
                psq = mis_tile()
                for c in range(NT):
                    nc.tensor.matmul(out=psq[:, c:c + 1],
                                     lhsT=xT32[:, PART * c:PART * (c + 1)],
                                     rhs=wvec[:], start=True, stop=True)
                nc.vector.tensor_copy(sqk[:], psq[:, 0:NT])

                psqT = mis_tile()
                nc.tensor.transpose(out=psqT[0:NT, 0:PART], in_=sqk[:], identity=ident[:])
                sqkT = sc.tile([NT, PART], f32, tag="sqkT")
                nc.vector.tensor_copy(sqkT[:], psqT[0:NT, 0:PART])
                nc.sync.dma_start(out=scr_row[:], in_=sqkT[:])
                nc.sync.dma_start(out=sqk_rep[:], in_=scr_row[:].to_broadcast([PART, L]))

                if debug:
                    nc.sync.dma_start(out=dbg["dbg_kr"][:], in_=kr[:])
                    nc.sync.dma_start(out=dbg["dbg_sqk"][:], in_=sqk[:])
                    nc.sync.dma_start(out=dbg["dbg_tk"][0:D, :], in_=kst["t1"][:])


            def emit_meanv_pe():
                # meanV on PE

                pmv = mis_tile()
                for c in range(NT):
                    nc.tensor.matmul(out=pmv[0:D + 1, 0:1], lhsT=vp[:, c, :], rhs=onesb[:],
                                     start=(c == 0), stop=(c == NT - 1))
                mv_col = sc.tile([D, 1], f32, tag="mv_col")
                nc.vector.tensor_scalar_mul(mv_col[:], pmv[0:D, 0:1], 1.0 / L)
                pmvT = mis_tile()
                nc.tensor.transpose(out=pmvT[0:1, 0:D], in_=mv_col[:],
                                    identity=ident[0:D, 0:D])
                mv_row = sc.tile([1, D], f32, tag="mv_row")
                nc.vector.tensor_copy(mv_row[:], pmvT[0:1, 0:D])
                pmvF = mis_tile()
                nc.tensor.matmul(out=pmvF[:, 0:D], lhsT=ones1x128[:], rhs=mv_row[:],
                                 start=True, stop=True)
                nc.vector.tensor_copy(mvf[:], pmvF[:, 0:D])

            # ---- selection: two full 128-ary passes on the replicated sqk,
            # then sparse_gather compacts the ~10 in-interval values and three
            # cheap passes on the compacted set finish to f32 resolution ----
            sel_state = {}
            DLT1 = 2.0 * BOUND / 129.0
            DLT2 = DLT1 / 129.0        # interval width after pass 2
            DLTC = [DLT2 / 129.0, DLT2 / 129.0 ** 2, DLT2 / 129.0 ** 3]

            def sel_pass_init():
                lo = mn.tile([PART, 1], f32, tag="lo_a")
                nc.vector.memset(lo[:], -BOUND)
                sel_state["lo"] = lo

            def sel_pass_full(it):
                # thresholds t_p = lo + p*dlt; count(sqk > t_p) per partition
                lo = sel_state["lo"]
                dlt = DLT1 if it == 0 else DLT2
                tvec = mn.tile([PART, 1], f32, tag=f"tv{it % 2}")
                nc.vector.tensor_scalar(out=tvec[:], in0=pidx1[:], scalar1=float(dlt),
                                        scalar2=None, op0=OP.mult)
                nc.vector.tensor_tensor(out=tvec[:], in0=tvec[:], in1=lo[:], op=OP.add)
                HL = L // 2
                cntq = mn.tile([PART, 1], f32, tag="cntq")
                cntp = mn.tile([PART, 1], f32, tag="cntp")
                nc.vector.tensor_scalar(out=cmp_rep[:, 0:HL], in0=sqk_rep[:, 0:HL],
                                        scalar1=tvec[:, 0:1], scalar2=None,
                                        op0=OP.is_gt, op1=OP.add, accum_out=cntq[:])
                nc.gpsimd.tensor_scalar(out=cmp_rep[:, HL:L], in0=sqk_rep[:, HL:L],
                                        scalar1=tvec[:, 0:1], scalar2=None,
                                        op0=OP.is_gt, op1=OP.add, accum_out=cntp[:])
                nc.vector.tensor_tensor(out=cntq[:], in0=cntq[:],
                                        in1=cntp[:], op=OP.add)
                sel = mn.tile([PART, 1], f32, tag="sel")
                nc.vector.tensor_scalar(out=sel[:], in0=cntq[:], scalar1=float(LQ),
                                        scalar2=None, op0=OP.is_ge)
                jsr = mn.tile([PART, 1], f32, tag="jsr")
                nc.gpsimd.partition_all_reduce(jsr[:], sel[:], channels=PART,
                                               reduce_op=bass_isa.ReduceOp.add)
                nlo = mn.tile([PART, 1], f32, tag=f"lo_{'b' if it % 2 == 0 else 'a'}")
                nc.vector.tensor_scalar(out=jsr[:], in0=jsr[:], scalar1=float(dlt),
                                        scalar2=None, op0=OP.mult)
                nc.vector.tensor_tensor(out=nlo[:], in0=lo[:], in1=jsr[:], op=OP.add)
                sel_state["lo"] = nlo

            def sel_compact():
                # threshold in (lo2, lo2 + DLT2]; c_hi = count(sqk > hi2) exact;
                # compact v' = sqk - lo2 for in-interval values via sparse_gather
                lo2 = sel_state["lo"]
                hi2 = mn.tile([PART, 1], f32, tag="hi2")
                nc.vector.tensor_scalar(out=hi2[:], in0=lo2[:], scalar1=float(DLT2),
                                        scalar2=None, op0=OP.add)
                nc.sync.dma_start(out=sqk16[:], in_=scr_row[0, :].rearrange(
                    "(f p) -> p f", p=16))
                # c_hi = count(sqk > hi2), counted on the [128, 32] per-query
                # tile + a channels=128 all-reduce (HW-proven path)
                j32 = sc.tile([PART, NT], f32, tag="j32")
                chi = sc.tile([PART, 1], f32, tag="chi")
                nc.vector.tensor_scalar(out=j32[:], in0=sqk[:],
                                        scalar1=hi2[:, 0:1], scalar2=None,
                                        op0=OP.is_gt, op1=OP.add, accum_out=chi[:])
                nc.gpsimd.partition_all_reduce(chi[:], chi[:], channels=PART,
                                               reduce_op=bass_isa.ReduceOp.add)
                rvec = sc.tile([PART, 1], f32, tag="rvec")
                nc.vector.tensor_scalar(out=rvec[:], in0=chi[:], scalar1=-1.0,
                                        scalar2=float(LQ), op0=OP.mult, op1=OP.add)
                sel_state["rvec"] = rvec
                # Tv = (sqk-lo2)*b + (b-1) with b = (sqk <= hi2): in-interval ->
                # positive v', others -> negative (sparse_gather keeps >= 0)
                a16 = sc.tile([16, 256], f32, tag="a16")
                nc.vector.tensor_scalar(out=a16[:], in0=sqk16[:],
                                        scalar1=lo2[0:16, 0:1], scalar2=None,
                                        op0=OP.subtract)
                b16 = sc.tile([16, 256], f32, tag="b16")
                nc.vector.tensor_scalar(out=b16[:], in0=sqk16[:],
                                        scalar1=hi2[0:16, 0:1], scalar2=None,
                                        op0=OP.is_le)
                tv16 = sc.tile([16, 256], f32, tag="tv16")
                nc.vector.tensor_tensor(out=tv16[:], in0=a16[:], in1=b16[:],
                                        op=OP.mult)
                nc.vector.tensor_scalar(out=b16[:], in0=b16[:], scalar1=1.0,
                                        scalar2=None, op0=OP.subtract)
                nc.vector.tensor_tensor(out=tv16[:], in0=tv16[:], in1=b16[:],
                                        op=OP.add)
                valc = sc.tile([16, 4], f32, tag="valc")
                nc.vector.memset(valc[:], -1.0)
                nfound = sc.tile([1, 1], mybir.dt.uint32, tag="nfound")
                nc.gpsimd.sparse_gather(valc[:], tv16[:], num_found=nfound[:])
                # HW sparse_gather leaves garbage beyond num_found: mask pads
                nf32 = sc.tile([1, 1], f32, tag="nf32")
                nc.vector.tensor_copy(nf32[:], nfound[:])
                nc.sync.dma_start(out=scr_chi[0:1, 0:1], in_=nf32[:])
                nfb = sc.tile([16, 1], f32, tag="nfb")
                nc.sync.dma_start(out=nfb[:],
                                  in_=scr_chi[0:1, 0:1].to_broadcast([16, 1]))
                vmask = sc.tile([16, 4], f32, tag="vmask")
                nc.vector.tensor_scalar(out=vmask[:], in0=iotc[:],
                                        scalar1=nfb[:, 0:1], scalar2=None,
                                        op0=OP.is_lt)
                nc.vector.tensor_tensor(out=valc[:], in0=valc[:], in1=vmask[:],
                                        op=OP.mult)
                nc.vector.tensor_scalar(out=vmask[:], in0=vmask[:], scalar1=1.0,
                                        scalar2=None, op0=OP.subtract)
                nc.vector.tensor_tensor(out=valc[:], in0=valc[:], in1=vmask[:],
                                        op=OP.add)
                nc.sync.dma_start(out=scr_cmp[0, :].rearrange("(f p) -> p f", p=16),
                                  in_=valc[:])
                nc.sync.dma_start(out=valrep[:],
                                  in_=scr_cmp[:].to_broadcast([PART, 64]))
                loc = mn.tile([PART, 1], f32, tag="loc_a")
                nc.vector.memset(loc[:], 0.0)
                sel_state["loc"] = loc

            def sel_pass_c(it):
                loc, rvec = sel_state["loc"], sel_state["rvec"]
                dlt = DLTC[it]
                tvec = mn.tile([PART, 1], f32, tag=f"tvc{it % 2}")
                nc.vector.tensor_scalar(out=tvec[:], in0=pidx1[:], scalar1=float(dlt),
                                        scalar2=None, op0=OP.mult)
                nc.vector.tensor_tensor(out=tvec[:], in0=tvec[:], in1=loc[:], op=OP.add)
                cntq = mn.tile([PART, 1], f32, tag="cntqc")
                nc.vector.tensor_scalar(out=cmpc[:], in0=valrep[:],
                                        scalar1=tvec[:, 0:1], scalar2=None,
                                        op0=OP.is_gt, op1=OP.add, accum_out=cntq[:])
                sel = mn.tile([PART, 1], f32, tag="selc")
                nc.vector.tensor_scalar(out=sel[:], in0=cntq[:],
                                        scalar1=rvec[:, 0:1], scalar2=None,
                                        op0=OP.is_ge)
                jsr = mn.tile([PART, 1], f32, tag="jsrc")
                nc.gpsimd.partition_all_reduce(jsr[:], sel[:], channels=PART,
                                               reduce_op=bass_isa.ReduceOp.add)
                nlo = mn.tile([PART, 1], f32, tag=f"loc_{'b' if it % 2 == 0 else 'a'}")
                nc.vector.tensor_scalar(out=jsr[:], in0=jsr[:], scalar1=float(dlt),
                                        scalar2=None, op0=OP.mult)
                nc.vector.tensor_tensor(out=nlo[:], in0=loc[:], in1=jsr[:], op=OP.add)
                sel_state["loc"] = nlo

            def sel_finish():
                lo2, loc = sel_state["lo"], sel_state["loc"]
                if debug:
                    dsel = mn.tile([PART, 8], f32, tag="dsel")
                    nc.vector.tensor_copy(dsel[:, 0:1], sel_state["rvec"][:])
                    nc.vector.tensor_copy(dsel[:, 1:2], lo2[:])
                    nc.vector.tensor_copy(dsel[:, 2:3], loc[:])
                    nc.sync.dma_start(out=dbg["dbg_sel"][:], in_=dsel[:])
                    nc.sync.dma_start(out=dbg["dbg_valrep"][:], in_=valrep[:, 0:64])
                v128 = mn.tile([PART, NT], f32, tag="v128")
                nc.vector.tensor_scalar(out=v128[:], in0=sqk[:],
                                        scalar1=lo2[:, 0:1], scalar2=None,
                                        op0=OP.subtract)
                nc.vector.tensor_scalar(out=mask[:], in0=v128[:],
                                        scalar1=loc[:, 0:1], scalar2=None,
                                        op0=OP.is_gt)
                nc.vector.tensor_scalar(out=inv_f[:], in0=mask[:], scalar1=-1.0,
                                        scalar2=1.0, op0=OP.mult, op1=OP.add)
                if debug:
                    thrd = mn.tile([PART, 1], f32, tag="thrd")
                    nc.vector.tensor_tensor(out=thrd[:], in0=lo2[:], in1=loc[:],
                                            op=OP.add)
                    nc.sync.dma_start(out=dbg["dbg_mask"][:], in_=mask[:])
                    nc.sync.dma_start(out=dbg["dbg_thr"][:], in_=thrd[:])
                    cntf = mn.tile([PART, 1], f32, tag="cntf")
                    cmpf = mn.tile([PART, NT], f32, tag="cmpf")
                    nc.vector.tensor_scalar(out=cmpf[:], in0=mask[:], scalar1=1.0,
                                            scalar2=None, op0=OP.mult, op1=OP.add,
                                            accum_out=cntf[:])
                    nc.sync.dma_start(out=dbg["dbg_cnt"][:], in_=cntf[:])

            # ---- attention slab machinery ----
            def score_group(s, ptc, gi):
                g0, glen = GROUPS[gi]
                is_dve = s >= DVE_FROM_SLAB and gi in DVE_GROUPS
                is_pool = gi in POOL_GROUPS
                if is_dve:
                    strip = ps_vstrip.tile([PART, 2, 512], f32, tag="vstrip")
                    for i in range(glen):
                        j = g0 + i
                        nc.tensor.matmul(out=strip[:, i, :],
                                         lhsT=kTb[:, PART * j:PART * (j + 1)],
                                         rhs=qTb[:, 512 * s:512 * (s + 1)],
                                         start=True, stop=True)
                    nc.vector.tensor_scalar(
                        out=ptc[:, g0:g0 + glen, :].bitcast(i16),
                        in0=strip[:, 0:glen, :], scalar1=SCH_A, scalar2=SCH_B,
                        op0=OP.mult, op1=OP.add)
                else:
                    strip = ps_strip.tile([PART, 2, 512], f32, tag="strip")
                    for i in range(glen):
                        j = g0 + i
                        nc.tensor.matmul(out=strip[:, i, :],
                                         lhsT=kTb[:, PART * j:PART * (j + 1)],
                                         rhs=qTb[:, 512 * s:512 * (s + 1)],
                                         start=True, stop=True)
                    nc.scalar.activation(out=ptc[:, g0:g0 + glen, :],
                                         in_=strip[:, 0:glen, :], func=AF.Exp,
                                         scale=0.125)

            def av_subtile(s, ptp, u):
                if "av" in KSKIP:
                    return
                c = 4 * s + u
                av = mis_tile()
                for j in range(NT):
                    nc.tensor.matmul(out=av[:, 0:D + 1],
                                     lhsT=ptp[:, j, PART * u:PART * (u + 1)],
                                     rhs=vp[:, j, :],
                                     start=(j == 0), stop=(j == NT - 1))
                rec = mn.tile([PART, 1], f32, tag="rec")
                nc.vector.reciprocal_approx_fast(rec[:], av[:, D:D + 1])
                nc.vector.tensor_scalar(out=res[:, c, :], in0=av[:, 0:D],
                                        scalar1=rec[:, 0:1], scalar2=None,
                                        op0=OP.mult)

            def emit_slab(s):
                """scores+exp of slab s (if any) interleaved with AV of s-1."""
                ptc, ptp = pt_of(s), pt_of(s - 1)
                for gi in range(len(GROUPS)):
                    if s < NS:
                        score_group(s, ptc, gi)
                    if gi in (4, 7, 10, 13):
                        av_subtile(s - 1, ptp, (gi - 4) // 3)

            def emit_blend(c):
                if "blend" in KSKIP:
                    return
                bl = mn.tile([PART, D], f32, tag="bl")
                nc.gpsimd.tensor_scalar(out=bl[:], in0=mvf[:],
                                        scalar1=inv_f[:, c:c + 1], scalar2=None,
                                        op0=OP.mult)
                nc.gpsimd.tensor_scalar(out=res[:, c, :], in0=res[:, c, :],
                                        scalar1=mask[:, c:c + 1], scalar2=None,
                                        op0=OP.mult)
                nc.gpsimd.tensor_tensor(out=res[:, c, :], in0=res[:, c, :],
                                        in1=bl[:], op=OP.add)
                nc.sync.dma_start(out=out_re[:, c:c + 1, :], in_=res[:, c:c + 1, :])

            # ---- main loop: scores(s) interleaved with AV(s-1) ----
            blended = 0
            for s in range(1, NS + 1):
                emit_slab(s)
                if "sel" not in KSKIP:
                    if s == 1:
                        emit_kred_b()
                        emit_sel_pe()
                        sel_pass_init()
                        sel_pass_full(0)
                    if s == 2:
                        emit_meanv_pe()
                        sel_pass_full(1)
                    if s == 3:
                        sel_compact()
                    if s == 4:
                        sel_pass_c(0); sel_pass_c(1); sel_pass_c(2)
                        sel_finish()
                elif s == 3:
                    nc.vector.memset(mask[:], 1.0)
                    nc.vector.memset(inv_f[:], 0.0)
                if s >= 6:
                    # mask is ready; drain blends gradually (a burst would
                    # clog the DVE queue and starve the strip rings)
                    cap = min(4 * (s - 1), blended + 6)
                    while blended < cap:
                        emit_blend(blended)
                        blended += 1
            while blended < NT:
                emit_blend(blended)
                blended += 1

    nc.finalize()
    return nc


_CACHE = {}


def _get_nc(debug=False):
    key = bool(debug)
    if key not in _CACHE:
        _CACHE[key] = build(debug=key)
    return _CACHE[key]


def kernel(x, Wq, Wk, Wv, debug=False):
    nc = _get_nc(debug=debug)
    x = np.asarray(x, dtype=np.float32)
    in_maps = [
        {"x": np.ascontiguousarray(x[i]),
         "Wq": np.asarray(Wq, np.float32), "Wk": np.asarray(Wk, np.float32),
         "Wv": np.asarray(Wv, np.float32)}
        for i in range(B)
    ]
    last_err = None
    for _attempt in range(3):
        try:
            r = run_bass_kernel_spmd(nc, in_maps, core_ids=list(range(N_CORES)))
            out = np.stack([r.results[i]["out"] for i in range(B)]).astype(np.float32)
            break
        except Exception as e:  # transient axon RPC failures
            last_err = e
    else:
        raise last_err
    if debug:
        return out, r.results
    return out



# revision 24
# speedup vs baseline: 1.0670x; 1.0146x over previous
"""Trainium2 Bass kernel for nn_AttentionBlock_33724083208839 (sparse_attention).

Data-parallel over batch (8 batches -> 8 cores). Per core:
  1. chunked x load (all DMAs issued upfront), PE transpose -> xT f32;
     K projected in exact f32 (feeds selection), Q in f32; both also copied
     to bf16 (kTb via ACT, qTb via DVE) for the attention matmuls; V in f32
     -> bf16 [V|1] tiles.
  2. K_reduce via the exact CVaR identity sum_top_l = l*t + sum(relu(x-t)),
     t from Gaussian quantile + one Newton step on the exact count (f32 DVE).
  3. query selection: sqk = x @ (Wq @ K_reduce) exactly on PE; threshold =
     LQ-th largest of sqk via two full 128-ary counting passes on a
     partition-replicated copy, then sparse_gather (GPSIMD) compacts the
     ~10 in-interval values (num_found-masked: HW leaves garbage pads) and
     three cheap passes on the compacted set finish to f32 resolution.
  4. attention for all 4096 queries, 512-query slabs with a 1-slab lag and
     triple-buffered P^T tiles (so scores of slab s never wait on AV of
     slab s-2 releasing a buffer); x is loaded through a 16-chunk rolling
     window:
     scores^T on PE (bf16, 2-ktile PSUM strips) -> exp split between ACT
     (exact, scale=1/8, own 2-slot strip ring) and DVE (Schraudolph bitcast
     exp int16(A*s+B) -> bf16, own strip ring) -> P^T bf16 -> reversed AV:
     lhsT = P^T tile (stationary), rhs = [V|1] (65 moving cols) accumulated
     over 32 k-tiles, interleaved into the score-group stream, so outputs
     land directly in [query-partition, dv] layout; normalize by the
     ones-column denominator (DVE), blend non-selected rows to meanV
     (copy_predicated), DMA out per 128-query chunk.
"""
import os
import sys

sys.path.insert(0, "/opt/trn_rl_repo")

KSKIP = set(os.environ.get("KSKIP", "").split(","))

import math
from statistics import NormalDist

import numpy as np

import concourse.bacc as bacc
import concourse.bass as bass
import concourse.bass_isa as bass_isa
import concourse.mybir as mybir
from concourse.tile import TileContext
from concourse.masks import make_identity
from concourse.bass_utils import run_bass_kernel_spmd

B, L, D = 8, 4096, 64
LQ = int((1.0 - 0.33) * L)  # 2744
PART = 128
NT = L // PART
NS = L // 512
N_CORES = 8

QFRAC = 1.0 - LQ / L
Z = NormalDist().inv_cdf(QFRAC)
PHI = math.exp(-Z * Z / 2.0) / math.sqrt(2.0 * math.pi)

f32 = mybir.dt.float32
f32r = mybir.dt.float32r
bf16 = mybir.dt.bfloat16
u8 = mybir.dt.uint8
i16 = mybir.dt.int16
i32 = mybir.dt.int32
AF = mybir.ActivationFunctionType
OP = mybir.AluOpType

N_PASS = 5
BOUND = 512.0

# Schraudolph exp for bf16 bit patterns: bf16_bits(exp(s/8)) ~= A*s + B.
# A = 128*log2(e)/8; B centers the piecewise-linear sawtooth (mean-unbiased)
# and adds +0.5 to compensate truncation in the float->int16 convert.
SCH_A = 128.0 * math.log2(math.e) / 8.0
SCH_B = 16256.0 + 0.5 - 128.0 * math.log2(1.0407)

GROUPS = [(g, 2) for g in range(0, NT, 2)]

# exp-engine split: selection runs on GPSIMD, so the DVE takes a fixed share
# of the exp strips (Schraudolph), spread through the slab so both engines
# drain the strip ring concurrently.
DVE_GROUPS = {2, 5, 8, 11, 14}
DVE_FROM_SLAB = 1
DVE_GROUPS_S0 = set()


def build(debug: bool = False):
    nc = bacc.Bacc("TRN2")
    x = nc.dram_tensor("x", [L, D], f32, kind="ExternalInput")
    wq = nc.dram_tensor("Wq", [D, D], f32, kind="ExternalInput")
    wk = nc.dram_tensor("Wk", [D, D], f32, kind="ExternalInput")
    wv = nc.dram_tensor("Wv", [D, D], f32, kind="ExternalInput")
    out = nc.dram_tensor("out", [L, D], f32, kind="ExternalOutput")
    scr_row = nc.dram_tensor("scr_row", [1, L], f32, kind="Internal")
    scr_cmp = nc.dram_tensor("scr_cmp", [1, 64], f32, kind="Internal")
    scr_chi = nc.dram_tensor("scr_chi", [1, 16], f32, kind="Internal")
    dbg = {}
    if debug:
        for name, shape in [
            ("dbg_kr", [D, 1]), ("dbg_thr", [PART, 1]), ("dbg_sqk", [PART, NT]),
            ("dbg_mask", [PART, NT]), ("dbg_tk", [PART, 1]), ("dbg_cnt", [PART, 1]),
            ("dbg_sel", [PART, 8]), ("dbg_valrep", [PART, 64]),
        ]:
            dbg[name] = nc.dram_tensor(name, shape, f32, kind="ExternalOutput")

    x_re = x[:].rearrange("(c p) d -> p c d", p=PART)
    out_re = out[:].rearrange("(c p) d -> p c d", p=PART)

    with TileContext(nc) as tc, \
         tc.tile_pool(name="cst", bufs=1) as cst, \
         tc.tile_pool(name="big", bufs=1) as big, \
         tc.tile_pool(name="sc", bufs=1) as sc, \
         tc.tile_pool(name="mn", bufs=2) as mn:

        # ---- warm the exp activation table immediately ----
        warm = cst.tile([1, 8], f32)
        nc.vector.memset(warm[:], 0.0)
        warm2 = cst.tile([1, 8], f32)
        nc.scalar.activation(out=warm2[:], in_=warm[:], func=AF.Exp)

        # ---- constants ----
        ident = cst.tile([PART, PART], f32)
        make_identity(nc, ident[:])
        onesb = cst.tile([PART, 1], bf16)
        nc.vector.memset(onesb[:], 1.0)
        ones1x128 = cst.tile([1, PART], f32)
        nc.vector.memset(ones1x128[:], 1.0)
        iotc_i = cst.tile([16, 4], i32)
        nc.gpsimd.iota(iotc_i[:], pattern=[[16, 4]], base=0, channel_multiplier=1)
        iotc = cst.tile([16, 4], f32)
        nc.vector.tensor_copy(iotc[:], iotc_i[:])
        pidx1i = cst.tile([PART, 1], i32)
        nc.gpsimd.iota(pidx1i[:], pattern=[[1, 1]], base=1, channel_multiplier=1)
        pidx1 = cst.tile([PART, 1], f32)
        nc.vector.tensor_copy(pidx1[:], pidx1i[:])

        # ---- persistent tensors ----
        x_sb = big.tile([PART, 16, D], f32)
        xT32 = big.tile([D, L], f32)
        xTb = big.tile([D, L], bf16)
        qTb = big.tile([D, L], bf16)
        kT32 = big.tile([D, L], f32)
        kTb = big.tile([D, L], bf16)
        vp = big.tile([PART, NT, D + 1], bf16)
        pt_a = big.tile([PART, NT, 512], bf16)
        pt_b = big.tile([PART, NT, 512], bf16)
        pt_c = big.tile([PART, NT, 512], bf16)
        res = big.tile([PART, NT, D], f32)
        mvf = big.tile([PART, D], f32)
        mask = big.tile([PART, NT], f32)
        inv_u8 = big.tile([PART, NT], u8)
        sqk = big.tile([PART, NT], f32)
        kr = big.tile([D, 1], f32)
        wvec = big.tile([D, 1], f32)
        sqk_rep = big.tile([PART, L], f32)
        cmp_rep = big.tile([PART, L], bf16)
        sqk16 = big.tile([16, 256], f32)
        valrep = big.tile([PART, 64], f32)
        cmpc = big.tile([PART, 64], bf16)

        # weights
        wq_s = cst.tile([D, D], f32)
        wk_s = cst.tile([D, D], f32)
        wv_s = cst.tile([D, D], f32)
        nc.sync.dma_start(out=wq_s[:], in_=wq[:])
        nc.sync.dma_start(out=wk_s[:], in_=wk[:])
        nc.sync.dma_start(out=wv_s[:], in_=wv[:])
        wq_b = cst.tile([D, D], bf16)
        wv_b = cst.tile([D, D], bf16)
        nc.gpsimd.tensor_copy(wq_b[:], wq_s[:])
        nc.gpsimd.tensor_copy(wv_b[:], wv_s[:])

        # =============== phase 1: load / project / slab-0 scores+exp ===============
        with tc.tile_pool(name="ps_xv", bufs=2, space="PSUM") as ps_xv, \
             tc.tile_pool(name="ps_pj", bufs=2, space="PSUM") as ps_pj, \
             tc.tile_pool(name="ps_s0", bufs=2, space="PSUM") as ps_s0:

            def load_tiles(c0, c1):
                for c in range(c0, c1):
                    pxt = ps_xv.tile([PART, PART], f32, tag="xv")
                    nc.tensor.transpose(out=pxt[0:D, :], in_=x_sb[:, c % 16, :],
                                        identity=ident[:])
                    nc.vector.tensor_copy(xT32[:, PART * c:PART * (c + 1)], pxt[0:D, :])
                    nc.scalar.copy(xTb[:, PART * c:PART * (c + 1)], pxt[0:D, :])
                if c0 + 16 < NT:
                    m = c0 % 16
                    nc.sync.dma_start(out=x_sb[:, m:m + (c1 - c0), :],
                                      in_=x_re[:, c0 + 16:c1 + 16, :])

            def proj_slab(s):
                sl = slice(512 * s, 512 * (s + 1))
                pk = ps_pj.tile([D, 512], f32, tag="pj")
                for h in range(2):
                    hs = slice(512 * s + 256 * h, 512 * s + 256 * (h + 1))
                    nc.tensor.matmul(out=pk[:, 256 * h:256 * (h + 1)], lhsT=wk_s[:],
                                     rhs=xT32[:, hs], start=True, stop=True)
                    nc.vector.tensor_copy(kT32[:, hs], pk[:, 256 * h:256 * (h + 1)])
                    nc.scalar.copy(kTb[:, hs], pk[:, 256 * h:256 * (h + 1)])
                pq = ps_pj.tile([D, 512], f32, tag="pj")
                nc.tensor.matmul(out=pq[:], lhsT=wq_b[:], rhs=xTb[:, sl],
                                 start=True, stop=True)
                nc.scalar.copy(qTb[:, sl], pq[:])

            def sg0(gi):
                g0, glen = GROUPS[gi]
                strip = ps_s0.tile([PART, 2, 512], f32, tag="s0")
                for i in range(glen):
                    j = g0 + i
                    nc.tensor.matmul(out=strip[:, i, :],
                                     lhsT=kTb[:, PART * j:PART * (j + 1)],
                                     rhs=qTb[:, 0:512], start=True, stop=True)
                if gi in DVE_GROUPS_S0:  # slab 0
                    nc.vector.tensor_scalar(
                        out=pt_a[:, g0:g0 + glen, :].bitcast(i16),
                        in0=strip[:, 0:glen, :], scalar1=SCH_A, scalar2=SCH_B,
                        op0=OP.mult, op1=OP.add)
                else:
                    nc.scalar.activation(out=pt_a[:, g0:g0 + glen, :],
                                         in_=strip[:, 0:glen, :], func=AF.Exp, scale=0.125)

            def proj_v(c0, c1):
                for c in range(c0, c1):
                    pv = ps_xv.tile([PART, PART], f32, tag="xv")
                    nc.tensor.matmul(out=pv[:, 0:D],
                                     lhsT=xTb[:, PART * c:PART * (c + 1)],
                                     rhs=wv_b[:], start=True, stop=True)
                    nc.vector.tensor_copy(vp[:, c, 0:D], pv[:, 0:D])

            for c0 in range(0, 16, 4):
                nc.sync.dma_start(out=x_sb[:, c0:c0 + 4, :], in_=x_re[:, c0:c0 + 4, :])
            load_tiles(0, 2)
            load_tiles(2, 4)
            proj_slab(0)
            proj_v(0, 4)
            sg0(0); sg0(1)
            load_tiles(4, 8)
            proj_slab(1)
            proj_v(4, 8)
            sg0(2); sg0(3)
            load_tiles(8, 12)
            load_tiles(12, 16)
            proj_slab(2); proj_v(8, 12); sg0(4); sg0(5)
            proj_slab(3); proj_v(12, 16); sg0(6); sg0(7)
            load_tiles(16, 20)
            load_tiles(20, 24)
            proj_slab(4); proj_v(16, 20); sg0(8); sg0(9)
            proj_slab(5); proj_v(20, 24); sg0(10); sg0(11)
            load_tiles(24, 28)
            load_tiles(28, 32)
            proj_slab(6); proj_v(24, 28); sg0(12); sg0(13)
            proj_slab(7); proj_v(28, 32)
            nc.vector.memset(vp[:, :, D:D + 1], 1.0)
            sg0(14)
            sg0(15)

        kst = {}

        def emit_kred_a():
            kst['bstats'] = sc.tile([D, 8, 6], f32, tag="bstats", name="bstats")
            for a in range(8):
                nc.vector.bn_stats(kst['bstats'][:, a, :], kT32[:, 512 * a:512 * (a + 1)])
            kst['aggr'] = sc.tile([D, 2], f32, tag="aggr", name="aggr")
            nc.vector.bn_aggr(kst['aggr'][:], kst['bstats'][:])
            kst['sig'] = sc.tile([D, 1], f32, tag="sig", name="sig")
            nc.vector.memset(kst['sig'][:], 1.0)
            for _ in range(4):
                kst['rs'] = sc.tile([D, 1], f32, tag="rs", name="rs")
                nc.vector.reciprocal(kst['rs'][:], kst['sig'][:])
                nc.vector.tensor_tensor(out=kst['rs'][:], in0=kst['rs'][:], in1=kst['aggr'][:, 1:2], op=OP.mult)
                nc.vector.tensor_tensor(out=kst['rs'][:], in0=kst['rs'][:], in1=kst['sig'][:], op=OP.add)
                nc.vector.tensor_scalar_mul(kst['sig'][:], kst['rs'][:], 0.5)
            kst['tk'] = sc.tile([D, 1], f32, tag="tk", name="tk")
            nc.vector.tensor_scalar(out=kst['tk'][:], in0=kst['sig'][:], scalar1=float(Z),
                                    scalar2=None, op0=OP.mult)
            nc.vector.tensor_tensor(out=kst['tk'][:], in0=kst['tk'][:], in1=kst['aggr'][:, 0:1], op=OP.add)
            kst['cnt_c'] = sc.tile([D, 1], f32, tag="cnt_c", name="cnt_c")
            nc.vector.tensor_scalar(out=sqk_rep[0:D, :], in0=kT32[:], scalar1=kst['tk'][:, 0:1],
                                    scalar2=None, op0=OP.is_gt, op1=OP.add,
                                    accum_out=kst['cnt_c'][:])
            kst['adj'] = sc.tile([D, 1], f32, tag="adj", name="adj")
            nc.vector.tensor_scalar(out=kst['adj'][:], in0=kst['cnt_c'][:], scalar1=float(-LQ),
                                    scalar2=1.0 / (L * PHI), op0=OP.add, op1=OP.mult)
            nc.vector.tensor_tensor(out=kst['adj'][:], in0=kst['adj'][:], in1=kst['sig'][:], op=OP.mult)
            kst['t1'] = sc.tile([D, 1], f32, tag="t1", name="t1")
            nc.vector.tensor_tensor(out=kst['t1'][:], in0=kst['tk'][:], in1=kst['adj'][:], op=OP.add)

        def emit_kred_b():
            nc.vector.tensor_scalar(out=sqk_rep[0:D, :], in0=kT32[:], scalar1=kst['t1'][:, 0:1],
                                    scalar2=0.0, op0=OP.subtract, op1=OP.max)
            kst['s1c'] = sc.tile([D, 1], f32, tag="s1c", name="s1c")
            nc.vector.tensor_reduce(out=kst['s1c'][:], in_=sqk_rep[0:D, :], axis=mybir.AxisListType.X,
                                    op=OP.add)
            nc.vector.tensor_scalar(out=kr[:], in0=kst['s1c'][:], scalar1=1.0 / LQ,
                                    scalar2=None, op0=OP.mult)
            nc.vector.tensor_tensor(out=kr[:], in0=kr[:], in1=kst['t1'][:], op=OP.add)

        emit_kred_a()

        # =============== phase 2: attention + selection ===============
        with tc.tile_pool(name="ps_strip", bufs=2, space="PSUM") as ps_strip, \
             tc.tile_pool(name="ps_dstrip", bufs=1, space="PSUM") as ps_dstrip, \
             tc.tile_pool(name="ps_av", bufs=2, space="PSUM") as ps_av:
            def mis_tile():
                return ps_av.tile([PART, PART], f32, tag="av", name="avm")

            def pt_of(s):
                return (pt_a, pt_b, pt_c)[s % 3]

            def emit_sel_pe():
                pwt = mis_tile()
                nc.tensor.transpose(out=pwt[0:D, 0:D], in_=wq_s[:], identity=ident[0:D, 0:D])
                wqT = sc.tile([D, D], f32, tag="wqT")
                nc.vector.tensor_copy(wqT[:], pwt[0:D, 0:D])
                pw = mis_tile()
                nc.tensor.matmul(out=pw[0:D, 0:1], lhsT=wqT[:], rhs=kr[:],
                                 start=True, stop=True)
                nc.vector.tensor_copy(wvec[:], pw[0:D, 0:1])
                psq = mis_tile()
                for c in range(NT):
                    nc.tensor.matmul(out=psq[:, c:c + 1],
                                     lhsT=xT32[:, PART * c:PART * (c + 1)],
                                     rhs=wvec[:], start=True, stop=True)
                nc.vector.tensor_copy(sqk[:], psq[:, 0:NT])

                psqT = mis_tile()
                nc.tensor.transpose(out=psqT[0:NT, 0:PART], in_=sqk[:], identity=ident[:])
                sqkT = sc.tile([NT, PART], f32, tag="sqkT")
                nc.vector.tensor_copy(sqkT[:], psqT[0:NT, 0:PART])
                nc.sync.dma_start(out=scr_row[:], in_=sqkT[:])
                nc.sync.dma_start(out=sqk_rep[:], in_=scr_row[:].to_broadcast([PART, L]))

                if debug:
                    nc.sync.dma_start(out=dbg["dbg_kr"][:], in_=kr[:])
                    nc.sync.dma_start(out=dbg["dbg_sqk"][:], in_=sqk[:])
                    nc.sync.dma_start(out=dbg["dbg_tk"][0:D, :], in_=kst["t1"][:])


            def emit_meanv_pe():
                # meanV on PE

                pmv = mis_tile()
                for c in range(NT):
                    nc.tensor.matmul(out=pmv[0:D + 1, 0:1], lhsT=vp[:, c, :], rhs=onesb[:],
                                     start=(c == 0), stop=(c == NT - 1))
                mv_col = sc.tile([D, 1], f32, tag="mv_col")
                nc.vector.tensor_scalar_mul(mv_col[:], pmv[0:D, 0:1], 1.0 / L)
                pmvT = mis_tile()
                nc.tensor.transpose(out=pmvT[0:1, 0:D], in_=mv_col[:],
                                    identity=ident[0:D, 0:D])
                mv_row = sc.tile([1, D], f32, tag="mv_row")
                nc.vector.tensor_copy(mv_row[:], pmvT[0:1, 0:D])
                pmvF = mis_tile()
                nc.tensor.matmul(out=pmvF[:, 0:D], lhsT=ones1x128[:], rhs=mv_row[:],
                                 start=True, stop=True)
                nc.vector.tensor_copy(mvf[:], pmvF[:, 0:D])

            # ---- selection: two full 128-ary passes on the replicated sqk,
            # then sparse_gather compacts the ~10 in-interval values and three
            # cheap passes on the compacted set finish to f32 resolution ----
            sel_state = {}
            DLT1 = 2.0 * BOUND / 129.0
            DLT2 = DLT1 / 129.0        # interval width after pass 2
            DLTC = [DLT2 / 129.0, DLT2 / 129.0 ** 2, DLT2 / 129.0 ** 3]

            def sel_pass_init():
                lo = mn.tile([PART, 1], f32, tag="lo_a")
                nc.vector.memset(lo[:], -BOUND)
                sel_state["lo"] = lo

            def sel_pass_full(it):
                # thresholds t_p = lo + p*dlt; count(sqk > t_p) per partition
                lo = sel_state["lo"]
                dlt = DLT1 if it == 0 else DLT2
                tvec = mn.tile([PART, 1], f32, tag=f"tv{it % 2}")
                nc.vector.tensor_scalar(out=tvec[:], in0=pidx1[:], scalar1=float(dlt),
                                        scalar2=None, op0=OP.mult)
                nc.vector.tensor_tensor(out=tvec[:], in0=tvec[:], in1=lo[:], op=OP.add)
                cntq = mn.tile([PART, 1], f32, tag="cntq")
                nc.vector.tensor_scalar(out=cmp_rep[:], in0=sqk_rep[:],
                                        scalar1=tvec[:, 0:1], scalar2=None,
                                        op0=OP.is_gt, op1=OP.add, accum_out=cntq[:])
                sel = mn.tile([PART, 1], f32, tag="sel")
                nc.vector.tensor_scalar(out=sel[:], in0=cntq[:], scalar1=float(LQ),
                                        scalar2=None, op0=OP.is_ge)
                jsr = mn.tile([PART, 1], f32, tag="jsr")
                nc.gpsimd.partition_all_reduce(jsr[:], sel[:], channels=PART,
                                               reduce_op=bass_isa.ReduceOp.add)
                nlo = mn.tile([PART, 1], f32, tag=f"lo_{'b' if it % 2 == 0 else 'a'}")
                nc.vector.tensor_scalar(out=jsr[:], in0=jsr[:], scalar1=float(dlt),
                                        scalar2=None, op0=OP.mult)
                nc.vector.tensor_tensor(out=nlo[:], in0=lo[:], in1=jsr[:], op=OP.add)
                sel_state["lo"] = nlo

            def sel_compact():
                # threshold in (lo2, lo2 + DLT2]; c_hi = count(sqk > hi2) exact;
                # compact v' = sqk - lo2 for in-interval values via sparse_gather
                lo2 = sel_state["lo"]
                hi2 = mn.tile([PART, 1], f32, tag="hi2")
                nc.vector.tensor_scalar(out=hi2[:], in0=lo2[:], scalar1=float(DLT2),
                                        scalar2=None, op0=OP.add)
                nc.sync.dma_start(out=sqk16[:], in_=scr_row[0, :].rearrange(
                    "(f p) -> p f", p=16))
                # c_hi = count(sqk > hi2), counted on the [128, 32] per-query
                # tile + a channels=128 all-reduce (HW-proven path)
                j32 = sc.tile([PART, NT], f32, tag="j32")
                chi = sc.tile([PART, 1], f32, tag="chi")
                nc.vector.tensor_scalar(out=j32[:], in0=sqk[:],
                                        scalar1=hi2[:, 0:1], scalar2=None,
                                        op0=OP.is_gt, op1=OP.add, accum_out=chi[:])
                nc.gpsimd.partition_all_reduce(chi[:], chi[:], channels=PART,
                                               reduce_op=bass_isa.ReduceOp.add)
                rvec = sc.tile([PART, 1], f32, tag="rvec")
                nc.vector.tensor_scalar(out=rvec[:], in0=chi[:], scalar1=-1.0,
                                        scalar2=float(LQ), op0=OP.mult, op1=OP.add)
                sel_state["rvec"] = rvec
                # Tv = (sqk-lo2)*b + (b-1) with b = (sqk <= hi2): in-interval ->
                # positive v', others -> negative (sparse_gather keeps >= 0)
                a16 = sc.tile([16, 256], f32, tag="a16")
                nc.vector.tensor_scalar(out=a16[:], in0=sqk16[:],
                                        scalar1=lo2[0:16, 0:1], scalar2=None,
                                        op0=OP.subtract)
                b16 = sc.tile([16, 256], f32, tag="b16")
                nc.vector.tensor_scalar(out=b16[:], in0=sqk16[:],
                                        scalar1=hi2[0:16, 0:1], scalar2=None,
                                        op0=OP.is_le)
                tv16 = sc.tile([16, 256], f32, tag="tv16")
                nc.vector.tensor_tensor(out=tv16[:], in0=a16[:], in1=b16[:],
                                        op=OP.mult)
                nc.vector.tensor_scalar(out=b16[:], in0=b16[:], scalar1=1.0,
                                        scalar2=None, op0=OP.subtract)
                nc.vector.tensor_tensor(out=tv16[:], in0=tv16[:], in1=b16[:],
                                        op=OP.add)
                valc = sc.tile([16, 4], f32, tag="valc")
                nc.vector.memset(valc[:], -1.0)
                nfound = sc.tile([1, 1], mybir.dt.uint32, tag="nfound")
                nc.gpsimd.sparse_gather(valc[:], tv16[:], num_found=nfound[:])
                # HW sparse_gather leaves garbage beyond num_found: mask pads
                nf32 = sc.tile([1, 1], f32, tag="nf32")
                nc.vector.tensor_copy(nf32[:], nfound[:])
                nc.sync.dma_start(out=scr_chi[0:1, 0:1], in_=nf32[:])
                nfb = sc.tile([16, 1], f32, tag="nfb")
                nc.sync.dma_start(out=nfb[:],
                                  in_=scr_chi[0:1, 0:1].to_broadcast([16, 1]))
                vmask = sc.tile([16, 4], f32, tag="vmask")
                nc.vector.tensor_scalar(out=vmask[:], in0=iotc[:],
                                        scalar1=nfb[:, 0:1], scalar2=None,
                                        op0=OP.is_lt)
                nc.vector.tensor_tensor(out=valc[:], in0=valc[:], in1=vmask[:],
                                        op=OP.mult)
                nc.vector.tensor_scalar(out=vmask[:], in0=vmask[:], scalar1=1.0,
                                        scalar2=None, op0=OP.subtract)
                nc.vector.tensor_tensor(out=valc[:], in0=valc[:], in1=vmask[:],
                                        op=OP.add)
                nc.sync.dma_start(out=scr_cmp[0, :].rearrange("(f p) -> p f", p=16),
                                  in_=valc[:])
                nc.sync.dma_start(out=valrep[:],
                                  in_=scr_cmp[:].to_broadcast([PART, 64]))
                loc = mn.tile([PART, 1], f32, tag="loc_a")
                nc.vector.memset(loc[:], 0.0)
                sel_state["loc"] = loc

            def sel_pass_c(it):
                loc, rvec = sel_state["loc"], sel_state["rvec"]
                dlt = DLTC[it]
                tvec = mn.tile([PART, 1], f32, tag=f"tvc{it % 2}")
                nc.vector.tensor_scalar(out=tvec[:], in0=pidx1[:], scalar1=float(dlt),
                                        scalar2=None, op0=OP.mult)
                nc.vector.tensor_tensor(out=tvec[:], in0=tvec[:], in1=loc[:], op=OP.add)
                cntq = mn.tile([PART, 1], f32, tag="cntqc")
                nc.vector.tensor_scalar(out=cmpc[:], in0=valrep[:],
                                        scalar1=tvec[:, 0:1], scalar2=None,
                                        op0=OP.is_gt, op1=OP.add, accum_out=cntq[:])
                sel = mn.tile([PART, 1], f32, tag="selc")
                nc.vector.tensor_scalar(out=sel[:], in0=cntq[:],
                                        scalar1=rvec[:, 0:1], scalar2=None,
                                        op0=OP.is_ge)
                jsr = mn.tile([PART, 1], f32, tag="jsrc")
                nc.gpsimd.partition_all_reduce(jsr[:], sel[:], channels=PART,
                                               reduce_op=bass_isa.ReduceOp.add)
                nlo = mn.tile([PART, 1], f32, tag=f"loc_{'b' if it % 2 == 0 else 'a'}")
                nc.vector.tensor_scalar(out=jsr[:], in0=jsr[:], scalar1=float(dlt),
                                        scalar2=None, op0=OP.mult)
                nc.vector.tensor_tensor(out=nlo[:], in0=loc[:], in1=jsr[:], op=OP.add)
                sel_state["loc"] = nlo

            def sel_finish():
                lo2, loc = sel_state["lo"], sel_state["loc"]
                if debug:
                    dsel = mn.tile([PART, 8], f32, tag="dsel")
                    nc.vector.tensor_copy(dsel[:, 0:1], sel_state["rvec"][:])
                    nc.vector.tensor_copy(dsel[:, 1:2], lo2[:])
                    nc.vector.tensor_copy(dsel[:, 2:3], loc[:])
                    nc.sync.dma_start(out=dbg["dbg_sel"][:], in_=dsel[:])
                    nc.sync.dma_start(out=dbg["dbg_valrep"][:], in_=valrep[:, 0:64])
                v128 = mn.tile([PART, NT], f32, tag="v128")
                nc.vector.tensor_scalar(out=v128[:], in0=sqk[:],
                                        scalar1=lo2[:, 0:1], scalar2=None,
                                        op0=OP.subtract)
                nc.vector.tensor_scalar(out=mask[:], in0=v128[:],
                                        scalar1=loc[:, 0:1], scalar2=None,
                                        op0=OP.is_gt)
                minv = mn.tile([PART, NT], f32, tag="minv")
                nc.vector.tensor_scalar(out=minv[:], in0=mask[:], scalar1=-1.0,
                                        scalar2=1.0, op0=OP.mult, op1=OP.add)
                nc.vector.tensor_copy(inv_u8[:], minv[:])
                if debug:
                    thrd = mn.tile([PART, 1], f32, tag="thrd")
                    nc.vector.tensor_tensor(out=thrd[:], in0=lo2[:], in1=loc[:],
                                            op=OP.add)
                    nc.sync.dma_start(out=dbg["dbg_mask"][:], in_=mask[:])
                    nc.sync.dma_start(out=dbg["dbg_thr"][:], in_=thrd[:])
                    cntf = mn.tile([PART, 1], f32, tag="cntf")
                    cmpf = mn.tile([PART, NT], f32, tag="cmpf")
                    nc.vector.tensor_scalar(out=cmpf[:], in0=mask[:], scalar1=1.0,
                                            scalar2=None, op0=OP.mult, op1=OP.add,
                                            accum_out=cntf[:])
                    nc.sync.dma_start(out=dbg["dbg_cnt"][:], in_=cntf[:])

            # ---- attention slab machinery ----
            def score_group(s, ptc, gi):
                g0, glen = GROUPS[gi]
                is_dve = s >= DVE_FROM_SLAB and gi in DVE_GROUPS
                if is_dve:
                    strip = ps_dstrip.tile([PART, 2, 512], f32, tag="dstrip")
                else:
                    strip = ps_strip.tile([PART, 2, 512], f32, tag="strip")
                for i in range(glen):
                    j = g0 + i
                    nc.tensor.matmul(out=strip[:, i, :],
                                     lhsT=kTb[:, PART * j:PART * (j + 1)],
                                     rhs=qTb[:, 512 * s:512 * (s + 1)],
                                     start=True, stop=True)
                if is_dve:
                    nc.vector.tensor_scalar(
                        out=ptc[:, g0:g0 + glen, :].bitcast(i16),
                        in0=strip[:, 0:glen, :], scalar1=SCH_A, scalar2=SCH_B,
                        op0=OP.mult, op1=OP.add)
                else:
                    nc.scalar.activation(out=ptc[:, g0:g0 + glen, :],
                                         in_=strip[:, 0:glen, :], func=AF.Exp,
                                         scale=0.125)

            def av_subtile(s, ptp, u):
                if "av" in KSKIP:
                    return
                c = 4 * s + u
                av = mis_tile()
                for j in range(NT):
                    nc.tensor.matmul(out=av[:, 0:D + 1],
                                     lhsT=ptp[:, j, PART * u:PART * (u + 1)],
                                     rhs=vp[:, j, :],
                                     start=(j == 0), stop=(j == NT - 1))
                rec = mn.tile([PART, 1], f32, tag="rec")
                nc.vector.reciprocal_approx_fast(rec[:], av[:, D:D + 1])
                nc.vector.tensor_scalar(out=res[:, c, :], in0=av[:, 0:D],
                                        scalar1=rec[:, 0:1], scalar2=None,
                                        op0=OP.mult)

            def emit_slab(s):
                """scores+exp of slab s (if any) interleaved with AV of s-1."""
                ptc, ptp = pt_of(s), pt_of(s - 1)
                for gi in range(len(GROUPS)):
                    if s < NS:
                        score_group(s, ptc, gi)
                    if gi in (4, 7, 10, 13):
                        av_subtile(s - 1, ptp, (gi - 4) // 3)

            def emit_blend(c):
                if "blend" in KSKIP:
                    return
                nc.vector.copy_predicated(res[:, c, :],
                                          inv_u8[:, c:c + 1].to_broadcast([PART, D]),
                                          mvf[:])
                nc.sync.dma_start(out=out_re[:, c:c + 1, :], in_=res[:, c:c + 1, :])

            # ---- main loop: scores(s) interleaved with AV(s-1) ----
            blended = 0
            for s in range(1, NS + 1):
                emit_slab(s)
                if "sel" not in KSKIP:
                    if s == 1:
                        emit_kred_b()
                        emit_sel_pe()
                        sel_pass_init()
                        sel_pass_full(0)
                    if s == 2:
                        emit_meanv_pe()
                        sel_pass_full(1)
                    if s == 3:
                        sel_compact()
                    if s == 4:
                        sel_pass_c(0); sel_pass_c(1); sel_pass_c(2)
                        sel_finish()
                elif s == 3:
                    nc.vector.memset(mask[:], 1.0)
                    nc.vector.memset(inv_u8[:], 0)
                if s >= 6:
                    # mask is ready; drain blends gradually (a burst would
                    # clog the DVE queue and starve the strip rings)
                    cap = min(4 * (s - 1), blended + 6)
                    while blended < cap:
                        emit_blend(blended)
                        blended += 1
            while blended < NT:
                emit_blend(blended)
                blended += 1

    nc.finalize()
    return nc


_CACHE = {}


def _get_nc(debug=False):
    key = bool(debug)
    if key not in _CACHE:
        _CACHE[key] = build(debug=key)
    return _CACHE[key]


def kernel(x, Wq, Wk, Wv, debug=False):
    nc = _get_nc(debug=debug)
    x = np.asarray(x, dtype=np.float32)
    in_maps = [
        {"x": np.ascontiguousarray(x[i]),
         "Wq": np.asarray(Wq, np.float32), "Wk": np.asarray(Wk, np.float32),
         "Wv": np.asarray(Wv, np.float32)}
        for i in range(B)
    ]
    last_err = None
    for _attempt in range(3):
        try:
            r = run_bass_kernel_spmd(nc, in_maps, core_ids=list(range(N_CORES)))
            out = np.stack([r.results[i]["out"] for i in range(B)]).astype(np.float32)
            break
        except Exception as e:  # transient axon RPC failures
            last_err = e
    else:
        raise last_err
    if debug:
        return out, r.results
    return out



# revision 28
# speedup vs baseline: 1.1412x; 1.0696x over previous
"""Trainium2 Bass kernel for nn_AttentionBlock_33724083208839 (sparse_attention).

Data-parallel over batch (8 batches -> 8 cores). Hybrid selected-only design:
  1. load x, PE-transpose -> xT32 (f32, exact) + xTb (bf16); K projected in
     exact f32 (feeds selection); Q/V projected in bf16; Q also staged
     row-major to DRAM (qdram [4104,128] bf16, padded cols + zero ghost row).
  2. selection (exact, baseline machinery, DVE+Pool split halves): K_reduce
     via CVaR identity; sqk exact on PE; threshold via two 128-ary counting
     passes on a DMA-replicated sqk + sparse_gather compaction (sentinel
     padding instead of num_found round-trips) + three compact passes.
  3. head: attention for ALL queries of slabs 0-1 (1024 queries) overlaps
     selection latency; blend non-selected head rows to meanV (Pool
     arithmetic), direct DMA out.
  4. tail: positions 1024..4095 -> compact ordered index list of selected
     queries (sparse_gather over masked iota, padded to 2240 with sentinel
     4096 by appending always-gathered sentinel values); dma_gather
     (transpose mode) pulls Q_sel^T tiles from qdram; 4.375 slabs of
     scores->exp(ACT/DVE Schraudolph)->AV; results scattered to out rows by
     indirect DMA (sentinels skipped via bounds_check); rows 1024..4095
     pre-filled with meanV by chunk DMAs before the scatters.
"""
import os
import sys

sys.path.insert(0, "/opt/trn_rl_repo")

import math
from statistics import NormalDist

KSKIP = set(os.environ.get('KSKIP', '').split(','))

import numpy as np

import concourse.bacc as bacc
import concourse.bass as bass
import concourse.bass_isa as bass_isa
import concourse.mybir as mybir
from concourse.tile import TileContext
from concourse.masks import make_identity
from concourse.bass_utils import run_bass_kernel_spmd

B, L, D = 8, 4096, 64
LQ = int((1.0 - 0.33) * L)  # 2744
PART = 128
NT = L // PART          # 32 key tiles / x chunks
N_CORES = 8

NTAIL = 2816            # padded compact count (LQ=2744 exact + sentinels)
NTC = 22                # compact chunks
TAIL_LENS = [512, 512, 512, 512, 512, 256]
SENT = L                # sentinel index -> qdram ghost row, scatter-skipped
QD_ROWS = L + 8         # qdram rows (ghost row at L)
N_VSENT = 64            # value-compaction sentinel count
N_ISENT = 592           # index-compaction sentinels (need >= NTAIL - LQ = 72)

QFRAC = 1.0 - LQ / L
Z = NormalDist().inv_cdf(QFRAC)
PHI = math.exp(-Z * Z / 2.0) / math.sqrt(2.0 * math.pi)

f32 = mybir.dt.float32
bf16 = mybir.dt.bfloat16
u8 = mybir.dt.uint8
i16 = mybir.dt.int16
i32 = mybir.dt.int32
u32 = mybir.dt.uint32
AF = mybir.ActivationFunctionType
OP = mybir.AluOpType

BOUND = 512.0

# Schraudolph exp for bf16 bit patterns: bf16_bits(exp(s/8)) ~= A*s + B.
SCH_A = 128.0 * math.log2(math.e) / 8.0
SCH_B = 16256.0 + 0.5 - 128.0 * math.log2(1.0407)

GROUPS = [(g, 2) for g in range(0, NT, 2)]

# head slabs: DVE takes the last 3 pair-groups (it is busy with selection
# until ~2/3 through each head slab); tail slabs: DVE takes 6.
DVE_GROUPS_TAIL = {3, 6, 8, 11, 14}


def build(debug: bool = False):
    nc = bacc.Bacc("TRN2")
    x = nc.dram_tensor("x", [L, D], f32, kind="ExternalInput")
    wq = nc.dram_tensor("Wq", [D, D], f32, kind="ExternalInput")
    wk = nc.dram_tensor("Wk", [D, D], f32, kind="ExternalInput")
    wv = nc.dram_tensor("Wv", [D, D], f32, kind="ExternalInput")
    out = nc.dram_tensor("out", [L + 8, D], f32, kind="ExternalOutput")
    scr_row = nc.dram_tensor("scr_row", [1, L], f32, kind="Internal")
    scr_cmp = nc.dram_tensor("scr_cmp", [1, 64], f32, kind="Internal")
    qdram = nc.dram_tensor("qdram", [QD_ROWS, PART], bf16, kind="Internal")
    scr_i32 = nc.dram_tensor("scr_i32", [1, NTC * PART], i32, kind="Internal")

    x_re = x[:].rearrange("(c p) d -> p c d", p=PART)
    # partition-contiguous x view: one 8KB/partition DMA
    x_rc = x[:].rearrange("(p c) d -> p c d", c=NT)
    out_re = out[0:L, :].rearrange("(c p) d -> p c d", p=PART)
    qd_re = qdram[0:L, :].rearrange("(c p) e -> p c e", p=PART)

    with TileContext(nc) as tc, \
         tc.tile_pool(name="cst", bufs=1) as cst, \
         tc.tile_pool(name="big", bufs=1) as big, \
         tc.tile_pool(name="sc", bufs=1) as sc, \
         tc.tile_pool(name="mn", bufs=2) as mn, \
         tc.tile_pool(name="qs", bufs=3) as qs:

        # ---- warm the exp activation table immediately ----
        warm = cst.tile([1, 8], f32)
        nc.vector.memset(warm[:], 0.0)
        warm2 = cst.tile([1, 8], f32)
        nc.scalar.activation(out=warm2[:], in_=warm[:], func=AF.Exp)

        # ---- constants ----
        ident = cst.tile([PART, PART], f32)
        make_identity(nc, ident[:])
        onesb = cst.tile([PART, 1], bf16)
        nc.vector.memset(onesb[:], 1.0)
        ones1x128 = cst.tile([1, PART], f32)
        nc.vector.memset(ones1x128[:], 1.0)
        pidx1i = cst.tile([PART, 1], i32)
        nc.gpsimd.iota(pidx1i[:], pattern=[[1, 1]], base=1, channel_multiplier=1)
        pidx1 = cst.tile([PART, 1], f32)
        nc.vector.tensor_copy(pidx1[:], pidx1i[:])
        # pass-1 thresholds are compile-time: tvec1 = pidx1*DLT1 - BOUND
        tvec1 = cst.tile([PART, 1], f32)
        nc.gpsimd.tensor_scalar(out=tvec1[:], in0=pidx1[:],
                                scalar1=2.0 * BOUND / 129.0, scalar2=-BOUND,
                                op0=OP.mult, op1=OP.add)
        # iot1[p, f] = (f*16 + p) + 1 = original position + 1, wrapped layout
        iot1i = cst.tile([16, 256], i32)
        nc.gpsimd.iota(iot1i[:], pattern=[[16, 256]], base=1, channel_multiplier=1)
        iot1 = cst.tile([16, 256], f32)
        nc.vector.tensor_copy(iot1[:], iot1i[:])

        # ---- persistent tensors ----
        x_sb = big.tile([PART, NT, D], f32)
        # x_sb is dead once the transposes finish; reuse it for res
        res = x_sb
        xT32 = big.tile([D, L], f32)
        xTb = big.tile([D, L], bf16)
        kT32 = big.tile([D, L], f32)
        kTb = big.tile([D, L], bf16)
        q_stage = big.tile([PART, NT, PART], bf16)
        vp = big.tile([PART, NT, D + 1], bf16)
        pt_a = big.tile([PART, NT, 512], bf16)
        pt_b = big.tile([PART, NT, 512], bf16)
        pt_c = big.tile([PART, NT, 512], bf16)
        mvf = big.tile([PART, D], f32)
        sqk = big.tile([PART, NT], f32)
        kr = big.tile([D, 1], f32)
        wvec = big.tile([D, 1], f32)
        sqk_rep = big.tile([PART, L], f32)
        sqk16 = big.tile([16, 256], f32)
        valrep = big.tile([PART, 64], f32)
        cmpc = big.tile([PART, 64], bf16)
        # value compaction: input [16, 256+4] (tv || 0.0-sentinels)
        tvv = big.tile([16, 256 + N_VSENT // 16], f32)
        valc = big.tile([16, 20], f32)
        nfdummy = big.tile([1, 1], u32)
        # index compaction: input [16, 256+10] (tvidx || 4096.0-sentinels)
        tvi = big.tile([16, 256 + N_ISENT // 16], f32)
        idxw = big.tile([16, 209], f32)
        idx16w = big.tile([PART, NTC * PART // 16], i16)
        idx32w = big.tile([16, NTC * PART // 16], i32)
        idxo = big.tile([PART, NTC], i32)

        # weights
        wq_s = cst.tile([D, D], f32)
        wk_s = cst.tile([D, D], f32)
        wv_s = cst.tile([D, D], f32)
        nc.sync.dma_start(out=wq_s[:], in_=wq[:])
        nc.sync.dma_start(out=wk_s[:], in_=wk[:])
        nc.sync.dma_start(out=wv_s[:], in_=wv[:])
        wq_b = cst.tile([D, D], bf16)
        wv_b = cst.tile([D, D], bf16)
        nc.gpsimd.tensor_copy(wq_b[:], wq_s[:])
        nc.gpsimd.tensor_copy(wv_b[:], wv_s[:])
        # sentinel regions (Pool, SBUF-only, one-time)
        nc.gpsimd.memset(tvv[:, 256:], 0.0)
        nc.gpsimd.memset(tvi[:, 256:], float(SENT))
        nc.gpsimd.memset(q_stage[:, :, D:PART], 0.0)
        # last tail chunk is half-height; zero the unwritten rows once
        nc.vector.memset(res[:, NTC - 1, :], 0.0)
        # ghost row of qdram <- zeros
        zrow = cst.tile([1, PART], bf16)
        nc.vector.memset(zrow[:], 0.0)
        nc.gpsimd.dma_start(out=qdram[L:L + 1, :], in_=zrow[:])

        kst = {}
        sel_state = {}
        DLT1 = 2.0 * BOUND / 129.0
        DLT2 = DLT1 / 129.0
        DLTC = [DLT2 / 129.0, DLT2 / 129.0 ** 2, DLT2 / 129.0 ** 3]

        # ====================== phase 1a: transposes + K ======================
        with tc.tile_pool(name="ps_xv", bufs=3, space="PSUM") as ps_xv, \
             tc.tile_pool(name="ps_pj", bufs=2, space="PSUM") as ps_pj:

            def load_tiles(c0):
                # 8 transposes into one 2-bank PSUM tile, one copy per engine
                pxt = ps_xv.tile([PART, 1024], f32, tag="xv")
                for i in range(8):
                    nc.tensor.transpose(out=pxt[0:D, PART * i:PART * (i + 1)],
                                        in_=x_sb[:, c0 + i, :], identity=ident[:])
                xv32 = xT32[:].rearrange("d (p c) -> d c p", c=NT)
                xvb = xTb[:].rearrange("d (p c) -> d c p", c=NT)
                pxt4 = pxt[0:D, :].rearrange("d (c p) -> d c p", c=8)
                nc.vector.tensor_copy(xv32[:, c0:c0 + 8, :], pxt4[:])
                nc.scalar.copy(xvb[:, c0:c0 + 8, :], pxt4[:])

            def proj_k(s):
                sl = slice(512 * s, 512 * (s + 1))
                pk = ps_pj.tile([D, 512], f32, tag="pj")
                for h in range(2):
                    nc.tensor.matmul(out=pk[:, 256 * h:256 * (h + 1)], lhsT=wk_s[:],
                                     rhs=xT32[:, 512 * s + 256 * h:512 * s + 256 * (h + 1)],
                                     start=True, stop=True)
                nc.vector.tensor_copy(kT32[:, sl], pk[:])
                nc.scalar.copy(kTb[:, sl], pk[:])

            nc.sync.dma_start(out=x_sb[:, 0:8, :], in_=x_rc[:, 0:8, :])
            nc.sync.dma_start(out=x_sb[:, 8:32, :], in_=x_rc[:, 8:32, :])
            for s in range(4):
                load_tiles(8 * s)
            for s in range(8):
                proj_k(s)

        # ====================== phase 1b: Q/V + head slab 0 =================
        with tc.tile_pool(name="ps_s0", bufs=2, space="PSUM") as ps_s0, \
             tc.tile_pool(name="ps_qv", bufs=2, space="PSUM") as ps_qv, \
             tc.tile_pool(name="ps_mis", bufs=2, space="PSUM") as ps_mis:

            def proj_qv(c0):
                # 4 chunks of Q and V into one PSUM bank (slots 0-3 Q, 4-7 V)
                pqv = ps_qv.tile([PART, 512], f32, tag="qv")
                pqv8 = pqv[:].rearrange("p (o d) -> p o d", o=8)
                for i in range(4):
                    nc.tensor.matmul(out=pqv8[:, i, :],
                                     lhsT=xTb[:, PART * (c0 + i):PART * (c0 + i + 1)],
                                     rhs=wq_b[:], start=True, stop=True)
                    nc.tensor.matmul(out=pqv8[:, 4 + i, :],
                                     lhsT=xTb[:, PART * (c0 + i):PART * (c0 + i + 1)],
                                     rhs=wv_b[:], start=True, stop=True)
                nc.scalar.copy(q_stage[:, c0:c0 + 4, 0:D], pqv8[:, 0:4, :])
                if (c0 // 4) % 2 == 0:
                    nc.vector.tensor_copy(vp[:, c0:c0 + 4, 0:D], pqv8[:, 4:8, :])
                else:
                    nc.scalar.copy(vp[:, c0:c0 + 4, 0:D], pqv8[:, 4:8, :])



            def emit_kred_a():
                kst['bstats'] = sc.tile([D, 1, 6], f32, tag="bstats", name="bstats")
                nc.vector.bn_stats(kst['bstats'][:, 0, :], kT32[:, 0:512])
                kst['aggr'] = sc.tile([D, 2], f32, tag="aggr", name="aggr")
                nc.vector.bn_aggr(kst['aggr'][:], kst['bstats'][:])
                kst['sig'] = sc.tile([D, 1], f32, tag="sig", name="sig")
                nc.vector.memset(kst['sig'][:], 1.0)
                for _ in range(2):
                    kst['rs'] = sc.tile([D, 1], f32, tag="rs", name="rs")
                    nc.vector.reciprocal(kst['rs'][:], kst['sig'][:])
                    nc.vector.tensor_tensor(out=kst['rs'][:], in0=kst['rs'][:],
                                            in1=kst['aggr'][:, 1:2], op=OP.mult)
                    nc.vector.tensor_tensor(out=kst['rs'][:], in0=kst['rs'][:],
                                            in1=kst['sig'][:], op=OP.add)
                    nc.vector.tensor_scalar_mul(kst['sig'][:], kst['rs'][:], 0.5)
                kst['tk'] = sc.tile([D, 1], f32, tag="tk", name="tk")
                nc.vector.tensor_scalar(out=kst['tk'][:], in0=kst['sig'][:],
                                        scalar1=float(Z), scalar2=None, op0=OP.mult)
                nc.gpsimd.tensor_tensor(out=kst['tk'][:], in0=kst['tk'][:],
                                        in1=kst['aggr'][:, 0:1], op=OP.add)
                HL = L // 2
                kst['cnt_c'] = sc.tile([D, 1], f32, tag="cnt_c", name="cnt_c")
                kst['cnt_p'] = sc.tile([D, 1], f32, tag="cnt_p", name="cnt_p")
                nc.vector.tensor_scalar(out=pt_c[0:D, 0:4, :], in0=kT32[:, 0:HL],
                                        scalar1=kst['tk'][:, 0:1], scalar2=None,
                                        op0=OP.is_gt, op1=OP.add,
                                        accum_out=kst['cnt_c'][:])
                nc.gpsimd.tensor_scalar(out=pt_c[0:D, 4:8, :], in0=kT32[:, HL:L],
                                        scalar1=kst['tk'][:, 0:1], scalar2=None,
                                        op0=OP.is_gt)
                nc.vector.tensor_scalar(out=pt_c[0:D, 8:12, :],
                                        in0=pt_c[0:D, 4:8, :],
                                        scalar1=1.0, scalar2=None,
                                        op0=OP.mult, op1=OP.add,
                                        accum_out=kst['cnt_p'][:])
                nc.gpsimd.tensor_tensor(out=kst['cnt_c'][:], in0=kst['cnt_c'][:],
                                        in1=kst['cnt_p'][:], op=OP.add)
                kst['adj'] = sc.tile([D, 1], f32, tag="adj", name="adj")
                nc.gpsimd.tensor_scalar(out=kst['adj'][:], in0=kst['cnt_c'][:],
                                        scalar1=float(-LQ), scalar2=1.0 / (L * PHI),
                                        op0=OP.add, op1=OP.mult)
                nc.gpsimd.tensor_tensor(out=kst['adj'][:], in0=kst['adj'][:],
                                        in1=kst['sig'][:], op=OP.mult)
                kst['t1'] = sc.tile([D, 1], f32, tag="t1", name="t1")
                nc.gpsimd.tensor_tensor(out=kst['t1'][:], in0=kst['tk'][:],
                                        in1=kst['adj'][:], op=OP.add)

            def emit_kred_b():
                HL = L // 2
                kst['s1c'] = sc.tile([D, 1], f32, tag="s1c", name="s1c")
                kst['s1p'] = sc.tile([D, 1], f32, tag="s1p", name="s1p")
                nc.vector.tensor_scalar(out=pt_c[0:D, 0:4, :], in0=kT32[:, 0:HL],
                                        scalar1=kst['t1'][:, 0:1],
                                        scalar2=0.0, op0=OP.subtract, op1=OP.max)
                nc.gpsimd.tensor_scalar(out=pt_c[0:D, 4:8, :], in0=kT32[:, HL:L],
                                        scalar1=kst['t1'][:, 0:1],
                                        scalar2=0.0, op0=OP.subtract, op1=OP.max)
                nc.vector.tensor_scalar(out=pt_c[0:D, 8:12, :],
                                        in0=pt_c[0:D, 0:4, :],
                                        scalar1=1.0, scalar2=None,
                                        op0=OP.mult, op1=OP.add,
                                        accum_out=kst['s1c'][:])
                nc.vector.tensor_scalar(out=pt_c[0:D, 8:12, :],
                                        in0=pt_c[0:D, 4:8, :],
                                        scalar1=1.0, scalar2=None,
                                        op0=OP.mult, op1=OP.add,
                                        accum_out=kst['s1p'][:])
                nc.gpsimd.tensor_tensor(out=kst['s1c'][:], in0=kst['s1c'][:],
                                        in1=kst['s1p'][:], op=OP.add)
                nc.gpsimd.tensor_scalar(out=kr[:], in0=kst['s1c'][:],
                                        scalar1=1.0 / LQ, scalar2=None, op0=OP.mult)
                nc.gpsimd.tensor_tensor(out=kr[:], in0=kr[:], in1=kst['t1'][:],
                                        op=OP.add)

            def emit_sqk():
                pwt = ps_mis.tile([PART, PART], f32, tag="mis", name="mis")
                nc.tensor.transpose(out=pwt[0:D, 0:D], in_=wq_s[:],
                                    identity=ident[0:D, 0:D])
                wqT = sc.tile([D, D], f32, tag="wqT")
                nc.vector.tensor_copy(wqT[:], pwt[0:D, 0:D])
                pw = ps_mis.tile([PART, PART], f32, tag="mis", name="mis")
                nc.tensor.matmul(out=pw[0:D, 0:1], lhsT=wqT[:], rhs=kr[:],
                                 start=True, stop=True)
                nc.vector.tensor_copy(wvec[:], pw[0:D, 0:1])
                psq = ps_mis.tile([PART, PART], f32, tag="mis", name="mis")
                for c in range(NT):
                    nc.tensor.matmul(out=psq[:, c:c + 1],
                                     lhsT=xT32[:, PART * c:PART * (c + 1)],
                                     rhs=wvec[:], start=True, stop=True)
                nc.vector.tensor_copy(sqk[:], psq[:, 0:NT])
                psqT = ps_mis.tile([PART, PART], f32, tag="mis", name="mis")
                nc.tensor.transpose(out=psqT[0:NT, 0:PART], in_=sqk[:],
                                    identity=ident[:])
                sqkT = sc.tile([NT, PART], f32, tag="sqkT")
                nc.vector.tensor_copy(sqkT[:], psqT[0:NT, 0:PART])
                nc.sync.dma_start(out=scr_row[:], in_=sqkT[:])
                # replicate in two halves on two queues so pass 1 starts early
                nc.sync.dma_start(out=sqk_rep[:, 0:L // 2],
                                  in_=scr_row[0:1, 0:L // 2].to_broadcast([PART, L // 2]))
                nc.gpsimd.dma_start(out=sqk_rep[:, L // 2:L],
                                    in_=scr_row[0:1, L // 2:L].to_broadcast([PART, L // 2]))
                # prefetch the wrapped view for the compact stage
                nc.sync.dma_start(out=sqk16[:], in_=scr_row[0, :].rearrange(
                    "(f p) -> p f", p=16))

            def emit_meanv():
                pmv = ps_mis.tile([PART, PART], f32, tag="mis", name="mis")
                for c in range(NT):
                    nc.tensor.matmul(out=pmv[0:D + 1, 0:1], lhsT=vp[:, c, :],
                                     rhs=onesb[:], start=(c == 0), stop=(c == NT - 1))
                mv_col = sc.tile([D, 1], f32, tag="mv_col")
                nc.vector.tensor_scalar_mul(mv_col[:], pmv[0:D, 0:1], 1.0 / L)
                pmvT = ps_mis.tile([PART, PART], f32, tag="mis", name="mis")
                nc.tensor.transpose(out=pmvT[0:1, 0:D], in_=mv_col[:],
                                    identity=ident[0:D, 0:D])
                mv_row = sc.tile([1, D], f32, tag="mv_row")
                nc.vector.tensor_copy(mv_row[:], pmvT[0:1, 0:D])
                pmvF = ps_mis.tile([PART, PART], f32, tag="mis", name="mis")
                nc.tensor.matmul(out=pmvF[:, 0:D], lhsT=ones1x128[:], rhs=mv_row[:],
                                 start=True, stop=True)
                nc.vector.tensor_copy(mvf[:], pmvF[:, 0:D])
                # meanV pre-fill of all tail rows (before any scatter: Tile
                # orders the overlapping out-tensor writes); Pool SWDGE queue
                if 'fill' not in KSKIP:
                    nc.gpsimd.dma_start(
                        out=out_re[:],
                        in_=mvf[:].rearrange("p d -> p () d").to_broadcast(
                            [PART, NT, D]))

            # ---- phase-1b emission schedule ----
            emit_kred_a()          # DVE/Pool, needs full kT32
            for g in range(8):
                proj_qv(4 * g)
            emit_kred_b()
            nc.vector.memset(vp[:, :, D:D + 1], 1.0)
            emit_sqk()
            # stage Q to DRAM in quarters (SP picks scr_row/sqk_rep first)
            for qq in range(4):
                nc.sync.dma_start(out=qd_re[:, 8 * qq:8 * (qq + 1), :],
                                  in_=q_stage[:, 8 * qq:8 * (qq + 1), :])
            emit_meanv()

        # ====================== selection pieces (emitted into phase 2) ====
        def sel_pass_init():
            lo = mn.tile([PART, 1], f32, tag="lo_a")
            nc.vector.memset(lo[:], -BOUND)
            sel_state["lo"] = lo

        def sel_pass_full(it):
            lo = sel_state["lo"]
            dlt = DLT1 if it == 0 else DLT2
            if it == 0:
                tvec = tvec1
            else:
                tvec = mn.tile([PART, 1], f32, tag=f"tv{it % 2}")
                nc.gpsimd.tensor_scalar(out=tvec[:], in0=pidx1[:], scalar1=float(dlt),
                                        scalar2=None, op0=OP.mult)
                nc.gpsimd.tensor_tensor(out=tvec[:], in0=tvec[:], in1=lo[:], op=OP.add)
            HL = L // 2
            cntq = mn.tile([PART, 1], f32, tag="cntq")
            cntp = mn.tile([PART, 1], f32, tag="cntp")
            nc.vector.tensor_scalar(out=pt_c[:, 0:4, :], in0=sqk_rep[:, 0:HL],
                                    scalar1=tvec[:, 0:1], scalar2=None,
                                    op0=OP.is_gt, op1=OP.add, accum_out=cntq[:])
            nc.gpsimd.tensor_scalar(out=pt_c[:, 4:8, :], in0=sqk_rep[:, HL:L],
                                    scalar1=tvec[:, 0:1], scalar2=None,
                                    op0=OP.is_gt)
            nc.vector.tensor_scalar(out=pt_c[:, 8:12, :], in0=pt_c[:, 4:8, :],
                                    scalar1=1.0, scalar2=None,
                                    op0=OP.mult, op1=OP.add, accum_out=cntp[:])
            nc.gpsimd.tensor_tensor(out=cntq[:], in0=cntq[:], in1=cntp[:], op=OP.add)
            sel = mn.tile([PART, 1], f32, tag="sel")
            nc.gpsimd.tensor_scalar(out=sel[:], in0=cntq[:], scalar1=float(LQ),
                                    scalar2=None, op0=OP.is_ge)
            jsr = mn.tile([PART, 1], f32, tag="jsr")
            nc.gpsimd.partition_all_reduce(jsr[:], sel[:], channels=PART,
                                           reduce_op=bass_isa.ReduceOp.add)
            nlo = mn.tile([PART, 1], f32, tag=f"lo_{'b' if it % 2 == 0 else 'a'}")
            nc.gpsimd.tensor_scalar(out=jsr[:], in0=jsr[:], scalar1=float(dlt),
                                    scalar2=None, op0=OP.mult)
            nc.gpsimd.tensor_tensor(out=nlo[:], in0=lo[:], in1=jsr[:], op=OP.add)
            sel_state["lo"] = nlo

        def sel_compact():
            lo2 = sel_state["lo"]
            hi2 = mn.tile([PART, 1], f32, tag="hi2")
            nc.gpsimd.tensor_scalar(out=hi2[:], in0=lo2[:], scalar1=float(DLT2),
                                    scalar2=None, op0=OP.add)
            # c_hi = count(sqk > hi2) exact on the [128, 32] tile
            j32 = sc.tile([PART, NT], f32, tag="j32")
            chi = sc.tile([PART, 1], f32, tag="chi")
            nc.vector.tensor_scalar(out=j32[:], in0=sqk[:],
                                    scalar1=hi2[:, 0:1], scalar2=None,
                                    op0=OP.is_gt, op1=OP.add, accum_out=chi[:])
            nc.gpsimd.partition_all_reduce(chi[:], chi[:], channels=PART,
                                           reduce_op=bass_isa.ReduceOp.add)
            rvec = sc.tile([PART, 1], f32, tag="rvec")
            nc.gpsimd.tensor_scalar(out=rvec[:], in0=chi[:], scalar1=-1.0,
                                    scalar2=float(LQ), op0=OP.mult, op1=OP.add)
            sel_state["rvec"] = rvec
            # tv = (sqk-lo2)*b + (b-1), b = (sqk <= hi2); in-interval -> v'>0
            a16 = sc.tile([16, 256], f32, tag="a16")
            nc.gpsimd.tensor_scalar(out=a16[:], in0=sqk16[:],
                                    scalar1=lo2[0:16, 0:1], scalar2=None,
                                    op0=OP.subtract)
            b16 = sc.tile([16, 256], f32, tag="b16")
            nc.gpsimd.tensor_scalar(out=b16[:], in0=sqk16[:],
                                    scalar1=hi2[0:16, 0:1], scalar2=None,
                                    op0=OP.is_le)
            nc.gpsimd.tensor_tensor(out=tvv[:, 0:256], in0=a16[:], in1=b16[:],
                                    op=OP.mult)
            nc.gpsimd.tensor_scalar(out=b16[:], in0=b16[:], scalar1=1.0,
                                    scalar2=None, op0=OP.subtract)
            nc.gpsimd.tensor_tensor(out=tvv[:, 0:256], in0=tvv[:, 0:256],
                                    in1=b16[:], op=OP.add)
            # compact; appended 0.0 sentinels guarantee the first 64 output
            # slots are always hardware-written (no num_found round-trip)
            nc.gpsimd.sparse_gather(valc[:], tvv[:], num_found=nfdummy[:])
            nc.gpsimd.dma_start(out=scr_cmp[0, :].rearrange("(f p) -> p f", p=16),
                                in_=valc[:, 0:4])
            nc.gpsimd.dma_start(out=valrep[:],
                                in_=scr_cmp[:].to_broadcast([PART, 64]))
            loc = mn.tile([PART, 1], f32, tag="loc_a")
            nc.gpsimd.memset(loc[:], 0.0)
            sel_state["loc"] = loc

        def sel_pass_c(it):
            loc, rvec = sel_state["loc"], sel_state["rvec"]
            dlt = DLTC[it]
            tvec = mn.tile([PART, 1], f32, tag=f"tvc{it % 2}")
            nc.gpsimd.tensor_scalar(out=tvec[:], in0=pidx1[:], scalar1=float(dlt),
                                    scalar2=None, op0=OP.mult)
            nc.gpsimd.tensor_tensor(out=tvec[:], in0=tvec[:], in1=loc[:], op=OP.add)
            cntq = mn.tile([PART, 1], f32, tag="cntqc")
            nc.vector.tensor_scalar(out=cmpc[:], in0=valrep[:],
                                    scalar1=tvec[:, 0:1], scalar2=None,
                                    op0=OP.is_gt, op1=OP.add, accum_out=cntq[:])
            sel = mn.tile([PART, 1], f32, tag="selc")
            nc.gpsimd.tensor_scalar(out=sel[:], in0=cntq[:],
                                    scalar1=rvec[:, 0:1], scalar2=None,
                                    op0=OP.is_ge)
            jsr = mn.tile([PART, 1], f32, tag="jsrc")
            nc.gpsimd.partition_all_reduce(jsr[:], sel[:], channels=PART,
                                           reduce_op=bass_isa.ReduceOp.add)
            nlo = mn.tile([PART, 1], f32, tag=f"loc_{'b' if it % 2 == 0 else 'a'}")
            nc.gpsimd.tensor_scalar(out=jsr[:], in0=jsr[:], scalar1=float(dlt),
                                    scalar2=None, op0=OP.mult)
            nc.gpsimd.tensor_tensor(out=nlo[:], in0=loc[:], in1=jsr[:], op=OP.add)
            sel_state["loc"] = nlo


        def emit_idx():
            # tail-selected ordered positions, sentinel-padded to NTAIL
            lo2, loc = sel_state["lo"], sel_state["loc"]
            a16 = sc.tile([16, 256], f32, tag="ai16")
            nc.gpsimd.tensor_scalar(out=a16[:], in0=sqk16[:],
                                    scalar1=lo2[0:16, 0:1], scalar2=None,
                                    op0=OP.subtract)
            b16 = sc.tile([16, 256], f32, tag="bi16")
            nc.gpsimd.tensor_scalar(out=b16[:], in0=a16[:],
                                    scalar1=loc[0:16, 0:1], scalar2=None,
                                    op0=OP.is_gt)
            nc.gpsimd.tensor_tensor(out=tvi[:, 0:256], in0=iot1[:], in1=b16[:],
                                    op=OP.mult)
            nc.gpsimd.tensor_scalar(out=tvi[:, 0:256], in0=tvi[:, 0:256],
                                    scalar1=1.0, scalar2=None, op0=OP.subtract)
            nc.gpsimd.sparse_gather(idxw[:], tvi[:], num_found=nfdummy[:])
            nc.gpsimd.tensor_copy(idx16w[0:16, :], idxw[:, 0:NTC * PART // 16])
            nc.gpsimd.tensor_copy(idx32w[:], idxw[:, 0:NTC * PART // 16])
            nc.sync.dma_start(out=scr_i32[0, :].rearrange("(s p) -> p s", p=16),
                              in_=idx32w[:])
            # replicate wrapped idx to all 8 core blocks, SBUF->SBUF
            for g in range(1, 8):
                eng = nc.sync if g % 2 == 0 else nc.gpsimd
                eng.dma_start(out=idx16w[16 * g:16 * (g + 1), :],
                              in_=idx16w[0:16, :])
            # chunk-layout scatter offsets
            nc.sync.dma_start(out=idxo[:],
                              in_=scr_i32[0, :].rearrange("(c p) -> p c", p=PART))

        # ====================== phase 2 ======================
        with tc.tile_pool(name="ps_strip", bufs=2, space="PSUM") as ps_strip, \
             tc.tile_pool(name="ps_vstrip", bufs=1, space="PSUM") as ps_vstrip, \
             tc.tile_pool(name="ps_av", bufs=2, space="PSUM") as ps_av:

            def pt_of(s):
                return (pt_a, pt_b, pt_c)[s % 3]

            def score_group(rhs_ap, qlen, ptc, gi, dve):
                g0, glen = GROUPS[gi]
                if dve:
                    strip = ps_vstrip.tile([PART, 2, 512], f32, tag="vstrip")
                else:
                    strip = ps_strip.tile([PART, 2, 512], f32, tag="strip")
                for i in range(glen):
                    j = g0 + i
                    nc.tensor.matmul(out=strip[:, i, 0:qlen],
                                     lhsT=kTb[:, PART * j:PART * (j + 1)],
                                     rhs=rhs_ap, start=True, stop=True)
                if dve:
                    nc.vector.tensor_scalar(
                        out=ptc[:, g0:g0 + glen, 0:qlen].bitcast(i16),
                        in0=strip[:, 0:glen, 0:qlen], scalar1=SCH_A, scalar2=SCH_B,
                        op0=OP.mult, op1=OP.add)
                else:
                    nc.scalar.activation(out=ptc[:, g0:g0 + glen, 0:qlen],
                                         in_=strip[:, 0:glen, 0:qlen], func=AF.Exp,
                                         scale=0.125)

            def av_subtile(ptp, u, qn, rchunk):
                # qn rows of AV for query sub-tile u of the previous slab
                av = ps_av.tile([PART, PART], f32, tag="av")
                for j in range(NT):
                    nc.tensor.matmul(out=av[0:qn, 0:D + 1],
                                     lhsT=ptp[:, j, PART * u:PART * u + qn],
                                     rhs=vp[:, j, :],
                                     start=(j == 0), stop=(j == NT - 1))
                rec = mn.tile([PART, 1], f32, tag="rec")
                nc.vector.reciprocal_approx_fast(rec[0:qn, :], av[0:qn, D:D + 1])
                nc.vector.tensor_scalar(out=res[0:qn, rchunk, :], in0=av[0:qn, 0:D],
                                        scalar1=rec[0:qn, 0:1], scalar2=None,
                                        op0=OP.mult)
                # scatter is an ADD onto the meanV fill: emit attn - meanV
                nc.vector.tensor_tensor(out=res[0:qn, rchunk, :],
                                        in0=res[0:qn, rchunk, :], in1=mvf[0:qn, :],
                                        op=OP.subtract)

            def emit_gather(ts):
                glen = 512 if ts < 5 else 256
                qsel = qs.tile([PART, 1, 512], bf16, tag="qsel")
                nc.gpsimd.dma_gather(
                    qsel[:, :, 0:glen], qdram[:],
                    idx16w[:, 32 * ts:32 * ts + glen // 16],
                    glen, glen, PART, elem_step=PART, transpose=True)
                return qsel

            def emit_scatter(ts):
                # scatter-add slab results (attn - meanV) onto the meanV fill
                if 'scat' in KSKIP:
                    return
                c0 = 4 * ts
                nq = TAIL_LENS[ts]
                nc.gpsimd.dma_scatter_add(
                    out[:], res[:, c0:c0 + (nq + 127) // 128, :],
                    idx16w[:, 32 * ts:32 * ts + (nq + 15) // 16],
                    nq, nq, D, elem_step=D)

            # ---- selection (uncontended), then gathered slabs ----
            qsel_tiles = {}
            sel_pass_init()
            sel_pass_full(0)
            sel_pass_full(1)
            sel_compact()
            sel_pass_c(0)
            sel_pass_c(1)
            sel_pass_c(2)
            emit_idx()

            def emit_tail_slab(ts):
                # scores+exp for slab ts; AV of previous slab interleaved
                qlen = TAIL_LENS[ts]
                ptc = pt_of(ts)
                qsel = qsel_tiles[ts]
                prev_av = []
                if ts > 0:
                    pq = TAIL_LENS[ts - 1]
                    ptp = pt_of(ts - 1)
                    prev_av = [(ptp, u, min(PART, pq - PART * u), 4 * (ts - 1) + u)
                               for u in range((pq + 127) // 128)]
                avi = 0
                for gi in range(len(GROUPS)):
                    dve = gi in DVE_GROUPS_TAIL
                    score_group(qsel[0:D, 0, 0:qlen], qlen, ptc, gi, dve)
                    if gi in (3, 6, 9, 12) and avi < len(prev_av):
                        av_subtile(*prev_av[avi])
                        avi += 1
                for a in prev_av[avi:]:
                    av_subtile(*a)

            qsel_tiles[0] = emit_gather(0)
            qsel_tiles[1] = emit_gather(1)
            emit_tail_slab(0)
            qsel_tiles[2] = emit_gather(2)
            emit_tail_slab(1)
            emit_scatter(0)
            qsel_tiles[3] = emit_gather(3)
            emit_tail_slab(2)
            emit_scatter(1)
            qsel_tiles[4] = emit_gather(4)
            emit_tail_slab(3)
            emit_scatter(2)
            qsel_tiles[5] = emit_gather(5)
            emit_tail_slab(4)
            emit_scatter(3)
            emit_tail_slab(5)
            emit_scatter(4)
            # AV for the last slab
            pq = TAIL_LENS[5]
            ptp = pt_of(5)
            for u in range((pq + 127) // 128):
                av_subtile(ptp, u, min(PART, pq - PART * u), 20 + u)
            emit_scatter(5)


# revision 29
# speedup vs baseline: 1.2156x; 1.0652x over previous
"""Trainium2 Bass kernel for nn_AttentionBlock_33724083208839 (sparse_attention).

Data-parallel over batch (8 batches -> 8 cores). Hybrid selected-only design:
  1. load x, PE-transpose -> xT32 (f32, exact) + xTb (bf16); K projected in
     exact f32 (feeds selection); Q/V projected in bf16; Q also staged
     row-major to DRAM (qdram [4104,128] bf16, padded cols + zero ghost row).
  2. selection (exact, baseline machinery, DVE+Pool split halves): K_reduce
     via CVaR identity; sqk exact on PE; threshold via two 128-ary counting
     passes on a DMA-replicated sqk + sparse_gather compaction (sentinel
     padding instead of num_found round-trips) + three compact passes.
  3. head: attention for ALL queries of slabs 0-1 (1024 queries) overlaps
     selection latency; blend non-selected head rows to meanV (Pool
     arithmetic), direct DMA out.
  4. tail: positions 1024..4095 -> compact ordered index list of selected
     queries (sparse_gather over masked iota, padded to 2240 with sentinel
     4096 by appending always-gathered sentinel values); dma_gather
     (transpose mode) pulls Q_sel^T tiles from qdram; 4.375 slabs of
     scores->exp(ACT/DVE Schraudolph)->AV; results scattered to out rows by
     indirect DMA (sentinels skipped via bounds_check); rows 1024..4095
     pre-filled with meanV by chunk DMAs before the scatters.
"""
import os
import sys

sys.path.insert(0, "/opt/trn_rl_repo")

import math
from statistics import NormalDist

KSKIP = set(os.environ.get('KSKIP', '').split(','))

import numpy as np

import concourse.bacc as bacc
import concourse.bass as bass
import concourse.bass_isa as bass_isa
import concourse.mybir as mybir
from concourse.tile import TileContext
from concourse.masks import make_identity
from concourse.bass_utils import run_bass_kernel_spmd

B, L, D = 8, 4096, 64
LQ = int((1.0 - 0.33) * L)  # 2744
PART = 128
NT = L // PART          # 32 key tiles / x chunks
N_CORES = 8

NTAIL = 2816            # padded compact count (LQ=2744 exact + sentinels)
NTC = 22                # compact chunks
TAIL_LENS = [512, 512, 512, 512, 512, 256]
SENT = L                # sentinel index -> qdram ghost row, scatter-skipped
QD_ROWS = L + 8         # qdram rows (ghost row at L)
N_VSENT = 64            # value-compaction sentinel count
N_ISENT = 592           # index-compaction sentinels (need >= NTAIL - LQ = 72)

QFRAC = 1.0 - LQ / L
Z = NormalDist().inv_cdf(QFRAC)
PHI = math.exp(-Z * Z / 2.0) / math.sqrt(2.0 * math.pi)

f32 = mybir.dt.float32
bf16 = mybir.dt.bfloat16
u8 = mybir.dt.uint8
i16 = mybir.dt.int16
i32 = mybir.dt.int32
u32 = mybir.dt.uint32
AF = mybir.ActivationFunctionType
OP = mybir.AluOpType

BOUND = 512.0

# Schraudolph exp for bf16 bit patterns: bf16_bits(exp(s/8)) ~= A*s + B.
SCH_A = 128.0 * math.log2(math.e) / 8.0
SCH_B = 16256.0 + 0.5 - 128.0 * math.log2(1.0407)

GROUPS = [(g, 2) for g in range(0, NT, 2)]

# head slabs: DVE takes the last 3 pair-groups (it is busy with selection
# until ~2/3 through each head slab); tail slabs: DVE takes 6.
DVE_GROUPS_TAIL = {3, 6, 8, 11, 14}


def build(debug: bool = False):
    nc = bacc.Bacc("TRN2")
    x = nc.dram_tensor("x", [L, D], f32, kind="ExternalInput")
    wq = nc.dram_tensor("Wq", [D, D], f32, kind="ExternalInput")
    wk = nc.dram_tensor("Wk", [D, D], f32, kind="ExternalInput")
    wv = nc.dram_tensor("Wv", [D, D], f32, kind="ExternalInput")
    out = nc.dram_tensor("out", [L + 8, D], f32, kind="ExternalOutput")
    scr_row = nc.dram_tensor("scr_row", [1, L], f32, kind="Internal")
    scr_cmp = nc.dram_tensor("scr_cmp", [1, 64], f32, kind="Internal")
    qdram = nc.dram_tensor("qdram", [QD_ROWS, PART], bf16, kind="Internal")
    scr_i32 = nc.dram_tensor("scr_i32", [1, NTC * PART], i32, kind="Internal")

    x_re = x[:].rearrange("(c p) d -> p c d", p=PART)
    # partition-contiguous x view: one 8KB/partition DMA
    x_rc = x[:].rearrange("(p c) d -> p c d", c=NT)
    out_re = out[0:L, :].rearrange("(c p) d -> p c d", p=PART)
    qd_re = qdram[0:L, :].rearrange("(c p) e -> p c e", p=PART)

    with TileContext(nc) as tc, \
         tc.tile_pool(name="cst", bufs=1) as cst, \
         tc.tile_pool(name="big", bufs=1) as big, \
         tc.tile_pool(name="sc", bufs=1) as sc, \
         tc.tile_pool(name="mn", bufs=2) as mn, \
         tc.tile_pool(name="qs", bufs=3) as qs:

        # ---- warm the exp activation table immediately ----
        warm = cst.tile([1, 8], f32)
        nc.vector.memset(warm[:], 0.0)
        warm2 = cst.tile([1, 8], f32)
        nc.scalar.activation(out=warm2[:], in_=warm[:], func=AF.Exp)

        # ---- constants ----
        ident = cst.tile([PART, PART], f32)
        make_identity(nc, ident[:])
        onesb = cst.tile([PART, 1], bf16)
        nc.vector.memset(onesb[:], 1.0)
        ones1x128 = cst.tile([1, PART], f32)
        nc.vector.memset(ones1x128[:], 1.0)
        pidx1i = cst.tile([PART, 1], i32)
        nc.gpsimd.iota(pidx1i[:], pattern=[[1, 1]], base=1, channel_multiplier=1)
        pidx1 = cst.tile([PART, 1], f32)
        nc.vector.tensor_copy(pidx1[:], pidx1i[:])
        # pass-1 thresholds are compile-time: tvec1 = pidx1*DLT1 - BOUND
        tvec1 = cst.tile([PART, 1], f32)
        nc.gpsimd.tensor_scalar(out=tvec1[:], in0=pidx1[:],
                                scalar1=2.0 * BOUND / 129.0, scalar2=-BOUND,
                                op0=OP.mult, op1=OP.add)
        # iot1[p, f] = (f*16 + p) + 1 = original position + 1, wrapped layout
        iot1i = cst.tile([16, 256], i32)
        nc.gpsimd.iota(iot1i[:], pattern=[[16, 256]], base=1, channel_multiplier=1)
        iot1 = cst.tile([16, 256], f32)
        nc.vector.tensor_copy(iot1[:], iot1i[:])

        # ---- persistent tensors ----
        x_sb = big.tile([PART, NT, D], f32)
        # x_sb is dead once the transposes finish; reuse it for res
        res = x_sb
        xT32 = big.tile([D, L], f32)
        xTb = big.tile([D, L], bf16)
        kT32 = big.tile([D, L], f32)
        kTb = big.tile([D, L], bf16)
        q_stage = big.tile([PART, NT, PART], bf16)
        vp = big.tile([PART, NT, D + 1], bf16)
        pt_a = big.tile([PART, NT, 512], bf16)
        pt_b = big.tile([PART, NT, 512], bf16)
        pt_c = big.tile([PART, NT, 512], bf16)
        mvf = big.tile([PART, D], f32)
        sqk = big.tile([PART, NT], f32)
        kr = big.tile([D, 1], f32)
        wvec = big.tile([D, 1], f32)
        sqk_rep = big.tile([PART, L], f32)
        sqk16 = big.tile([16, 256], f32)
        valrep = big.tile([PART, 64], f32)
        cmpc = big.tile([PART, 64], bf16)
        # value compaction: input [16, 256+4] (tv || 0.0-sentinels)
        tvv = big.tile([16, 256 + N_VSENT // 16], f32)
        valc = big.tile([16, 20], f32)
        nfdummy = big.tile([1, 1], u32)
        # index compaction: input [16, 256+10] (tvidx || 4096.0-sentinels)
        tvi = big.tile([16, 256 + N_ISENT // 16], f32)
        idxw = big.tile([16, 209], f32)
        idx16w = big.tile([PART, NTC * PART // 16], i16)
        idx32w = big.tile([16, NTC * PART // 16], i32)
        idxo = big.tile([PART, NTC], i32)

        # weights
        wq_s = cst.tile([D, D], f32)
        wk_s = cst.tile([D, D], f32)
        wv_s = cst.tile([D, D], f32)
        nc.sync.dma_start(out=wq_s[:], in_=wq[:])
        nc.sync.dma_start(out=wk_s[:], in_=wk[:])
        nc.sync.dma_start(out=wv_s[:], in_=wv[:])
        wq_b = cst.tile([D, D], bf16)
        wv_b = cst.tile([D, D], bf16)
        nc.gpsimd.tensor_copy(wq_b[:], wq_s[:])
        nc.gpsimd.tensor_copy(wv_b[:], wv_s[:])
        # sentinel regions (Pool, SBUF-only, one-time)
        nc.gpsimd.memset(tvv[:, 256:], 0.0)
        nc.gpsimd.memset(tvi[:, 256:], float(SENT))
        nc.gpsimd.memset(q_stage[:, :, D:PART], 0.0)
        # last tail chunk is half-height; zero the unwritten rows once
        nc.vector.memset(res[:, NTC - 1, :], 0.0)
        # ghost row of qdram <- zeros
        zrow = cst.tile([1, PART], bf16)
        nc.vector.memset(zrow[:], 0.0)
        nc.gpsimd.dma_start(out=qdram[L:L + 1, :], in_=zrow[:])

        kst = {}
        sel_state = {}
        DLT1 = 2.0 * BOUND / 129.0
        DLT2 = DLT1 / 129.0
        DLTC = [DLT2 / 129.0, DLT2 / 129.0 ** 2, DLT2 / 129.0 ** 3]

        # ====================== phase 1a: transposes + K ======================
        with tc.tile_pool(name="ps_xv", bufs=3, space="PSUM") as ps_xv, \
             tc.tile_pool(name="ps_pj", bufs=2, space="PSUM") as ps_pj:

            def load_tiles(c0):
                # 8 transposes into one 2-bank PSUM tile, one copy per engine
                pxt = ps_xv.tile([PART, 1024], f32, tag="xv")
                for i in range(8):
                    nc.tensor.transpose(out=pxt[0:D, PART * i:PART * (i + 1)],
                                        in_=x_sb[:, c0 + i, :], identity=ident[:])
                xv32 = xT32[:].rearrange("d (p c) -> d c p", c=NT)
                xvb = xTb[:].rearrange("d (p c) -> d c p", c=NT)
                pxt4 = pxt[0:D, :].rearrange("d (c p) -> d c p", c=8)
                nc.vector.tensor_copy(xv32[:, c0:c0 + 8, :], pxt4[:])
                nc.scalar.copy(xvb[:, c0:c0 + 8, :], pxt4[:])

            def proj_k(s):
                sl = slice(512 * s, 512 * (s + 1))
                pk = ps_pj.tile([D, 512], f32, tag="pj")
                for h in range(2):
                    nc.tensor.matmul(out=pk[:, 256 * h:256 * (h + 1)], lhsT=wk_s[:],
                                     rhs=xT32[:, 512 * s + 256 * h:512 * s + 256 * (h + 1)],
                                     start=True, stop=True)
                nc.vector.tensor_copy(kT32[:, sl], pk[:])
                nc.scalar.copy(kTb[:, sl], pk[:])

            nc.sync.dma_start(out=x_sb[:, 0:8, :], in_=x_rc[:, 0:8, :])
            nc.sync.dma_start(out=x_sb[:, 8:32, :], in_=x_rc[:, 8:32, :])
            for s in range(4):
                load_tiles(8 * s)
            for s in range(8):
                proj_k(s)

        # ====================== phase 1b: Q/V + head slab 0 =================
        with tc.tile_pool(name="ps_s0", bufs=2, space="PSUM") as ps_s0, \
             tc.tile_pool(name="ps_qv", bufs=2, space="PSUM") as ps_qv, \
             tc.tile_pool(name="ps_mis", bufs=2, space="PSUM") as ps_mis:

            def proj_qv(c0):
                # 4 chunks of Q and V into one PSUM bank (slots 0-3 Q, 4-7 V)
                pqv = ps_qv.tile([PART, 512], f32, tag="qv")
                pqv8 = pqv[:].rearrange("p (o d) -> p o d", o=8)
                for i in range(4):
                    nc.tensor.matmul(out=pqv8[:, i, :],
                                     lhsT=xTb[:, PART * (c0 + i):PART * (c0 + i + 1)],
                                     rhs=wq_b[:], start=True, stop=True)
                    nc.tensor.matmul(out=pqv8[:, 4 + i, :],
                                     lhsT=xTb[:, PART * (c0 + i):PART * (c0 + i + 1)],
                                     rhs=wv_b[:], start=True, stop=True)
                nc.scalar.copy(q_stage[:, c0:c0 + 4, 0:D], pqv8[:, 0:4, :])
                if (c0 // 4) % 2 == 0:
                    nc.vector.tensor_copy(vp[:, c0:c0 + 4, 0:D], pqv8[:, 4:8, :])
                else:
                    nc.scalar.copy(vp[:, c0:c0 + 4, 0:D], pqv8[:, 4:8, :])



            def emit_kred_a():
                kst['bstats'] = sc.tile([D, 8, 6], f32, tag="bstats", name="bstats")
                for a in range(8):
                    nc.vector.bn_stats(kst['bstats'][:, a, :],
                                       kT32[:, 512 * a:512 * (a + 1)])
                kst['aggr'] = sc.tile([D, 2], f32, tag="aggr", name="aggr")
                nc.vector.bn_aggr(kst['aggr'][:], kst['bstats'][:])
                kst['sig'] = sc.tile([D, 1], f32, tag="sig", name="sig")
                nc.vector.memset(kst['sig'][:], 1.0)
                for _ in range(2):
                    kst['rs'] = sc.tile([D, 1], f32, tag="rs", name="rs")
                    nc.vector.reciprocal(kst['rs'][:], kst['sig'][:])
                    nc.vector.tensor_tensor(out=kst['rs'][:], in0=kst['rs'][:],
                                            in1=kst['aggr'][:, 1:2], op=OP.mult)
                    nc.vector.tensor_tensor(out=kst['rs'][:], in0=kst['rs'][:],
                                            in1=kst['sig'][:], op=OP.add)
                    nc.vector.tensor_scalar_mul(kst['sig'][:], kst['rs'][:], 0.5)
                kst['tk'] = sc.tile([D, 1], f32, tag="tk", name="tk")
                nc.vector.tensor_scalar(out=kst['tk'][:], in0=kst['sig'][:],
                                        scalar1=float(Z), scalar2=None, op0=OP.mult)
                nc.gpsimd.tensor_tensor(out=kst['tk'][:], in0=kst['tk'][:],
                                        in1=kst['aggr'][:, 0:1], op=OP.add)
                HL = L // 2
                kst['cnt_c'] = sc.tile([D, 1], f32, tag="cnt_c", name="cnt_c")
                kst['cnt_p'] = sc.tile([D, 1], f32, tag="cnt_p", name="cnt_p")
                nc.vector.tensor_scalar(out=pt_c[0:D, 0:4, :], in0=kT32[:, 0:HL],
                                        scalar1=kst['tk'][:, 0:1], scalar2=None,
                                        op0=OP.is_gt, op1=OP.add,
                                        accum_out=kst['cnt_c'][:])
                nc.gpsimd.tensor_scalar(out=pt_c[0:D, 4:8, :], in0=kT32[:, HL:L],
                                        scalar1=kst['tk'][:, 0:1], scalar2=None,
                                        op0=OP.is_gt)
                nc.vector.tensor_scalar(out=pt_c[0:D, 8:12, :],
                                        in0=pt_c[0:D, 4:8, :],
                                        scalar1=1.0, scalar2=None,
                                        op0=OP.mult, op1=OP.add,
                                        accum_out=kst['cnt_p'][:])
                nc.gpsimd.tensor_tensor(out=kst['cnt_c'][:], in0=kst['cnt_c'][:],
                                        in1=kst['cnt_p'][:], op=OP.add)
                kst['adj'] = sc.tile([D, 1], f32, tag="adj", name="adj")
                nc.gpsimd.tensor_scalar(out=kst['adj'][:], in0=kst['cnt_c'][:],
                                        scalar1=float(-LQ), scalar2=1.0 / (L * PHI),
                                        op0=OP.add, op1=OP.mult)
                nc.gpsimd.tensor_tensor(out=kst['adj'][:], in0=kst['adj'][:],
                                        in1=kst['sig'][:], op=OP.mult)
                kst['t1'] = sc.tile([D, 1], f32, tag="t1", name="t1")
                nc.gpsimd.tensor_tensor(out=kst['t1'][:], in0=kst['tk'][:],
                                        in1=kst['adj'][:], op=OP.add)

            def emit_kred_b():
                HL = L // 2
                kst['s1c'] = sc.tile([D, 1], f32, tag="s1c", name="s1c")
                kst['s1p'] = sc.tile([D, 1], f32, tag="s1p", name="s1p")
                nc.vector.tensor_scalar(out=sqk_rep[0:D, 0:HL], in0=kT32[:, 0:HL],
                                        scalar1=kst['t1'][:, 0:1],
                                        scalar2=0.0, op0=OP.subtract, op1=OP.max)
                nc.gpsimd.tensor_scalar(out=sqk_rep[0:D, HL:L], in0=kT32[:, HL:L],
                                        scalar1=kst['t1'][:, 0:1],
                                        scalar2=0.0, op0=OP.subtract, op1=OP.max)
                nc.vector.tensor_reduce(out=kst['s1c'][:], in_=sqk_rep[0:D, 0:HL],
                                        axis=mybir.AxisListType.X, op=OP.add)
                nc.vector.tensor_reduce(out=kst['s1p'][:], in_=sqk_rep[0:D, HL:L],
                                        axis=mybir.AxisListType.X, op=OP.add)
                nc.gpsimd.tensor_tensor(out=kst['s1c'][:], in0=kst['s1c'][:],
                                        in1=kst['s1p'][:], op=OP.add)
                nc.gpsimd.tensor_scalar(out=kr[:], in0=kst['s1c'][:],
                                        scalar1=1.0 / LQ, scalar2=None, op0=OP.mult)
                nc.gpsimd.tensor_tensor(out=kr[:], in0=kr[:], in1=kst['t1'][:],
                                        op=OP.add)

            def emit_sqk():
                pwt = ps_mis.tile([PART, PART], f32, tag="mis", name="mis")
                nc.tensor.transpose(out=pwt[0:D, 0:D], in_=wq_s[:],
                                    identity=ident[0:D, 0:D])
                wqT = sc.tile([D, D], f32, tag="wqT")
                nc.vector.tensor_copy(wqT[:], pwt[0:D, 0:D])
                pw = ps_mis.tile([PART, PART], f32, tag="mis", name="mis")
                nc.tensor.matmul(out=pw[0:D, 0:1], lhsT=wqT[:], rhs=kr[:],
                                 start=True, stop=True)
                nc.vector.tensor_copy(wvec[:], pw[0:D, 0:1])
                psq = ps_mis.tile([PART, PART], f32, tag="mis", name="mis")
                for c in range(NT):
                    nc.tensor.matmul(out=psq[:, c:c + 1],
                                     lhsT=xT32[:, PART * c:PART * (c + 1)],
                                     rhs=wvec[:], start=True, stop=True)
                nc.vector.tensor_copy(sqk[:], psq[:, 0:NT])
                psqT = ps_mis.tile([PART, PART], f32, tag="mis", name="mis")
                nc.tensor.transpose(out=psqT[0:NT, 0:PART], in_=sqk[:],
                                    identity=ident[:])
                sqkT = sc.tile([NT, PART], f32, tag="sqkT")
                nc.vector.tensor_copy(sqkT[:], psqT[0:NT, 0:PART])
                nc.sync.dma_start(out=scr_row[:], in_=sqkT[:])
                # replicate in two halves on two queues so pass 1 starts early
                nc.sync.dma_start(out=sqk_rep[:, 0:L // 2],
                                  in_=scr_row[0:1, 0:L // 2].to_broadcast([PART, L // 2]))
                nc.gpsimd.dma_start(out=sqk_rep[:, L // 2:L],
                                    in_=scr_row[0:1, L // 2:L].to_broadcast([PART, L // 2]))
                # prefetch the wrapped view for the compact stage
                nc.sync.dma_start(out=sqk16[:], in_=scr_row[0, :].rearrange(
                    "(f p) -> p f", p=16))

            def emit_meanv():
                pmv = ps_mis.tile([PART, PART], f32, tag="mis", name="mis")
                for c in range(NT):
                    nc.tensor.matmul(out=pmv[0:D + 1, 0:1], lhsT=vp[:, c, :],
                                     rhs=onesb[:], start=(c == 0), stop=(c == NT - 1))
                mv_col = sc.tile([D, 1], f32, tag="mv_col")
                nc.vector.tensor_scalar_mul(mv_col[:], pmv[0:D, 0:1], 1.0 / L)
                pmvT = ps_mis.tile([PART, PART], f32, tag="mis", name="mis")
                nc.tensor.transpose(out=pmvT[0:1, 0:D], in_=mv_col[:],
                                    identity=ident[0:D, 0:D])
                mv_row = sc.tile([1, D], f32, tag="mv_row")
                nc.vector.tensor_copy(mv_row[:], pmvT[0:1, 0:D])
                pmvF = ps_mis.tile([PART, PART], f32, tag="mis", name="mis")
                nc.tensor.matmul(out=pmvF[:, 0:D], lhsT=ones1x128[:], rhs=mv_row[:],
                                 start=True, stop=True)
                nc.vector.tensor_copy(mvf[:], pmvF[:, 0:D])
                # meanV pre-fill of all tail rows (before any scatter: Tile
                # orders the overlapping out-tensor writes); Pool SWDGE queue
                if 'fill' not in KSKIP:
                    nc.gpsimd.dma_start(
                        out=out_re[:],
                        in_=mvf[:].rearrange("p d -> p () d").to_broadcast(
                            [PART, NT, D]))

            # ---- phase-1b emission schedule ----
            emit_kred_a()          # DVE/Pool, needs full kT32
            for g in range(8):
                proj_qv(4 * g)
            emit_kred_b()
            nc.vector.memset(vp[:, :, D:D + 1], 1.0)
            emit_sqk()
            # stage Q to DRAM in quarters (SP picks scr_row/sqk_rep first)
            for qq in range(4):
                nc.sync.dma_start(out=qd_re[:, 8 * qq:8 * (qq + 1), :],
                                  in_=q_stage[:, 8 * qq:8 * (qq + 1), :])
            emit_meanv()

        # ====================== selection pieces (emitted into phase 2) ====
        def sel_pass_init():
            lo = mn.tile([PART, 1], f32, tag="lo_a")
            nc.vector.memset(lo[:], -BOUND)
            sel_state["lo"] = lo

        def sel_pass_full(it):
            lo = sel_state["lo"]
            dlt = DLT1 if it == 0 else DLT2
            if it == 0:
                tvec = tvec1
            else:
                tvec = mn.tile([PART, 1], f32, tag=f"tv{it % 2}")
                nc.gpsimd.tensor_scalar(out=tvec[:], in0=pidx1[:], scalar1=float(dlt),
                                        scalar2=None, op0=OP.mult)
                nc.gpsimd.tensor_tensor(out=tvec[:], in0=tvec[:], in1=lo[:], op=OP.add)
            HL = L // 2
            cntq = mn.tile([PART, 1], f32, tag="cntq")
            cntp = mn.tile([PART, 1], f32, tag="cntp")
            nc.vector.tensor_scalar(out=pt_c[:, 0:4, :], in0=sqk_rep[:, 0:HL],
                                    scalar1=tvec[:, 0:1], scalar2=None,
                                    op0=OP.is_gt, op1=OP.add, accum_out=cntq[:])
            nc.gpsimd.tensor_scalar(out=pt_c[:, 4:8, :], in0=sqk_rep[:, HL:L],
                                    scalar1=tvec[:, 0:1], scalar2=None,
                                    op0=OP.is_gt)
            nc.vector.tensor_scalar(out=pt_c[:, 8:12, :], in0=pt_c[:, 4:8, :],
                                    scalar1=1.0, scalar2=None,
                                    op0=OP.mult, op1=OP.add, accum_out=cntp[:])
            nc.gpsimd.tensor_tensor(out=cntq[:], in0=cntq[:], in1=cntp[:], op=OP.add)
            sel = mn.tile([PART, 1], f32, tag="sel")
            nc.gpsimd.tensor_scalar(out=sel[:], in0=cntq[:], scalar1=float(LQ),
                                    scalar2=None, op0=OP.is_ge)
            jsr = mn.tile([PART, 1], f32, tag="jsr")
            nc.gpsimd.partition_all_reduce(jsr[:], sel[:], channels=PART,
                                           reduce_op=bass_isa.ReduceOp.add)
            nlo = mn.tile([PART, 1], f32, tag=f"lo_{'b' if it % 2 == 0 else 'a'}")
            nc.gpsimd.tensor_scalar(out=jsr[:], in0=jsr[:], scalar1=float(dlt),
                                    scalar2=None, op0=OP.mult)
            nc.gpsimd.tensor_tensor(out=nlo[:], in0=lo[:], in1=jsr[:], op=OP.add)
            sel_state["lo"] = nlo

        def sel_compact():
            lo2 = sel_state["lo"]
            hi2 = mn.tile([PART, 1], f32, tag="hi2")
            nc.gpsimd.tensor_scalar(out=hi2[:], in0=lo2[:], scalar1=float(DLT2),
                                    scalar2=None, op0=OP.add)
            # c_hi = count(sqk > hi2) exact on the [128, 32] tile
            j32 = sc.tile([PART, NT], f32, tag="j32")
            chi = sc.tile([PART, 1], f32, tag="chi")
            nc.vector.tensor_scalar(out=j32[:], in0=sqk[:],
                                    scalar1=hi2[:, 0:1], scalar2=None,
                                    op0=OP.is_gt, op1=OP.add, accum_out=chi[:])
            nc.gpsimd.partition_all_reduce(chi[:], chi[:], channels=PART,
                                           reduce_op=bass_isa.ReduceOp.add)
            rvec = sc.tile([PART, 1], f32, tag="rvec")
            nc.gpsimd.tensor_scalar(out=rvec[:], in0=chi[:], scalar1=-1.0,
                                    scalar2=float(LQ), op0=OP.mult, op1=OP.add)
            sel_state["rvec"] = rvec
            # tv = (sqk-lo2)*b + (b-1), b = (sqk <= hi2); in-interval -> v'>0
            a16 = sc.tile([16, 256], f32, tag="a16")
            nc.gpsimd.tensor_scalar(out=a16[:], in0=sqk16[:],
                                    scalar1=lo2[0:16, 0:1], scalar2=None,
                                    op0=OP.subtract)
            b16 = sc.tile([16, 256], f32, tag="b16")
            nc.gpsimd.tensor_scalar(out=b16[:], in0=sqk16[:],
                                    scalar1=hi2[0:16, 0:1], scalar2=None,
                                    op0=OP.is_le)
            nc.gpsimd.tensor_tensor(out=tvv[:, 0:256], in0=a16[:], in1=b16[:],
                                    op=OP.mult)
            nc.gpsimd.tensor_scalar(out=b16[:], in0=b16[:], scalar1=1.0,
                                    scalar2=None, op0=OP.subtract)
            nc.gpsimd.tensor_tensor(out=tvv[:, 0:256], in0=tvv[:, 0:256],
                                    in1=b16[:], op=OP.add)
            # compact; appended 0.0 sentinels guarantee the first 64 output
            # slots are always hardware-written (no num_found round-trip)
            nc.gpsimd.sparse_gather(valc[:], tvv[:], num_found=nfdummy[:])
            nc.gpsimd.dma_start(out=scr_cmp[0, :].rearrange("(f p) -> p f", p=16),
                                in_=valc[:, 0:4])
            nc.gpsimd.dma_start(out=valrep[:],
                                in_=scr_cmp[:].to_broadcast([PART, 64]))
            loc = mn.tile([PART, 1], f32, tag="loc_a")
            nc.gpsimd.memset(loc[:], 0.0)
            sel_state["loc"] = loc

        def sel_pass_c(it):
            loc, rvec = sel_state["loc"], sel_state["rvec"]
            dlt = DLTC[it]
            tvec = mn.tile([PART, 1], f32, tag=f"tvc{it % 2}")
            nc.gpsimd.tensor_scalar(out=tvec[:], in0=pidx1[:], scalar1=float(dlt),
                                    scalar2=None, op0=OP.mult)
            nc.gpsimd.tensor_tensor(out=tvec[:], in0=tvec[:], in1=loc[:], op=OP.add)
            cntq = mn.tile([PART, 1], f32, tag="cntqc")
            nc.vector.tensor_scalar(out=cmpc[:], in0=valrep[:],
                                    scalar1=tvec[:, 0:1], scalar2=None,
                                    op0=OP.is_gt, op1=OP.add, accum_out=cntq[:])
            sel = mn.tile([PART, 1], f32, tag="selc")
            nc.gpsimd.tensor_scalar(out=sel[:], in0=cntq[:],
                                    scalar1=rvec[:, 0:1], scalar2=None,
                                    op0=OP.is_ge)
            jsr = mn.tile([PART, 1], f32, tag="jsrc")
            nc.gpsimd.partition_all_reduce(jsr[:], sel[:], channels=PART,
                                           reduce_op=bass_isa.ReduceOp.add)
            nlo = mn.tile([PART, 1], f32, tag=f"loc_{'b' if it % 2 == 0 else 'a'}")
            nc.gpsimd.tensor_scalar(out=jsr[:], in0=jsr[:], scalar1=float(dlt),
                                    scalar2=None, op0=OP.mult)
            nc.gpsimd.tensor_tensor(out=nlo[:], in0=loc[:], in1=jsr[:], op=OP.add)
            sel_state["loc"] = nlo


        def emit_idx():
            # tail-selected ordered positions, sentinel-padded to NTAIL
            lo2, loc = sel_state["lo"], sel_state["loc"]
            a16 = sc.tile([16, 256], f32, tag="ai16")
            nc.gpsimd.tensor_scalar(out=a16[:], in0=sqk16[:],
                                    scalar1=lo2[0:16, 0:1], scalar2=None,
                                    op0=OP.subtract)
            b16 = sc.tile([16, 256], f32, tag="bi16")
            nc.gpsimd.tensor_scalar(out=b16[:], in0=a16[:],
                                    scalar1=loc[0:16, 0:1], scalar2=None,
                                    op0=OP.is_gt)
            nc.gpsimd.tensor_tensor(out=tvi[:, 0:256], in0=iot1[:], in1=b16[:],
                                    op=OP.mult)
            nc.gpsimd.tensor_scalar(out=tvi[:, 0:256], in0=tvi[:, 0:256],
                                    scalar1=1.0, scalar2=None, op0=OP.subtract)
            nc.gpsimd.sparse_gather(idxw[:], tvi[:], num_found=nfdummy[:])
            nc.gpsimd.tensor_copy(idx16w[0:16, :], idxw[:, 0:NTC * PART // 16])
            nc.gpsimd.tensor_copy(idx32w[:], idxw[:, 0:NTC * PART // 16])
            nc.sync.dma_start(out=scr_i32[0, :].rearrange("(s p) -> p s", p=16),
                              in_=idx32w[:])
            # replicate wrapped idx to all 8 core blocks, SBUF->SBUF
            for g in range(1, 8):
                eng = nc.sync if g % 2 == 0 else nc.gpsimd
                eng.dma_start(out=idx16w[16 * g:16 * (g + 1), :],
                              in_=idx16w[0:16, :])
            # chunk-layout scatter offsets
            nc.sync.dma_start(out=idxo[:],
                              in_=scr_i32[0, :].rearrange("(c p) -> p c", p=PART))

        # ====================== phase 2 ======================
        with tc.tile_pool(name="ps_strip", bufs=2, space="PSUM") as ps_strip, \
             tc.tile_pool(name="ps_vstrip", bufs=1, space="PSUM") as ps_vstrip, \
             tc.tile_pool(name="ps_av", bufs=2, space="PSUM") as ps_av:

            def pt_of(s):
                return (pt_a, pt_b, pt_c)[s % 3]

            def score_group(rhs_ap, qlen, ptc, gi, dve):
                g0, glen = GROUPS[gi]
                if dve:
                    strip = ps_vstrip.tile([PART, 2, 512], f32, tag="vstrip")
                else:
                    strip = ps_strip.tile([PART, 2, 512], f32, tag="strip")
                for i in range(glen):
                    j = g0 + i
                    nc.tensor.matmul(out=strip[:, i, 0:qlen],
                                     lhsT=kTb[:, PART * j:PART * (j + 1)],
                                     rhs=rhs_ap, start=True, stop=True)
                if dve:
                    nc.vector.tensor_scalar(
                        out=ptc[:, g0:g0 + glen, 0:qlen].bitcast(i16),
                        in0=strip[:, 0:glen, 0:qlen], scalar1=SCH_A, scalar2=SCH_B,
                        op0=OP.mult, op1=OP.add)
                else:
                    nc.scalar.activation(out=ptc[:, g0:g0 + glen, 0:qlen],
                                         in_=strip[:, 0:glen, 0:qlen], func=AF.Exp,
                                         scale=0.125)

            def av_subtile(ptp, u, qn, rchunk):
                # qn rows of AV for query sub-tile u of the previous slab
                av = ps_av.tile([PART, PART], f32, tag="av")
                for j in range(NT):
                    nc.tensor.matmul(out=av[0:qn, 0:D + 1],
                                     lhsT=ptp[:, j, PART * u:PART * u + qn],
                                     rhs=vp[:, j, :],
                                     start=(j == 0), stop=(j == NT - 1))
                rec = mn.tile([PART, 1], f32, tag="rec")
                nc.vector.reciprocal_approx_fast(rec[0:qn, :], av[0:qn, D:D + 1])
                nc.vector.tensor_scalar(out=res[0:qn, rchunk, :], in0=av[0:qn, 0:D],
                                        scalar1=rec[0:qn, 0:1], scalar2=None,
                                        op0=OP.mult)
                # scatter is an ADD onto the meanV fill: emit attn - meanV
                nc.vector.tensor_tensor(out=res[0:qn, rchunk, :],
                                        in0=res[0:qn, rchunk, :], in1=mvf[0:qn, :],
                                        op=OP.subtract)

            def emit_gather(ts):
                glen = 512 if ts < 5 else 256
                qsel = qs.tile([PART, 1, 512], bf16, tag="qsel")
                nc.gpsimd.dma_gather(
                    qsel[:, :, 0:glen], qdram[:],
                    idx16w[:, 32 * ts:32 * ts + glen // 16],
                    glen, glen, PART, elem_step=PART, transpose=True)
                return qsel

            def emit_scatter(ts):
                # scatter-add slab results (attn - meanV) onto the meanV fill
                if 'scat' in KSKIP:
                    return
                c0 = 4 * ts
                nq = TAIL_LENS[ts]
                nc.gpsimd.dma_scatter_add(
                    out[:], res[:, c0:c0 + (nq + 127) // 128, :],
                    idx16w[:, 32 * ts:32 * ts + (nq + 15) // 16],
                    nq, nq, D, elem_step=D)

            # ---- selection (uncontended), then gathered slabs ----
            qsel_tiles = {}
            sel_pass_init()
            sel_pass_full(0)
            sel_pass_full(1)
            sel_compact()
            sel_pass_c(0)
            sel_pass_c(1)
            sel_pass_c(2)
            emit_idx()

            def emit_tail_slab(ts):
                # scores+exp for slab ts; AV of previous slab interleaved
                qlen = TAIL_LENS[ts]
                ptc = pt_of(ts)
                qsel = qsel_tiles[ts]
                prev_av = []
                if ts > 0:
                    pq = TAIL_LENS[ts - 1]
                    ptp = pt_of(ts - 1)
                    prev_av = [(ptp, u, min(PART, pq - PART * u), 4 * (ts - 1) + u)
                               for u in range((pq + 127) // 128)]
                avi = 0
                for gi in range(len(GROUPS)):
                    dve = gi in DVE_GROUPS_TAIL
                    score_group(qsel[0:D, 0, 0:qlen], qlen, ptc, gi, dve)
                    if gi in (3, 6, 9, 12) and avi < len(prev_av):
                        av_subtile(*prev_av[avi])
                        avi += 1
                for a in prev_av[avi:]:
                    av_subtile(*a)

            qsel_tiles[0] = emit_gather(0)
            qsel_tiles[1] = emit_gather(1)
            emit_tail_slab(0)
            qsel_tiles[2] = emit_gather(2)
            emit_tail_slab(1)
            emit_scatter(0)
            qsel_tiles[3] = emit_gather(3)
            emit_tail_slab(2)
            emit_scatter(1)
            qsel_tiles[4] = emit_gather(4)
            emit_tail_slab(3)
            emit_scatter(2)
            qsel_tiles[5] = emit_gather(5)
            emit_tail_slab(4)
            emit_scatter(3)
            emit_tail_slab(5)
            emit_scatter(4)
            # AV for the last slab
            pq = TAIL_LENS[5]
            ptp = pt_of(5)
            for u in range((pq + 127) // 128):
                av_subtile(ptp, u, min(PART, pq - PART * u), 20 + u)
            emit_scatter(5)


# revision 30
# speedup vs baseline: 1.2369x; 1.0175x over previous
"""Trainium2 Bass kernel for nn_AttentionBlock_33724083208839 (sparse_attention).

Data-parallel over batch (8 batches -> 8 cores). Hybrid selected-only design:
  1. load x, PE-transpose -> xT32 (f32, exact) + xTb (bf16); K projected in
     exact f32 (feeds selection); Q/V projected in bf16; Q also staged
     row-major to DRAM (qdram [4104,128] bf16, padded cols + zero ghost row).
  2. selection (exact, baseline machinery, DVE+Pool split halves): K_reduce
     via CVaR identity; sqk exact on PE; threshold via two 128-ary counting
     passes on a DMA-replicated sqk + sparse_gather compaction (sentinel
     padding instead of num_found round-trips) + three compact passes.
  3. head: attention for ALL queries of slabs 0-1 (1024 queries) overlaps
     selection latency; blend non-selected head rows to meanV (Pool
     arithmetic), direct DMA out.
  4. tail: positions 1024..4095 -> compact ordered index list of selected
     queries (sparse_gather over masked iota, padded to 2240 with sentinel
     4096 by appending always-gathered sentinel values); dma_gather
     (transpose mode) pulls Q_sel^T tiles from qdram; 4.375 slabs of
     scores->exp(ACT/DVE Schraudolph)->AV; results scattered to out rows by
     indirect DMA (sentinels skipped via bounds_check); rows 1024..4095
     pre-filled with meanV by chunk DMAs before the scatters.
"""
import os
import sys

sys.path.insert(0, "/opt/trn_rl_repo")

import math
from statistics import NormalDist

KSKIP = set(os.environ.get('KSKIP', '').split(','))

import numpy as np

import concourse.bacc as bacc
import concourse.bass as bass
import concourse.bass_isa as bass_isa
import concourse.mybir as mybir
from concourse.tile import TileContext
from concourse.masks import make_identity
from concourse.bass_utils import run_bass_kernel_spmd

B, L, D = 8, 4096, 64
LQ = int((1.0 - 0.33) * L)  # 2744
PART = 128
NT = L // PART          # 32 key tiles / x chunks
N_CORES = 8

NTAIL = 2816            # padded compact count (LQ=2744 exact + sentinels)
NTC = 22                # compact chunks
TAIL_LENS = [512, 512, 512, 512, 512, 256]
SENT = L                # sentinel index -> qdram ghost row, scatter-skipped
QD_ROWS = L + 8         # qdram rows (ghost row at L)
N_VSENT = 64            # value-compaction sentinel count
N_ISENT = 592           # index-compaction sentinels (need >= NTAIL - LQ = 72)

QFRAC = 1.0 - LQ / L
Z = NormalDist().inv_cdf(QFRAC)
PHI = math.exp(-Z * Z / 2.0) / math.sqrt(2.0 * math.pi)

f32 = mybir.dt.float32
bf16 = mybir.dt.bfloat16
u8 = mybir.dt.uint8
i16 = mybir.dt.int16
i32 = mybir.dt.int32
u32 = mybir.dt.uint32
AF = mybir.ActivationFunctionType
OP = mybir.AluOpType

BOUND = 512.0

# Schraudolph exp for bf16 bit patterns: bf16_bits(exp(s/8)) ~= A*s + B.
SCH_A = 128.0 * math.log2(math.e) / 8.0
SCH_B = 16256.0 + 0.5 - 128.0 * math.log2(1.0407)

GROUPS = [(g, 2) for g in range(0, NT, 2)]

# head slabs: DVE takes the last 3 pair-groups (it is busy with selection
# until ~2/3 through each head slab); tail slabs: DVE takes 6.
DVE_GROUPS_TAIL = {3, 6, 8, 11, 14}


def build(debug: bool = False):
    nc = bacc.Bacc("TRN2")
    x = nc.dram_tensor("x", [L, D], f32, kind="ExternalInput")
    wq = nc.dram_tensor("Wq", [D, D], f32, kind="ExternalInput")
    wk = nc.dram_tensor("Wk", [D, D], f32, kind="ExternalInput")
    wv = nc.dram_tensor("Wv", [D, D], f32, kind="ExternalInput")
    out = nc.dram_tensor("out", [L + 8, D], f32, kind="ExternalOutput")
    scr_row = nc.dram_tensor("scr_row", [1, L], f32, kind="Internal")
    scr_cmp = nc.dram_tensor("scr_cmp", [1, 64], f32, kind="Internal")
    qdram = nc.dram_tensor("qdram", [QD_ROWS, PART], bf16, kind="Internal")
    scr_i32 = nc.dram_tensor("scr_i32", [1, NTC * PART], i32, kind="Internal")

    x_re = x[:].rearrange("(c p) d -> p c d", p=PART)
    # partition-contiguous x view: one 8KB/partition DMA
    x_rc = x[:].rearrange("(p c) d -> p c d", c=NT)
    out_re = out[0:L, :].rearrange("(c p) d -> p c d", p=PART)
    qd_re = qdram[0:L, :].rearrange("(c p) e -> p c e", p=PART)

    with TileContext(nc) as tc, \
         tc.tile_pool(name="cst", bufs=1) as cst, \
         tc.tile_pool(name="big", bufs=1) as big, \
         tc.tile_pool(name="sc", bufs=1) as sc, \
         tc.tile_pool(name="mn", bufs=2) as mn, \
         tc.tile_pool(name="qs", bufs=3) as qs:

        # ---- warm the exp activation table immediately ----
        warm = cst.tile([1, 8], f32)
        nc.vector.memset(warm[:], 0.0)
        warm2 = cst.tile([1, 8], f32)
        nc.scalar.activation(out=warm2[:], in_=warm[:], func=AF.Exp)

        # ---- constants ----
        ident = cst.tile([PART, PART], f32)
        make_identity(nc, ident[:])
        onesb = cst.tile([PART, 1], bf16)
        nc.vector.memset(onesb[:], 1.0)
        ones1x128 = cst.tile([1, PART], f32)
        nc.vector.memset(ones1x128[:], 1.0)
        pidx1i = cst.tile([PART, 1], i32)
        nc.gpsimd.iota(pidx1i[:], pattern=[[1, 1]], base=1, channel_multiplier=1)
        pidx1 = cst.tile([PART, 1], f32)
        nc.vector.tensor_copy(pidx1[:], pidx1i[:])
        # pass-1 thresholds are compile-time: tvec1 = pidx1*DLT1 - BOUND
        tvec1 = cst.tile([PART, 1], f32)
        nc.gpsimd.tensor_scalar(out=tvec1[:], in0=pidx1[:],
                                scalar1=2.0 * BOUND / 129.0, scalar2=-BOUND,
                                op0=OP.mult, op1=OP.add)
        # iot1[p, f] = (f*16 + p) + 1 = original position + 1, wrapped layout
        iot1i = cst.tile([16, 256], i32)
        nc.gpsimd.iota(iot1i[:], pattern=[[16, 256]], base=1, channel_multiplier=1)
        iot1 = cst.tile([16, 256], f32)
        nc.vector.tensor_copy(iot1[:], iot1i[:])

        # ---- persistent tensors ----
        x_sb = big.tile([PART, NT, D], f32)
        # x_sb is dead once the transposes finish; reuse it for res
        res = x_sb
        xT32 = big.tile([D, L], f32)
        xTb = big.tile([D, L], bf16)
        kT32 = big.tile([D, L], f32)
        kTb = big.tile([D, L], bf16)
        q_stage = big.tile([PART, NT, PART], bf16)
        vp = big.tile([PART, NT, D + 1], bf16)
        pt_a = big.tile([PART, NT, 512], bf16)
        pt_b = big.tile([PART, NT, 512], bf16)
        pt_c = big.tile([PART, NT, 512], bf16)
        mvf = big.tile([PART, D], f32)
        mvf2 = big.tile([PART, D], f32)
        sqk = big.tile([PART, NT], f32)
        kr = big.tile([D, 1], f32)
        wvec = big.tile([D, 1], f32)
        sqk_rep = big.tile([PART, L], f32)
        sqk16 = big.tile([16, 256], f32)
        valrep = big.tile([PART, 64], f32)
        cmpc = big.tile([PART, 64], bf16)
        # value compaction: input [16, 256+4] (tv || 0.0-sentinels)
        tvv = big.tile([16, 256 + N_VSENT // 16], f32)
        valc = big.tile([16, 20], f32)
        nfdummy = big.tile([1, 1], u32)
        # index compaction: input [16, 256+10] (tvidx || 4096.0-sentinels)
        tvi = big.tile([16, 256 + N_ISENT // 16], f32)
        idxw = big.tile([16, 209], f32)
        idx16w = big.tile([PART, NTC * PART // 16], i16)
        idx32w = big.tile([16, NTC * PART // 16], i32)
        idxo = big.tile([PART, NTC], i32)

        # weights
        wq_s = cst.tile([D, D], f32)
        wk_s = cst.tile([D, D], f32)
        wv_s = cst.tile([D, D], f32)
        nc.gpsimd.dma_start(out=wq_s[:], in_=wq[:])
        nc.gpsimd.dma_start(out=wk_s[:], in_=wk[:])
        nc.gpsimd.dma_start(out=wv_s[:], in_=wv[:])
        wq_b = cst.tile([D, D], bf16)
        wv_b = cst.tile([D, D], bf16)
        nc.gpsimd.tensor_copy(wq_b[:], wq_s[:])
        nc.gpsimd.tensor_copy(wv_b[:], wv_s[:])
        # sentinel regions (Pool, SBUF-only, one-time)
        nc.gpsimd.memset(tvv[:, 256:], 0.0)
        nc.gpsimd.memset(tvi[:, 256:], float(SENT))
        nc.gpsimd.memset(q_stage[:, :, D:PART], 0.0)
        # last tail chunk is half-height; zero the unwritten rows once
        nc.vector.memset(res[:, NTC - 1, :], 0.0)
        # ghost row of qdram <- zeros
        zrow = cst.tile([1, PART], bf16)
        nc.vector.memset(zrow[:], 0.0)
        nc.gpsimd.dma_start(out=qdram[L:L + 1, :], in_=zrow[:])

        kst = {}
        sel_state = {}
        DLT1 = 2.0 * BOUND / 129.0
        DLT2 = DLT1 / 129.0
        DLTC = [DLT2 / 129.0, DLT2 / 129.0 ** 2, DLT2 / 129.0 ** 3]

        # ====================== phase 1a: transposes + K ======================
        with tc.tile_pool(name="ps_xv", bufs=3, space="PSUM") as ps_xv, \
             tc.tile_pool(name="ps_pj", bufs=2, space="PSUM") as ps_pj:

            def load_tiles(c0):
                # 8 transposes into one 2-bank PSUM tile, one copy per engine
                pxt = ps_xv.tile([PART, 1024], f32, tag="xv")
                for i in range(8):
                    nc.tensor.transpose(out=pxt[0:D, PART * i:PART * (i + 1)],
                                        in_=x_sb[:, c0 + i, :], identity=ident[:])
                xv32 = xT32[:].rearrange("d (p c) -> d c p", c=NT)
                xvb = xTb[:].rearrange("d (p c) -> d c p", c=NT)
                pxt4 = pxt[0:D, :].rearrange("d (c p) -> d c p", c=8)
                nc.vector.tensor_copy(xv32[:, c0:c0 + 8, :], pxt4[:])
                nc.scalar.copy(xvb[:, c0:c0 + 8, :], pxt4[:])

            def proj_k(s):
                sl = slice(512 * s, 512 * (s + 1))
                pk = ps_pj.tile([D, 512], f32, tag="pj")
                for h in range(2):
                    nc.tensor.matmul(out=pk[:, 256 * h:256 * (h + 1)], lhsT=wk_s[:],
                                     rhs=xT32[:, 512 * s + 256 * h:512 * s + 256 * (h + 1)],
                                     start=True, stop=True)
                nc.vector.tensor_copy(kT32[:, sl], pk[:])
                nc.scalar.copy(kTb[:, sl], pk[:])

            nc.sync.dma_start(out=x_sb[:, 0:8, :], in_=x_rc[:, 0:8, :])
            nc.sync.dma_start(out=x_sb[:, 8:32, :], in_=x_rc[:, 8:32, :])
            for s in range(4):
                load_tiles(8 * s)
            for s in range(8):
                proj_k(s)

        # ====================== phase 1b: Q/V + head slab 0 =================
        with tc.tile_pool(name="ps_s0", bufs=2, space="PSUM") as ps_s0, \
             tc.tile_pool(name="ps_qv", bufs=2, space="PSUM") as ps_qv, \
             tc.tile_pool(name="ps_mis", bufs=2, space="PSUM") as ps_mis:

            def proj_qv(c0):
                # 4 chunks of Q and V into one PSUM bank (slots 0-3 Q, 4-7 V)
                pqv = ps_qv.tile([PART, 512], f32, tag="qv")
                pqv8 = pqv[:].rearrange("p (o d) -> p o d", o=8)
                for i in range(4):
                    nc.tensor.matmul(out=pqv8[:, i, :],
                                     lhsT=xTb[:, PART * (c0 + i):PART * (c0 + i + 1)],
                                     rhs=wq_b[:], start=True, stop=True)
                    nc.tensor.matmul(out=pqv8[:, 4 + i, :],
                                     lhsT=xTb[:, PART * (c0 + i):PART * (c0 + i + 1)],
                                     rhs=wv_b[:], start=True, stop=True)
                nc.scalar.copy(q_stage[:, c0:c0 + 4, 0:D], pqv8[:, 0:4, :])
                if (c0 // 4) % 2 == 0:
                    nc.vector.tensor_copy(vp[:, c0:c0 + 4, 0:D], pqv8[:, 4:8, :])
                else:
                    nc.scalar.copy(vp[:, c0:c0 + 4, 0:D], pqv8[:, 4:8, :])



            def emit_kred_a():
                kst['bstats'] = sc.tile([D, 4, 6], f32, tag="bstats", name="bstats")
                for a in range(4):
                    nc.vector.bn_stats(kst['bstats'][:, a, :],
                                       kT32[:, 1024 * a:1024 * a + 512])
                kst['aggr'] = sc.tile([D, 2], f32, tag="aggr", name="aggr")
                nc.vector.bn_aggr(kst['aggr'][:], kst['bstats'][:])
                kst['sig'] = sc.tile([D, 1], f32, tag="sig", name="sig")
                nc.vector.memset(kst['sig'][:], 1.0)
                for _ in range(2):
                    kst['rs'] = sc.tile([D, 1], f32, tag="rs", name="rs")
                    nc.vector.reciprocal(kst['rs'][:], kst['sig'][:])
                    nc.vector.tensor_tensor(out=kst['rs'][:], in0=kst['rs'][:],
                                            in1=kst['aggr'][:, 1:2], op=OP.mult)
                    nc.vector.tensor_tensor(out=kst['rs'][:], in0=kst['rs'][:],
                                            in1=kst['sig'][:], op=OP.add)
                    nc.vector.tensor_scalar_mul(kst['sig'][:], kst['rs'][:], 0.5)
                kst['tk'] = sc.tile([D, 1], f32, tag="tk", name="tk")
                nc.vector.tensor_scalar(out=kst['tk'][:], in0=kst['sig'][:],
                                        scalar1=float(Z), scalar2=None, op0=OP.mult)
                nc.gpsimd.tensor_tensor(out=kst['tk'][:], in0=kst['tk'][:],
                                        in1=kst['aggr'][:, 0:1], op=OP.add)
                HL = L // 2
                kst['cnt_c'] = sc.tile([D, 1], f32, tag="cnt_c", name="cnt_c")
                kst['cnt_p'] = sc.tile([D, 1], f32, tag="cnt_p", name="cnt_p")
                nc.vector.tensor_scalar(out=pt_c[0:D, 0:4, :], in0=kT32[:, 0:HL],
                                        scalar1=kst['tk'][:, 0:1], scalar2=None,
                                        op0=OP.is_gt, op1=OP.add,
                                        accum_out=kst['cnt_c'][:])
                nc.gpsimd.tensor_scalar(out=pt_c[0:D, 4:8, :], in0=kT32[:, HL:L],
                                        scalar1=kst['tk'][:, 0:1], scalar2=None,
                                        op0=OP.is_gt)
                nc.vector.tensor_scalar(out=pt_c[0:D, 8:12, :],
                                        in0=pt_c[0:D, 4:8, :],
                                        scalar1=1.0, scalar2=None,
                                        op0=OP.mult, op1=OP.add,
                                        accum_out=kst['cnt_p'][:])
                nc.gpsimd.tensor_tensor(out=kst['cnt_c'][:], in0=kst['cnt_c'][:],
                                        in1=kst['cnt_p'][:], op=OP.add)
                kst['adj'] = sc.tile([D, 1], f32, tag="adj", name="adj")
                nc.gpsimd.tensor_scalar(out=kst['adj'][:], in0=kst['cnt_c'][:],
                                        scalar1=float(-LQ), scalar2=1.0 / (L * PHI),
                                        op0=OP.add, op1=OP.mult)
                nc.gpsimd.tensor_tensor(out=kst['adj'][:], in0=kst['adj'][:],
                                        in1=kst['sig'][:], op=OP.mult)
                kst['t1'] = sc.tile([D, 1], f32, tag="t1", name="t1")
                nc.gpsimd.tensor_tensor(out=kst['t1'][:], in0=kst['tk'][:],
                                        in1=kst['adj'][:], op=OP.add)

            def emit_kred_b():
                HL = L // 2
                kst['s1c'] = sc.tile([D, 1], f32, tag="s1c", name="s1c")
                kst['s1p'] = sc.tile([D, 1], f32, tag="s1p", name="s1p")
                nc.vector.tensor_scalar(out=sqk_rep[0:D, 0:HL], in0=kT32[:, 0:HL],
                                        scalar1=kst['t1'][:, 0:1],
                                        scalar2=0.0, op0=OP.subtract, op1=OP.max)
                nc.gpsimd.tensor_scalar(out=sqk_rep[0:D, HL:L], in0=kT32[:, HL:L],
                                        scalar1=kst['t1'][:, 0:1],
                                        scalar2=0.0, op0=OP.subtract, op1=OP.max)
                nc.vector.tensor_reduce(out=kst['s1c'][:], in_=sqk_rep[0:D, 0:HL],
                                        axis=mybir.AxisListType.X, op=OP.add)
                nc.vector.tensor_reduce(out=kst['s1p'][:], in_=sqk_rep[0:D, HL:L],
                                        axis=mybir.AxisListType.X, op=OP.add)
                nc.gpsimd.tensor_tensor(out=kst['s1c'][:], in0=kst['s1c'][:],
                                        in1=kst['s1p'][:], op=OP.add)
                nc.gpsimd.tensor_scalar(out=kr[:], in0=kst['s1c'][:],
                                        scalar1=1.0 / LQ, scalar2=None, op0=OP.mult)
                nc.gpsimd.tensor_tensor(out=kr[:], in0=kr[:], in1=kst['t1'][:],
                                        op=OP.add)

            def emit_sqk():
                pwt = ps_mis.tile([PART, PART], f32, tag="mis", name="mis")
                nc.tensor.transpose(out=pwt[0:D, 0:D], in_=wq_s[:],
                                    identity=ident[0:D, 0:D])
                wqT = sc.tile([D, D], f32, tag="wqT")
                nc.vector.tensor_copy(wqT[:], pwt[0:D, 0:D])
                pw = ps_mis.tile([PART, PART], f32, tag="mis", name="mis")
                nc.tensor.matmul(out=pw[0:D, 0:1], lhsT=wqT[:], rhs=kr[:],
                                 start=True, stop=True)
                nc.vector.tensor_copy(wvec[:], pw[0:D, 0:1])
                psq = ps_mis.tile([PART, PART], f32, tag="mis", name="mis")
                for c in range(NT):
                    nc.tensor.matmul(out=psq[:, c:c + 1],
                                     lhsT=xT32[:, PART * c:PART * (c + 1)],
                                     rhs=wvec[:], start=True, stop=True)
                nc.vector.tensor_copy(sqk[:], psq[:, 0:NT])
                psqT = ps_mis.tile([PART, PART], f32, tag="mis", name="mis")
                nc.tensor.transpose(out=psqT[0:NT, 0:PART], in_=sqk[:],
                                    identity=ident[:])
                sqkT = sc.tile([NT, PART], f32, tag="sqkT")
                nc.vector.tensor_copy(sqkT[:], psqT[0:NT, 0:PART])
                nc.sync.dma_start(out=scr_row[:], in_=sqkT[:])
                # replicate in two halves on two queues so pass 1 starts early
                nc.sync.dma_start(out=sqk_rep[:, 0:L // 2],
                                  in_=scr_row[0:1, 0:L // 2].to_broadcast([PART, L // 2]))
                nc.gpsimd.dma_start(out=sqk_rep[:, L // 2:L],
                                    in_=scr_row[0:1, L // 2:L].to_broadcast([PART, L // 2]))
                # prefetch the wrapped view for the compact stage
                nc.sync.dma_start(out=sqk16[:], in_=scr_row[0, :].rearrange(
                    "(f p) -> p f", p=16))

            def emit_meanv():
                pmv = ps_mis.tile([PART, PART], f32, tag="mis", name="mis")
                for c in range(NT):
                    nc.tensor.matmul(out=pmv[0:D + 1, 0:1], lhsT=vp[:, c, :],
                                     rhs=onesb[:], start=(c == 0), stop=(c == NT - 1))
                mv_col = sc.tile([D, 1], f32, tag="mv_col")
                nc.vector.tensor_scalar_mul(mv_col[:], pmv[0:D, 0:1], 1.0 / L)
                pmvT = ps_mis.tile([PART, PART], f32, tag="mis", name="mis")
                nc.tensor.transpose(out=pmvT[0:1, 0:D], in_=mv_col[:],
                                    identity=ident[0:D, 0:D])
                mv_row = sc.tile([1, D], f32, tag="mv_row")
                nc.vector.tensor_copy(mv_row[:], pmvT[0:1, 0:D])
                pmvF = ps_mis.tile([PART, PART], f32, tag="mis", name="mis")
                nc.tensor.matmul(out=pmvF[:, 0:D], lhsT=ones1x128[:], rhs=mv_row[:],
                                 start=True, stop=True)
                nc.vector.tensor_copy(mvf[:], pmvF[:, 0:D])
                # meanV pre-fill of all tail rows (before any scatter: Tile
                # orders the overlapping out-tensor writes); Pool SWDGE queue
                pass

            # ---- phase-1b emission schedule ----
            emit_kred_a()          # DVE/Pool, needs full kT32
            for g in range(8):
                proj_qv(4 * g)
            emit_kred_b()
            nc.vector.memset(vp[:, :, D:D + 1], 1.0)
            emit_sqk()
            # stage Q to DRAM in quarters (SP picks scr_row/sqk_rep first)
            for qq in range(4):
                nc.sync.dma_start(out=qd_re[:, 8 * qq:8 * (qq + 1), :],
                                  in_=q_stage[:, 8 * qq:8 * (qq + 1), :])
            emit_meanv()

        # ====================== selection pieces (emitted into phase 2) ====
        def sel_pass_init():
            lo = mn.tile([PART, 1], f32, tag="lo_a")
            nc.vector.memset(lo[:], -BOUND)
            sel_state["lo"] = lo

        def sel_pass_full(it):
            lo = sel_state["lo"]
            dlt = DLT1 if it == 0 else DLT2
            if it == 0:
                tvec = tvec1
            else:
                tvec = mn.tile([PART, 1], f32, tag=f"tv{it % 2}")
                nc.gpsimd.tensor_scalar(out=tvec[:], in0=pidx1[:], scalar1=float(dlt),
                                        scalar2=None, op0=OP.mult)
                nc.gpsimd.tensor_tensor(out=tvec[:], in0=tvec[:], in1=lo[:], op=OP.add)
            HL = L // 2
            cntq = mn.tile([PART, 1], f32, tag="cntq")
            cntp = mn.tile([PART, 1], f32, tag="cntp")
            nc.vector.tensor_scalar(out=pt_c[:, 0:4, :], in0=sqk_rep[:, 0:HL],
                                    scalar1=tvec[:, 0:1], scalar2=None,
                                    op0=OP.is_gt, op1=OP.add, accum_out=cntq[:])
            nc.gpsimd.tensor_scalar(out=pt_c[:, 4:8, :], in0=sqk_rep[:, HL:L],
                                    scalar1=tvec[:, 0:1], scalar2=None,
                                    op0=OP.is_gt)
            nc.vector.tensor_scalar(out=pt_c[:, 8:12, :], in0=pt_c[:, 4:8, :],
                                    scalar1=1.0, scalar2=None,
                                    op0=OP.mult, op1=OP.add, accum_out=cntp[:])
            nc.gpsimd.tensor_tensor(out=cntq[:], in0=cntq[:], in1=cntp[:], op=OP.add)
            sel = mn.tile([PART, 1], f32, tag="sel")
            nc.gpsimd.tensor_scalar(out=sel[:], in0=cntq[:], scalar1=float(LQ),
                                    scalar2=None, op0=OP.is_ge)
            jsr = mn.tile([PART, 1], f32, tag="jsr")
            nc.gpsimd.partition_all_reduce(jsr[:], sel[:], channels=PART,
                                           reduce_op=bass_isa.ReduceOp.add)
            nlo = mn.tile([PART, 1], f32, tag=f"lo_{'b' if it % 2 == 0 else 'a'}")
            nc.gpsimd.tensor_scalar(out=jsr[:], in0=jsr[:], scalar1=float(dlt),
                                    scalar2=None, op0=OP.mult)
            nc.gpsimd.tensor_tensor(out=nlo[:], in0=lo[:], in1=jsr[:], op=OP.add)
            sel_state["lo"] = nlo

        def sel_compact():
            lo2 = sel_state["lo"]
            hi2 = mn.tile([PART, 1], f32, tag="hi2")
            nc.gpsimd.tensor_scalar(out=hi2[:], in0=lo2[:], scalar1=float(DLT2),
                                    scalar2=None, op0=OP.add)
            # c_hi = count(sqk > hi2) exact on the [128, 32] tile
            j32 = sc.tile([PART, NT], f32, tag="j32")
            chi = sc.tile([PART, 1], f32, tag="chi")
            nc.vector.tensor_scalar(out=j32[:], in0=sqk[:],
                                    scalar1=hi2[:, 0:1], scalar2=None,
                                    op0=OP.is_gt, op1=OP.add, accum_out=chi[:])
            nc.gpsimd.partition_all_reduce(chi[:], chi[:], channels=PART,
                                           reduce_op=bass_isa.ReduceOp.add)
            rvec = sc.tile([PART, 1], f32, tag="rvec")
            nc.gpsimd.tensor_scalar(out=rvec[:], in0=chi[:], scalar1=-1.0,
                                    scalar2=float(LQ), op0=OP.mult, op1=OP.add)
            sel_state["rvec"] = rvec
            # tv = (sqk-lo2)*b + (b-1), b = (sqk <= hi2); in-interval -> v'>0
            a16 = sc.tile([16, 256], f32, tag="a16")
            nc.gpsimd.tensor_scalar(out=a16[:], in0=sqk16[:],
                                    scalar1=lo2[0:16, 0:1], scalar2=None,
                                    op0=OP.subtract)
            b16 = sc.tile([16, 256], f32, tag="b16")
            nc.gpsimd.tensor_scalar(out=b16[:], in0=sqk16[:],
                                    scalar1=hi2[0:16, 0:1], scalar2=None,
                                    op0=OP.is_le)
            nc.gpsimd.tensor_tensor(out=tvv[:, 0:256], in0=a16[:], in1=b16[:],
                                    op=OP.mult)
            nc.gpsimd.tensor_scalar(out=b16[:], in0=b16[:], scalar1=1.0,
                                    scalar2=None, op0=OP.subtract)
            nc.gpsimd.tensor_tensor(out=tvv[:, 0:256], in0=tvv[:, 0:256],
                                    in1=b16[:], op=OP.add)
            # compact; appended 0.0 sentinels guarantee the first 64 output
            # slots are always hardware-written (no num_found round-trip)
            nc.gpsimd.sparse_gather(valc[:], tvv[:], num_found=nfdummy[:])
            nc.gpsimd.dma_start(out=scr_cmp[0, :].rearrange("(f p) -> p f", p=16),
                                in_=valc[:, 0:4])
            nc.gpsimd.dma_start(out=valrep[:],
                                in_=scr_cmp[:].to_broadcast([PART, 64]))
            loc = mn.tile([PART, 1], f32, tag="loc_a")
            nc.gpsimd.memset(loc[:], 0.0)
            sel_state["loc"] = loc

        def sel_pass_c(it):
            loc, rvec = sel_state["loc"], sel_state["rvec"]
            dlt = DLTC[it]
            tvec = mn.tile([PART, 1], f32, tag=f"tvc{it % 2}")
            nc.gpsimd.tensor_scalar(out=tvec[:], in0=pidx1[:], scalar1=float(dlt),
                                    scalar2=None, op0=OP.mult)
            nc.gpsimd.tensor_tensor(out=tvec[:], in0=tvec[:], in1=loc[:], op=OP.add)
            cntq = mn.tile([PART, 1], f32, tag="cntqc")
            nc.vector.tensor_scalar(out=cmpc[:], in0=valrep[:],
                                    scalar1=tvec[:, 0:1], scalar2=None,
                                    op0=OP.is_gt, op1=OP.add, accum_out=cntq[:])
            sel = mn.tile([PART, 1], f32, tag="selc")
            nc.gpsimd.tensor_scalar(out=sel[:], in0=cntq[:],
                                    scalar1=rvec[:, 0:1], scalar2=None,
                                    op0=OP.is_ge)
            jsr = mn.tile([PART, 1], f32, tag="jsrc")
            nc.gpsimd.partition_all_reduce(jsr[:], sel[:], channels=PART,
                                           reduce_op=bass_isa.ReduceOp.add)
            nlo = mn.tile([PART, 1], f32, tag=f"loc_{'b' if it % 2 == 0 else 'a'}")
            nc.gpsimd.tensor_scalar(out=jsr[:], in0=jsr[:], scalar1=float(dlt),
                                    scalar2=None, op0=OP.mult)
            nc.gpsimd.tensor_tensor(out=nlo[:], in0=loc[:], in1=jsr[:], op=OP.add)
            sel_state["loc"] = nlo


        def emit_idx():
            # tail-selected ordered positions, sentinel-padded to NTAIL
            lo2, loc = sel_state["lo"], sel_state["loc"]
            a16 = sc.tile([16, 256], f32, tag="ai16")
            nc.gpsimd.tensor_scalar(out=a16[:], in0=sqk16[:],
                                    scalar1=lo2[0:16, 0:1], scalar2=None,
                                    op0=OP.subtract)
            b16 = sc.tile([16, 256], f32, tag="bi16")
            nc.gpsimd.tensor_scalar(out=b16[:], in0=a16[:],
                                    scalar1=loc[0:16, 0:1], scalar2=None,
                                    op0=OP.is_gt)
            nc.gpsimd.tensor_tensor(out=tvi[:, 0:256], in0=iot1[:], in1=b16[:],
                                    op=OP.mult)
            nc.gpsimd.tensor_scalar(out=tvi[:, 0:256], in0=tvi[:, 0:256],
                                    scalar1=1.0, scalar2=None, op0=OP.subtract)
            nc.gpsimd.sparse_gather(idxw[:], tvi[:], num_found=nfdummy[:])
            nc.gpsimd.tensor_copy(idx16w[0:16, :], idxw[:, 0:NTC * PART // 16])
            nc.gpsimd.tensor_copy(idx32w[:], idxw[:, 0:NTC * PART // 16])
            nc.sync.dma_start(out=scr_i32[0, :].rearrange("(s p) -> p s", p=16),
                              in_=idx32w[:])
            # replicate wrapped idx to all 8 core blocks, SBUF->SBUF
            for g in range(1, 8):
                eng = nc.sync if g % 2 == 0 else nc.gpsimd
                eng.dma_start(out=idx16w[16 * g:16 * (g + 1), :],
                              in_=idx16w[0:16, :])
            # chunk-layout scatter offsets
            nc.sync.dma_start(out=idxo[:],
                              in_=scr_i32[0, :].rearrange("(c p) -> p c", p=PART))

        # ====================== phase 2 ======================
        with tc.tile_pool(name="ps_strip", bufs=2, space="PSUM") as ps_strip, \
             tc.tile_pool(name="ps_vstrip", bufs=1, space="PSUM") as ps_vstrip, \
             tc.tile_pool(name="ps_av", bufs=2, space="PSUM") as ps_av:

            def pt_of(s):
                return (pt_a, pt_b, pt_c)[s % 3]

            def score_group(rhs_ap, qlen, ptc, gi, dve):
                g0, glen = GROUPS[gi]
                if dve:
                    strip = ps_vstrip.tile([PART, 2, 512], f32, tag="vstrip")
                else:
                    strip = ps_strip.tile([PART, 2, 512], f32, tag="strip")
                for i in range(glen):
                    j = g0 + i
                    nc.tensor.matmul(out=strip[:, i, 0:qlen],
                                     lhsT=kTb[:, PART * j:PART * (j + 1)],
                                     rhs=rhs_ap, start=True, stop=True)
                if dve:
                    nc.vector.tensor_scalar(
                        out=ptc[:, g0:g0 + glen, 0:qlen].bitcast(i16),
                        in0=strip[:, 0:glen, 0:qlen], scalar1=SCH_A, scalar2=SCH_B,
                        op0=OP.mult, op1=OP.add)
                else:
                    nc.scalar.activation(out=ptc[:, g0:g0 + glen, 0:qlen],
                                         in_=strip[:, 0:glen, 0:qlen], func=AF.Exp,
                                         scale=0.125)

            def av_subtile(ptp, u, qn, rchunk):
                # qn rows of AV for query sub-tile u of the previous slab
                av = ps_av.tile([PART, PART], f32, tag="av")
                for j in range(NT):
                    nc.tensor.matmul(out=av[0:qn, 0:D + 1],
                                     lhsT=ptp[:, j, PART * u:PART * u + qn],
                                     rhs=vp[:, j, :],
                                     start=(j == 0), stop=(j == NT - 1))
                rec = mn.tile([PART, 1], f32, tag="rec")
                nc.vector.reciprocal_approx_fast(rec[0:qn, :], av[0:qn, D:D + 1])
                nc.vector.tensor_scalar(out=res[0:qn, rchunk, :], in0=av[0:qn, 0:D],
                                        scalar1=rec[0:qn, 0:1], scalar2=None,
                                        op0=OP.mult)
                # scatter is an ADD onto the meanV fill: emit attn - meanV
                nc.vector.tensor_tensor(out=res[0:qn, rchunk, :],
                                        in0=res[0:qn, rchunk, :], in1=mvf[0:qn, :],
                                        op=OP.subtract)

            def emit_gather(ts):
                glen = 512 if ts < 5 else 256
                qsel = qs.tile([PART, 1, 512], bf16, tag="qsel")
                nc.gpsimd.dma_gather(
                    qsel[:, :, 0:glen], qdram[:],
                    idx16w[:, 32 * ts:32 * ts + glen // 16],
                    glen, glen, PART, elem_step=PART, transpose=True)
                return qsel

            def emit_scatter(ts):
                # scatter-add slab results (attn - meanV) onto the meanV fill
                if 'scat' in KSKIP:
                    return
                c0 = 4 * ts
                nq = TAIL_LENS[ts]
                nc.gpsimd.dma_scatter_add(
                    out[:], res[:, c0:c0 + (nq + 127) // 128, :],
                    idx16w[:, 32 * ts:32 * ts + (nq + 15) // 16],
                    nq, nq, D, elem_step=D)

            # ---- selection (uncontended), then gathered slabs ----
            qsel_tiles = {}
            sel_pass_init()
            sel_pass_full(0)
            sel_pass_full(1)
            # meanV fill: delay readiness past pass-2 so it cannot front-run
            # the selection-critical DMAs on the greedy scheduler
            zt = mn.tile([PART, 1], f32, tag="zt")
            nc.gpsimd.tensor_scalar(out=zt[:], in0=sel_state["lo"][:],
                                    scalar1=0.0, scalar2=None, op0=OP.mult)
            nc.gpsimd.tensor_scalar(out=mvf2[:], in0=mvf[:],
                                    scalar1=zt[:, 0:1], scalar2=None, op0=OP.add)
            if 'fill' not in KSKIP:
                nc.sync.dma_start(
                    out=out_re[:],
                    in_=mvf2[:].rearrange("p d -> p () d").to_broadcast(
                        [PART, NT, D]))
            sel_compact()
            sel_pass_c(0)
            sel_pass_c(1)
            sel_pass_c(2)
            emit_idx()

            def emit_tail_slab(ts):
                # scores+exp for slab ts; AV of previous slab interleaved
                qlen = TAIL_LENS[ts]
                ptc = pt_of(ts)
                qsel = qsel_tiles[ts]
                prev_av = []
                if ts > 0:
                    pq = TAIL_LENS[ts - 1]
                    ptp = pt_of(ts - 1)
                    prev_av = [(ptp, u, min(PART, pq - PART * u), 4 * (ts - 1) + u)
                               for u in range((pq + 127) // 128)]
                avi = 0
                for gi in range(len(GROUPS)):
                    dve = gi in DVE_GROUPS_TAIL
                    score_group(qsel[0:D, 0, 0:qlen], qlen, ptc, gi, dve)
                    if gi in (3, 6, 9, 12) and avi < len(prev_av):
                        av_subtile(*prev_av[avi])
                        avi += 1
                for a in prev_av[avi:]:
                    av_subtile(*a)

            qsel_tiles[0] = emit_gather(0)
            qsel_tiles[1] = emit_gather(1)
            emit_tail_slab(0)
            qsel_tiles[2] = emit_gather(2)
            emit_tail_slab(1)
            emit_scatter(0)
            qsel_tiles[3] = emit_gather(3)
            emit_tail_slab(2)
            emit_scatter(1)
            qsel_tiles[4] = emit_gather(4)
            emit_tail_slab(3)
            emit_scatter(2)
            qsel_tiles[5] = emit_gather(5)
            emit_tail_slab(4)
            emit_scatter(3)
            emit_tail_slab(5)
            emit_scatter(4)
            # AV for the last slab
            pq = TAIL_LENS[5]
            ptp = pt_of(5)
            for u in range((pq + 127) // 128):
                av_subtile(ptp, u, min(PART, pq - PART * u), 20 + u)
            emit_scatter(5)


# revision 31
# speedup vs baseline: 1.2539x; 1.0138x over previous
"""Trainium2 Bass kernel for nn_AttentionBlock_33724083208839 (sparse_attention).

Data-parallel over batch (8 batches -> 8 cores). Hybrid selected-only design:
  1. load x, PE-transpose -> xT32 (f32, exact) + xTb (bf16); K projected in
     exact f32 (feeds selection); Q/V projected in bf16; Q also staged
     row-major to DRAM (qdram [4104,128] bf16, padded cols + zero ghost row).
  2. selection (exact, baseline machinery, DVE+Pool split halves): K_reduce
     via CVaR identity; sqk exact on PE; threshold via two 128-ary counting
     passes on a DMA-replicated sqk + sparse_gather compaction (sentinel
     padding instead of num_found round-trips) + three compact passes.
  3. head: attention for ALL queries of slabs 0-1 (1024 queries) overlaps
     selection latency; blend non-selected head rows to meanV (Pool
     arithmetic), direct DMA out.
  4. tail: positions 1024..4095 -> compact ordered index list of selected
     queries (sparse_gather over masked iota, padded to 2240 with sentinel
     4096 by appending always-gathered sentinel values); dma_gather
     (transpose mode) pulls Q_sel^T tiles from qdram; 4.375 slabs of
     scores->exp(ACT/DVE Schraudolph)->AV; results scattered to out rows by
     indirect DMA (sentinels skipped via bounds_check); rows 1024..4095
     pre-filled with meanV by chunk DMAs before the scatters.
"""
import os
import sys

sys.path.insert(0, "/opt/trn_rl_repo")

import math
from statistics import NormalDist

KSKIP = set(os.environ.get('KSKIP', '').split(','))

import numpy as np

import concourse.bacc as bacc
import concourse.bass as bass
import concourse.bass_isa as bass_isa
import concourse.mybir as mybir
from concourse.tile import TileContext
from concourse.masks import make_identity
from concourse.bass_utils import run_bass_kernel_spmd

B, L, D = 8, 4096, 64
LQ = int((1.0 - 0.33) * L)  # 2744
PART = 128
NT = L // PART          # 32 key tiles / x chunks
N_CORES = 8

NTAIL = 2816            # padded compact count (LQ=2744 exact + sentinels)
NTC = 22                # compact chunks
TAIL_LENS = [512, 512, 512, 512, 512, 256]
SENT = L                # sentinel index -> qdram ghost row, scatter-skipped
QD_ROWS = L + 8         # qdram rows (ghost row at L)
N_VSENT = 64            # value-compaction sentinel count
N_ISENT = 592           # index-compaction sentinels (need >= NTAIL - LQ = 72)

QFRAC = 1.0 - LQ / L
Z = NormalDist().inv_cdf(QFRAC)
PHI = math.exp(-Z * Z / 2.0) / math.sqrt(2.0 * math.pi)

f32 = mybir.dt.float32
bf16 = mybir.dt.bfloat16
u8 = mybir.dt.uint8
i16 = mybir.dt.int16
i32 = mybir.dt.int32
u32 = mybir.dt.uint32
AF = mybir.ActivationFunctionType
OP = mybir.AluOpType

BOUND = 512.0

# Schraudolph exp for bf16 bit patterns: bf16_bits(exp(s/8)) ~= A*s + B.
SCH_A = 128.0 * math.log2(math.e) / 8.0
SCH_B = 16256.0 + 0.5 - 128.0 * math.log2(1.0407)

GROUPS = [(g, 2) for g in range(0, NT, 2)]

# head slabs: DVE takes the last 3 pair-groups (it is busy with selection
# until ~2/3 through each head slab); tail slabs: DVE takes 6.
DVE_GROUPS_TAIL = {3, 6, 8, 11, 14}


def build(debug: bool = False):
    nc = bacc.Bacc("TRN2")
    x = nc.dram_tensor("x", [L, D], f32, kind="ExternalInput")
    wq = nc.dram_tensor("Wq", [D, D], f32, kind="ExternalInput")
    wk = nc.dram_tensor("Wk", [D, D], f32, kind="ExternalInput")
    wv = nc.dram_tensor("Wv", [D, D], f32, kind="ExternalInput")
    out = nc.dram_tensor("out", [L + 8, D], f32, kind="ExternalOutput")
    scr_row = nc.dram_tensor("scr_row", [1, L], f32, kind="Internal")
    scr_cmp = nc.dram_tensor("scr_cmp", [1, 64], f32, kind="Internal")
    qdram = nc.dram_tensor("qdram", [QD_ROWS, PART], bf16, kind="Internal")
    scr_i32 = nc.dram_tensor("scr_i32", [1, NTC * PART], i32, kind="Internal")

    x_re = x[:].rearrange("(c p) d -> p c d", p=PART)
    # partition-contiguous x view: one 8KB/partition DMA
    x_rc = x[:].rearrange("(p c) d -> p c d", c=NT)
    out_re = out[0:L, :].rearrange("(c p) d -> p c d", p=PART)
    qd_re = qdram[0:L, :].rearrange("(c p) e -> p c e", p=PART)

    with TileContext(nc) as tc, \
         tc.tile_pool(name="cst", bufs=1) as cst, \
         tc.tile_pool(name="big", bufs=1) as big, \
         tc.tile_pool(name="sc", bufs=1) as sc, \
         tc.tile_pool(name="mn", bufs=2) as mn, \
         tc.tile_pool(name="qs", bufs=3) as qs:

        # ---- warm the exp activation table immediately ----
        warm = cst.tile([1, 8], f32)
        nc.vector.memset(warm[:], 0.0)
        warm2 = cst.tile([1, 8], f32)
        nc.scalar.activation(out=warm2[:], in_=warm[:], func=AF.Exp)

        # ---- constants ----
        ident = cst.tile([PART, PART], f32)
        make_identity(nc, ident[:])
        onesb = cst.tile([PART, 1], bf16)
        nc.vector.memset(onesb[:], 1.0)
        ones1x128 = cst.tile([1, PART], f32)
        nc.vector.memset(ones1x128[:], 1.0)
        pidx1i = cst.tile([PART, 1], i32)
        nc.gpsimd.iota(pidx1i[:], pattern=[[1, 1]], base=1, channel_multiplier=1)
        pidx1 = cst.tile([PART, 1], f32)
        nc.vector.tensor_copy(pidx1[:], pidx1i[:])
        # pass-1 thresholds are compile-time: tvec1 = pidx1*DLT1 - BOUND
        tvec1 = cst.tile([PART, 1], f32)
        nc.gpsimd.tensor_scalar(out=tvec1[:], in0=pidx1[:],
                                scalar1=2.0 * BOUND / 129.0, scalar2=-BOUND,
                                op0=OP.mult, op1=OP.add)
        # iot1[p, f] = (f*16 + p) + 1 = original position + 1, wrapped layout
        iot1i = cst.tile([16, 256], i32)
        nc.gpsimd.iota(iot1i[:], pattern=[[16, 256]], base=1, channel_multiplier=1)
        iot1 = cst.tile([16, 256], f32)
        nc.vector.tensor_copy(iot1[:], iot1i[:])

        # ---- persistent tensors ----
        x_sb = big.tile([PART, NT, D], f32)
        # x_sb is dead once the transposes finish; reuse it for res
        res = x_sb
        xT32 = big.tile([D, L], f32)
        xTb = big.tile([D, L], bf16)
        kT32 = big.tile([D, L], f32)
        kTb = big.tile([D, L], bf16)
        q_stage = big.tile([PART, NT, PART], bf16)
        vp = big.tile([PART, NT, D + 1], bf16)
        pt_a = big.tile([PART, NT, 512], bf16)
        pt_b = big.tile([PART, NT, 512], bf16)
        pt_c = big.tile([PART, NT, 512], bf16)
        mvf = big.tile([PART, D], f32)
        mvf2 = big.tile([PART, D], f32)
        sqk = big.tile([PART, NT], f32)
        kr = big.tile([D, 1], f32)
        wvec = big.tile([D, 1], f32)
        sqk_rep = big.tile([PART, L], f32)
        sqk16 = big.tile([16, 256], f32)
        valrep = big.tile([PART, 64], f32)
        cmpc = big.tile([PART, 64], bf16)
        # value compaction: input [16, 256+4] (tv || 0.0-sentinels)
        tvv = big.tile([16, 256 + N_VSENT // 16], f32)
        valc = big.tile([16, 20], f32)
        nfdummy = big.tile([1, 1], u32)
        # index compaction: input [16, 256+10] (tvidx || 4096.0-sentinels)
        tvi = big.tile([16, 256 + N_ISENT // 16], f32)
        idxw = big.tile([16, 209], f32)
        idx16w = big.tile([PART, NTC * PART // 16], i16)
        idx32w = big.tile([16, NTC * PART // 16], i32)
        idxo = big.tile([PART, NTC], i32)

        # weights
        wq_s = cst.tile([D, D], f32)
        wk_s = cst.tile([D, D], f32)
        wv_s = cst.tile([D, D], f32)
        nc.gpsimd.dma_start(out=wq_s[:], in_=wq[:])
        nc.gpsimd.dma_start(out=wk_s[:], in_=wk[:])
        nc.gpsimd.dma_start(out=wv_s[:], in_=wv[:])
        wq_b = cst.tile([D, D], bf16)
        wv_b = cst.tile([D, D], bf16)
        nc.gpsimd.tensor_copy(wq_b[:], wq_s[:])
        nc.gpsimd.tensor_copy(wv_b[:], wv_s[:])
        # sentinel regions (Pool, SBUF-only, one-time)
        nc.gpsimd.memset(tvv[:, 256:], 0.0)
        nc.gpsimd.memset(tvi[:, 256:], float(SENT))
        nc.gpsimd.memset(q_stage[:, :, D:PART], 0.0)
        # last tail chunk is half-height; zero the unwritten rows once
        nc.vector.memset(res[:, NTC - 1, :], 0.0)
        # ghost row of qdram <- zeros
        zrow = cst.tile([1, PART], bf16)
        nc.vector.memset(zrow[:], 0.0)
        nc.gpsimd.dma_start(out=qdram[L:L + 1, :], in_=zrow[:])

        kst = {}
        sel_state = {}
        DLT1 = 2.0 * BOUND / 129.0
        DLT2 = DLT1 / 129.0
        DLTC = [DLT2 / 129.0, DLT2 / 129.0 ** 2, DLT2 / 129.0 ** 3]

        # ====================== phase 1a: transposes + K ======================
        with tc.tile_pool(name="ps_xv", bufs=3, space="PSUM") as ps_xv, \
             tc.tile_pool(name="ps_pj", bufs=2, space="PSUM") as ps_pj:

            def load_tiles(c0):
                # 8 transposes into one 2-bank PSUM tile, one copy per engine
                pxt = ps_xv.tile([PART, 1024], f32, tag="xv")
                for i in range(8):
                    nc.tensor.transpose(out=pxt[0:D, PART * i:PART * (i + 1)],
                                        in_=x_sb[:, c0 + i, :], identity=ident[:])
                xv32 = xT32[:].rearrange("d (p c) -> d c p", c=NT)
                xvb = xTb[:].rearrange("d (p c) -> d c p", c=NT)
                pxt4 = pxt[0:D, :].rearrange("d (c p) -> d c p", c=8)
                nc.vector.tensor_copy(xv32[:, c0:c0 + 8, :], pxt4[:])
                nc.scalar.copy(xvb[:, c0:c0 + 8, :], pxt4[:])

            def proj_k(s):
                sl = slice(512 * s, 512 * (s + 1))
                pk = ps_pj.tile([D, 512], f32, tag="pj")
                for h in range(2):
                    nc.tensor.matmul(out=pk[:, 256 * h:256 * (h + 1)], lhsT=wk_s[:],
                                     rhs=xT32[:, 512 * s + 256 * h:512 * s + 256 * (h + 1)],
                                     start=True, stop=True)
                nc.vector.tensor_copy(kT32[:, sl], pk[:])
                nc.scalar.copy(kTb[:, sl], pk[:])

            nc.sync.dma_start(out=x_sb[:, 0:8, :], in_=x_rc[:, 0:8, :])
            nc.sync.dma_start(out=x_sb[:, 8:32, :], in_=x_rc[:, 8:32, :])
            for s in range(4):
                load_tiles(8 * s)
            for s in range(8):
                proj_k(s)

        # ====================== phase 1b: Q/V + head slab 0 =================
        with tc.tile_pool(name="ps_s0", bufs=2, space="PSUM") as ps_s0, \
             tc.tile_pool(name="ps_qv", bufs=2, space="PSUM") as ps_qv, \
             tc.tile_pool(name="ps_mis", bufs=2, space="PSUM") as ps_mis:

            def proj_qv(c0):
                # 4 chunks of Q and V into one PSUM bank (slots 0-3 Q, 4-7 V)
                pqv = ps_qv.tile([PART, 512], f32, tag="qv")
                pqv8 = pqv[:].rearrange("p (o d) -> p o d", o=8)
                for i in range(4):
                    nc.tensor.matmul(out=pqv8[:, i, :],
                                     lhsT=xTb[:, PART * (c0 + i):PART * (c0 + i + 1)],
                                     rhs=wq_b[:], start=True, stop=True)
                    nc.tensor.matmul(out=pqv8[:, 4 + i, :],
                                     lhsT=xTb[:, PART * (c0 + i):PART * (c0 + i + 1)],
                                     rhs=wv_b[:], start=True, stop=True)
                nc.scalar.copy(q_stage[:, c0:c0 + 4, 0:D], pqv8[:, 0:4, :])
                if (c0 // 4) % 2 == 0:
                    nc.vector.tensor_copy(vp[:, c0:c0 + 4, 0:D], pqv8[:, 4:8, :])
                else:
                    nc.scalar.copy(vp[:, c0:c0 + 4, 0:D], pqv8[:, 4:8, :])



            def emit_kred_a():
                kst['bstats'] = sc.tile([D, 4, 6], f32, tag="bstats", name="bstats")
                for a in range(4):
                    nc.vector.bn_stats(kst['bstats'][:, a, :],
                                       kT32[:, 1024 * a:1024 * a + 512])
                kst['aggr'] = sc.tile([D, 2], f32, tag="aggr", name="aggr")
                nc.vector.bn_aggr(kst['aggr'][:], kst['bstats'][:])
                kst['sig'] = sc.tile([D, 1], f32, tag="sig", name="sig")
                nc.vector.memset(kst['sig'][:], 1.0)
                for _ in range(2):
                    kst['rs'] = sc.tile([D, 1], f32, tag="rs", name="rs")
                    nc.vector.reciprocal(kst['rs'][:], kst['sig'][:])
                    nc.vector.tensor_tensor(out=kst['rs'][:], in0=kst['rs'][:],
                                            in1=kst['aggr'][:, 1:2], op=OP.mult)
                    nc.vector.tensor_tensor(out=kst['rs'][:], in0=kst['rs'][:],
                                            in1=kst['sig'][:], op=OP.add)
                    nc.vector.tensor_scalar_mul(kst['sig'][:], kst['rs'][:], 0.5)
                kst['tk'] = sc.tile([D, 1], f32, tag="tk", name="tk")
                nc.vector.tensor_scalar(out=kst['tk'][:], in0=kst['sig'][:],
                                        scalar1=float(Z), scalar2=None, op0=OP.mult)
                nc.gpsimd.tensor_tensor(out=kst['tk'][:], in0=kst['tk'][:],
                                        in1=kst['aggr'][:, 0:1], op=OP.add)
                HL = L // 2
                kst['cnt_c'] = sc.tile([D, 1], f32, tag="cnt_c", name="cnt_c")
                kst['cnt_p'] = sc.tile([D, 1], f32, tag="cnt_p", name="cnt_p")
                nc.vector.tensor_scalar(out=pt_c[0:D, 0:4, :], in0=kT32[:, 0:HL],
                                        scalar1=kst['tk'][:, 0:1], scalar2=None,
                                        op0=OP.is_gt, op1=OP.add,
                                        accum_out=kst['cnt_c'][:])
                nc.gpsimd.tensor_scalar(out=pt_c[0:D, 4:8, :], in0=kT32[:, HL:L],
                                        scalar1=kst['tk'][:, 0:1], scalar2=None,
                                        op0=OP.is_gt)
                nc.vector.tensor_scalar(out=pt_c[0:D, 8:12, :],
                                        in0=pt_c[0:D, 4:8, :],
                                        scalar1=1.0, scalar2=None,
                                        op0=OP.mult, op1=OP.add,
                                        accum_out=kst['cnt_p'][:])
                nc.gpsimd.tensor_tensor(out=kst['cnt_c'][:], in0=kst['cnt_c'][:],
                                        in1=kst['cnt_p'][:], op=OP.add)
                kst['adj'] = sc.tile([D, 1], f32, tag="adj", name="adj")
                nc.gpsimd.tensor_scalar(out=kst['adj'][:], in0=kst['cnt_c'][:],
                                        scalar1=float(-LQ), scalar2=1.0 / (L * PHI),
                                        op0=OP.add, op1=OP.mult)
                nc.gpsimd.tensor_tensor(out=kst['adj'][:], in0=kst['adj'][:],
                                        in1=kst['sig'][:], op=OP.mult)
                kst['t1'] = sc.tile([D, 1], f32, tag="t1", name="t1")
                nc.gpsimd.tensor_tensor(out=kst['t1'][:], in0=kst['tk'][:],
                                        in1=kst['adj'][:], op=OP.add)

            def emit_kred_b():
                HL = L // 2
                kst['s1c'] = sc.tile([D, 1], f32, tag="s1c", name="s1c")
                kst['s1p'] = sc.tile([D, 1], f32, tag="s1p", name="s1p")
                nc.vector.tensor_scalar(out=sqk_rep[0:D, 0:HL], in0=kT32[:, 0:HL],
                                        scalar1=kst['t1'][:, 0:1],
                                        scalar2=0.0, op0=OP.subtract, op1=OP.max)
                nc.gpsimd.tensor_scalar(out=sqk_rep[0:D, HL:L], in0=kT32[:, HL:L],
                                        scalar1=kst['t1'][:, 0:1],
                                        scalar2=0.0, op0=OP.subtract, op1=OP.max)
                nc.vector.tensor_reduce(out=kst['s1c'][:], in_=sqk_rep[0:D, 0:HL],
                                        axis=mybir.AxisListType.X, op=OP.add)
                nc.vector.tensor_reduce(out=kst['s1p'][:], in_=sqk_rep[0:D, HL:L],
                                        axis=mybir.AxisListType.X, op=OP.add)
                nc.gpsimd.tensor_tensor(out=kst['s1c'][:], in0=kst['s1c'][:],
                                        in1=kst['s1p'][:], op=OP.add)
                nc.gpsimd.tensor_scalar(out=kr[:], in0=kst['s1c'][:],
                                        scalar1=1.0 / LQ, scalar2=None, op0=OP.mult)
                nc.gpsimd.tensor_tensor(out=kr[:], in0=kr[:], in1=kst['t1'][:],
                                        op=OP.add)

            def emit_sqk():
                pwt = ps_mis.tile([PART, PART], f32, tag="mis", name="mis")
                nc.tensor.transpose(out=pwt[0:D, 0:D], in_=wq_s[:],
                                    identity=ident[0:D, 0:D])
                wqT = sc.tile([D, D], f32, tag="wqT")
                nc.vector.tensor_copy(wqT[:], pwt[0:D, 0:D])
                pw = ps_mis.tile([PART, PART], f32, tag="mis", name="mis")
                nc.tensor.matmul(out=pw[0:D, 0:1], lhsT=wqT[:], rhs=kr[:],
                                 start=True, stop=True)
                nc.vector.tensor_copy(wvec[:], pw[0:D, 0:1])
                psq = ps_mis.tile([PART, PART], f32, tag="mis", name="mis")
                for c in range(NT):
                    nc.tensor.matmul(out=psq[:, c:c + 1],
                                     lhsT=xT32[:, PART * c:PART * (c + 1)],
                                     rhs=wvec[:], start=True, stop=True)
                nc.vector.tensor_copy(sqk[:], psq[:, 0:NT])
                psqT = ps_mis.tile([PART, PART], f32, tag="mis", name="mis")
                nc.tensor.transpose(out=psqT[0:NT, 0:PART], in_=sqk[:],
                                    identity=ident[:])
                sqkT = sc.tile([NT, PART], f32, tag="sqkT")
                nc.vector.tensor_copy(sqkT[:], psqT[0:NT, 0:PART])
                nc.sync.dma_start(out=scr_row[:], in_=sqkT[:])
                # replicate in two halves on two queues so pass 1 starts early
                QL = L // 4
                for qq in range(4):
                    eng = nc.sync if qq % 2 == 0 else nc.gpsimd
                    eng.dma_start(
                        out=sqk_rep[:, QL * qq:QL * (qq + 1)],
                        in_=scr_row[0:1, QL * qq:QL * (qq + 1)].to_broadcast(
                            [PART, QL]))
                # prefetch the wrapped view for the compact stage
                nc.sync.dma_start(out=sqk16[:], in_=scr_row[0, :].rearrange(
                    "(f p) -> p f", p=16))

            def emit_meanv():
                pmv = ps_mis.tile([PART, PART], f32, tag="mis", name="mis")
                for c in range(NT):
                    nc.tensor.matmul(out=pmv[0:D + 1, 0:1], lhsT=vp[:, c, :],
                                     rhs=onesb[:], start=(c == 0), stop=(c == NT - 1))
                mv_col = sc.tile([D, 1], f32, tag="mv_col")
                nc.vector.tensor_scalar_mul(mv_col[:], pmv[0:D, 0:1], 1.0 / L)
                pmvT = ps_mis.tile([PART, PART], f32, tag="mis", name="mis")
                nc.tensor.transpose(out=pmvT[0:1, 0:D], in_=mv_col[:],
                                    identity=ident[0:D, 0:D])
                mv_row = sc.tile([1, D], f32, tag="mv_row")
                nc.vector.tensor_copy(mv_row[:], pmvT[0:1, 0:D])
                pmvF = ps_mis.tile([PART, PART], f32, tag="mis", name="mis")
                nc.tensor.matmul(out=pmvF[:, 0:D], lhsT=ones1x128[:], rhs=mv_row[:],
                                 start=True, stop=True)
                nc.vector.tensor_copy(mvf[:], pmvF[:, 0:D])
                # meanV pre-fill of all tail rows (before any scatter: Tile
                # orders the overlapping out-tensor writes); Pool SWDGE queue
                pass

            # ---- phase-1b emission schedule ----
            emit_kred_a()          # DVE/Pool, needs full kT32
            for g in range(8):
                proj_qv(4 * g)
            emit_kred_b()
            nc.vector.memset(vp[:, :, D:D + 1], 1.0)
            emit_sqk()
            # stage Q to DRAM in quarters (SP picks scr_row/sqk_rep first)
            for qq in range(4):
                nc.sync.dma_start(out=qd_re[:, 8 * qq:8 * (qq + 1), :],
                                  in_=q_stage[:, 8 * qq:8 * (qq + 1), :])
            emit_meanv()

        # ====================== selection pieces (emitted into phase 2) ====
        def sel_pass_init():
            lo = mn.tile([PART, 1], f32, tag="lo_a")
            nc.vector.memset(lo[:], -BOUND)
            sel_state["lo"] = lo

        def sel_pass_full(it):
            lo = sel_state["lo"]
            dlt = DLT1 if it == 0 else DLT2
            if it == 0:
                tvec = tvec1
            else:
                tvec = mn.tile([PART, 1], f32, tag=f"tv{it % 2}")
                nc.gpsimd.tensor_scalar(out=tvec[:], in0=pidx1[:], scalar1=float(dlt),
                                        scalar2=None, op0=OP.mult)
                nc.gpsimd.tensor_tensor(out=tvec[:], in0=tvec[:], in1=lo[:], op=OP.add)
            HL = L // 2
            cntq = mn.tile([PART, 1], f32, tag="cntq")
            cntp = mn.tile([PART, 1], f32, tag="cntp")
            nc.vector.tensor_scalar(out=pt_c[:, 0:4, :], in0=sqk_rep[:, 0:HL],
                                    scalar1=tvec[:, 0:1], scalar2=None,
                                    op0=OP.is_gt, op1=OP.add, accum_out=cntq[:])
            nc.gpsimd.tensor_scalar(out=pt_c[:, 4:8, :], in0=sqk_rep[:, HL:L],
                                    scalar1=tvec[:, 0:1], scalar2=None,
                                    op0=OP.is_gt)
            nc.vector.tensor_scalar(out=pt_c[:, 8:12, :], in0=pt_c[:, 4:8, :],
                                    scalar1=1.0, scalar2=None,
                                    op0=OP.mult, op1=OP.add, accum_out=cntp[:])
            nc.gpsimd.tensor_tensor(out=cntq[:], in0=cntq[:], in1=cntp[:], op=OP.add)
            sel = mn.tile([PART, 1], f32, tag="sel")
            nc.gpsimd.tensor_scalar(out=sel[:], in0=cntq[:], scalar1=float(LQ),
                                    scalar2=None, op0=OP.is_ge)
            jsr = mn.tile([PART, 1], f32, tag="jsr")
            nc.gpsimd.partition_all_reduce(jsr[:], sel[:], channels=PART,
                                           reduce_op=bass_isa.ReduceOp.add)
            nlo = mn.tile([PART, 1], f32, tag=f"lo_{'b' if it % 2 == 0 else 'a'}")
            nc.gpsimd.tensor_scalar(out=jsr[:], in0=jsr[:], scalar1=float(dlt),
                                    scalar2=None, op0=OP.mult)
            nc.gpsimd.tensor_tensor(out=nlo[:], in0=lo[:], in1=jsr[:], op=OP.add)
            sel_state["lo"] = nlo

        def sel_compact():
            lo2 = sel_state["lo"]
            hi2 = mn.tile([PART, 1], f32, tag="hi2")
            nc.gpsimd.tensor_scalar(out=hi2[:], in0=lo2[:], scalar1=float(DLT2),
                                    scalar2=None, op0=OP.add)
            # c_hi = count(sqk > hi2) exact on the [128, 32] tile
            j32 = sc.tile([PART, NT], f32, tag="j32")
            chi = sc.tile([PART, 1], f32, tag="chi")
            nc.vector.tensor_scalar(out=j32[:], in0=sqk[:],
                                    scalar1=hi2[:, 0:1], scalar2=None,
                                    op0=OP.is_gt, op1=OP.add, accum_out=chi[:])
            nc.gpsimd.partition_all_reduce(chi[:], chi[:], channels=PART,
                                           reduce_op=bass_isa.ReduceOp.add)
            rvec = sc.tile([PART, 1], f32, tag="rvec")
            nc.gpsimd.tensor_scalar(out=rvec[:], in0=chi[:], scalar1=-1.0,
                                    scalar2=float(LQ), op0=OP.mult, op1=OP.add)
            sel_state["rvec"] = rvec
            # tv = (sqk-lo2)*b + (b-1), b = (sqk <= hi2); in-interval -> v'>0
            a16 = sc.tile([16, 256], f32, tag="a16")
            nc.gpsimd.tensor_scalar(out=a16[:], in0=sqk16[:],
                                    scalar1=lo2[0:16, 0:1], scalar2=None,
                                    op0=OP.subtract)
            b16 = sc.tile([16, 256], f32, tag="b16")
            nc.gpsimd.tensor_scalar(out=b16[:], in0=sqk16[:],
                                    scalar1=hi2[0:16, 0:1], scalar2=None,
                                    op0=OP.is_le)
            nc.gpsimd.tensor_tensor(out=tvv[:, 0:256], in0=a16[:], in1=b16[:],
                                    op=OP.mult)
            nc.gpsimd.tensor_scalar(out=b16[:], in0=b16[:], scalar1=1.0,
                                    scalar2=None, op0=OP.subtract)
            nc.gpsimd.tensor_tensor(out=tvv[:, 0:256], in0=tvv[:, 0:256],
                                    in1=b16[:], op=OP.add)
            # compact; appended 0.0 sentinels guarantee the first 64 output
            # slots are always hardware-written (no num_found round-trip)
            nc.gpsimd.sparse_gather(valc[:], tvv[:], num_found=nfdummy[:])
            nc.gpsimd.dma_start(out=scr_cmp[0, :].rearrange("(f p) -> p f", p=16),
                                in_=valc[:, 0:4])
            nc.gpsimd.dma_start(out=valrep[:],
                                in_=scr_cmp[:].to_broadcast([PART, 64]))
            loc = mn.tile([PART, 1], f32, tag="loc_a")
            nc.gpsimd.memset(loc[:], 0.0)
            sel_state["loc"] = loc

        def sel_pass_c(it):
            loc, rvec = sel_state["loc"], sel_state["rvec"]
            dlt = DLTC[it]
            tvec = mn.tile([PART, 1], f32, tag=f"tvc{it % 2}")
            nc.gpsimd.tensor_scalar(out=tvec[:], in0=pidx1[:], scalar1=float(dlt),
                                    scalar2=None, op0=OP.mult)
            nc.gpsimd.tensor_tensor(out=tvec[:], in0=tvec[:], in1=loc[:], op=OP.add)
            cntq = mn.tile([PART, 1], f32, tag="cntqc")
            nc.vector.tensor_scalar(out=cmpc[:], in0=valrep[:],
                                    scalar1=tvec[:, 0:1], scalar2=None,
                                    op0=OP.is_gt, op1=OP.add, accum_out=cntq[:])
            sel = mn.tile([PART, 1], f32, tag="selc")
            nc.gpsimd.tensor_scalar(out=sel[:], in0=cntq[:],
                                    scalar1=rvec[:, 0:1], scalar2=None,
                                    op0=OP.is_ge)
            jsr = mn.tile([PART, 1], f32, tag="jsrc")
            nc.gpsimd.partition_all_reduce(jsr[:], sel[:], channels=PART,
                                           reduce_op=bass_isa.ReduceOp.add)
            nlo = mn.tile([PART, 1], f32, tag=f"loc_{'b' if it % 2 == 0 else 'a'}")
            nc.gpsimd.tensor_scalar(out=jsr[:], in0=jsr[:], scalar1=float(dlt),
                                    scalar2=None, op0=OP.mult)
            nc.gpsimd.tensor_tensor(out=nlo[:], in0=loc[:], in1=jsr[:], op=OP.add)
            sel_state["loc"] = nlo


        def emit_idx():
            # tail-selected ordered positions, sentinel-padded to NTAIL
            lo2, loc = sel_state["lo"], sel_state["loc"]
            a16 = sc.tile([16, 256], f32, tag="ai16")
            nc.gpsimd.tensor_scalar(out=a16[:], in0=sqk16[:],
                                    scalar1=lo2[0:16, 0:1], scalar2=None,
                                    op0=OP.subtract)
            b16 = sc.tile([16, 256], f32, tag="bi16")
            nc.gpsimd.tensor_scalar(out=b16[:], in0=a16[:],
                                    scalar1=loc[0:16, 0:1], scalar2=None,
                                    op0=OP.is_gt)
            nc.gpsimd.tensor_tensor(out=tvi[:, 0:256], in0=iot1[:], in1=b16[:],
                                    op=OP.mult)
            nc.gpsimd.tensor_scalar(out=tvi[:, 0:256], in0=tvi[:, 0:256],
                                    scalar1=1.0, scalar2=None, op0=OP.subtract)
            nc.gpsimd.sparse_gather(idxw[:], tvi[:], num_found=nfdummy[:])
            nc.gpsimd.tensor_copy(idx16w[0:16, :], idxw[:, 0:NTC * PART // 16])
            nc.gpsimd.tensor_copy(idx32w[:], idxw[:, 0:NTC * PART // 16])
            nc.sync.dma_start(out=scr_i32[0, :].rearrange("(s p) -> p s", p=16),
                              in_=idx32w[:])
            # replicate wrapped idx to all 8 core blocks, SBUF->SBUF
            for g in range(1, 8):
                eng = nc.sync if g % 2 == 0 else nc.gpsimd
                eng.dma_start(out=idx16w[16 * g:16 * (g + 1), :],
                              in_=idx16w[0:16, :])
            # chunk-layout scatter offsets
            nc.sync.dma_start(out=idxo[:],
                              in_=scr_i32[0, :].rearrange("(c p) -> p c", p=PART))

        # ====================== phase 2 ======================
        with tc.tile_pool(name="ps_strip", bufs=2, space="PSUM") as ps_strip, \
             tc.tile_pool(name="ps_vstrip", bufs=1, space="PSUM") as ps_vstrip, \
             tc.tile_pool(name="ps_av", bufs=2, space="PSUM") as ps_av:

            def pt_of(s):
                return (pt_a, pt_b, pt_c)[s % 3]

            def score_group(rhs_ap, qlen, ptc, gi, dve):
                g0, glen = GROUPS[gi]
                if dve:
                    strip = ps_vstrip.tile([PART, 2, 512], f32, tag="vstrip")
                else:
                    strip = ps_strip.tile([PART, 2, 512], f32, tag="strip")
                for i in range(glen):
                    j = g0 + i
                    nc.tensor.matmul(out=strip[:, i, 0:qlen],
                                     lhsT=kTb[:, PART * j:PART * (j + 1)],
                                     rhs=rhs_ap, start=True, stop=True)
                if dve:
                    nc.vector.tensor_scalar(
                        out=ptc[:, g0:g0 + glen, 0:qlen].bitcast(i16),
                        in0=strip[:, 0:glen, 0:qlen], scalar1=SCH_A, scalar2=SCH_B,
                        op0=OP.mult, op1=OP.add)
                else:
                    nc.scalar.activation(out=ptc[:, g0:g0 + glen, 0:qlen],
                                         in_=strip[:, 0:glen, 0:qlen], func=AF.Exp,
                                         scale=0.125)

            def av_subtile(ptp, u, qn, rchunk):
                # qn rows of AV for query sub-tile u of the previous slab
                av = ps_av.tile([PART, PART], f32, tag="av")
                for j in range(NT):
                    nc.tensor.matmul(out=av[0:qn, 0:D + 1],
                                     lhsT=ptp[:, j, PART * u:PART * u + qn],
                                     rhs=vp[:, j, :],
                                     start=(j == 0), stop=(j == NT - 1))
                rec = mn.tile([PART, 1], f32, tag="rec")
                nc.vector.reciprocal_approx_fast(rec[0:qn, :], av[0:qn, D:D + 1])
                nc.vector.tensor_scalar(out=res[0:qn, rchunk, :], in0=av[0:qn, 0:D],
                                        scalar1=rec[0:qn, 0:1], scalar2=None,
                                        op0=OP.mult)
                # scatter is an ADD onto the meanV fill: emit attn - meanV
                nc.vector.tensor_tensor(out=res[0:qn, rchunk, :],
                                        in0=res[0:qn, rchunk, :], in1=mvf[0:qn, :],
                                        op=OP.subtract)

            def emit_gather(ts):
                glen = 512 if ts < 5 else 256
                qsel = qs.tile([PART, 1, 512], bf16, tag="qsel")
                nc.gpsimd.dma_gather(
                    qsel[:, :, 0:glen], qdram[:],
                    idx16w[:, 32 * ts:32 * ts + glen // 16],
                    glen, glen, PART, elem_step=PART, transpose=True)
                return qsel

            def emit_scatter(ts):
                # scatter-add slab results (attn - meanV) onto the meanV fill
                if 'scat' in KSKIP:
                    return
                c0 = 4 * ts
                nq = TAIL_LENS[ts]
                nc.gpsimd.dma_scatter_add(
                    out[:], res[:, c0:c0 + (nq + 127) // 128, :],
                    idx16w[:, 32 * ts:32 * ts + (nq + 15) // 16],
                    nq, nq, D, elem_step=D)

            # ---- selection (uncontended), then gathered slabs ----
            qsel_tiles = {}
            sel_pass_init()
            sel_pass_full(0)
            sel_pass_full(1)
            # meanV fill: delay readiness past pass-2 so it cannot front-run
            # the selection-critical DMAs on the greedy scheduler
            zt = mn.tile([PART, 1], f32, tag="zt")
            nc.gpsimd.tensor_scalar(out=zt[:], in0=sel_state["lo"][:],
                                    scalar1=0.0, scalar2=None, op0=OP.mult)
            nc.gpsimd.tensor_scalar(out=mvf2[:], in0=mvf[:],
                                    scalar1=zt[:, 0:1], scalar2=None, op0=OP.add)
            if 'fill' not in KSKIP:
                nc.scalar.dma_start(
                    out=out_re[:],
                    in_=mvf2[:].rearrange("p d -> p () d").to_broadcast(
                        [PART, NT, D]))
            sel_compact()
            sel_pass_c(0)
            sel_pass_c(1)
            sel_pass_c(2)
            emit_idx()

            def emit_tail_slab(ts):
                # scores+exp for slab ts; AV of previous slab interleaved
                qlen = TAIL_LENS[ts]
                ptc = pt_of(ts)
                qsel = qsel_tiles[ts]
                prev_av = []
                if ts > 0:
                    pq = TAIL_LENS[ts - 1]
                    ptp = pt_of(ts - 1)
                    prev_av = [(ptp, u, min(PART, pq - PART * u), 4 * (ts - 1) + u)
                               for u in range((pq + 127) // 128)]
                avi = 0
                for gi in range(len(GROUPS)):
                    dve = gi in DVE_GROUPS_TAIL
                    score_group(qsel[0:D, 0, 0:qlen], qlen, ptc, gi, dve)
                    if gi in (3, 6, 9, 12) and avi < len(prev_av):
                        av_subtile(*prev_av[avi])
                        avi += 1
                for a in prev_av[avi:]:
                    av_subtile(*a)

            qsel_tiles[0] = emit_gather(0)
            qsel_tiles[1] = emit_gather(1)
            emit_tail_slab(0)
            qsel_tiles[2] = emit_gather(2)
            emit_tail_slab(1)
            emit_scatter(0)
            qsel_tiles[3] = emit_gather(3)
            emit_tail_slab(2)
            emit_scatter(1)
            qsel_tiles[4] = emit_gather(4)
            emit_tail_slab(3)
            emit_scatter(2)
            qsel_tiles[5] = emit_gather(5)
            emit_tail_slab(4)
            emit_scatter(3)
            emit_tail_slab(5)
            emit_scatter(4)
            # AV for the last slab
            pq = TAIL_LENS[5]
            ptp = pt_of(5)
            for u in range((pq + 127) // 128):
                av_subtile(ptp, u, min(PART, pq - PART * u), 20 + u)
            emit_scatter(5)
